# revision 1
# baseline (speedup 1.0000x reference)
"""Fused self-attention + LayerNorm kernel for Trainium2 (8 NeuronCores).

Problem: B=8, S=2048, D=512 dense transformer attention layer.
  q = x@Wq + bq; k = x@Wk + bk; v = x@Wv + bv
  logits = q @ k^T / sqrt(D); attn = softmax(logits)  (mask is all-ones)
  out = LayerNorm(attn @ v) * gamma + beta

Sharding: batch-data-parallel, one batch element per core, no collectives.

Per-core kernel (all matmuls bf16 with f32 PSUM accumulation):
  - host passes x pre-transposed (xT [D,S]) so no on-chip transposes of x
  - qT/kT computed directly in [D,S] layout (W as stationary operand)
  - v computed in natural [S,D] layout (xT blocks as stationary)
  - logits [sq,sk] per 128-row chunk; exp on ACT with fused row-sum
    (accum_out); no max-subtraction (logits are provably small: |l|<~2.5)
  - attn blocks transposed on the PE array (bf16, 1 cycle/row), packed
    4-per-PSUM-bank, evicted by DVE
  - attn@v accumulated over 16 sk-blocks; softmax normalization folded
    into the LayerNorm epilogue analytically
"""

import sys

import numpy as np

_BASS_REPO = "/opt/trn_rl_repo"
if _BASS_REPO not in sys.path:
    sys.path.insert(0, _BASS_REPO)

import ml_dtypes  # noqa: E402

B, S, D = 8, 2048, 512
P = 128
NC_D = D // P  # 4 contraction chunks
SEG = 512
NSEG = S // SEG  # 4 free-dim segments
NBLK = S // P  # 16 row blocks
EPS = 1e-5
BF = ml_dtypes.bfloat16

_cached_nc = None
last_results = None  # BassKernelResults of the most recent run (for test.py)


def _build_nc():
    import concourse.mybir as mybir
    from concourse import bacc
    from concourse.masks import make_identity
    from concourse.tile import TileContext

    BF16 = mybir.dt.bfloat16
    F32 = mybir.dt.float32
    Alu = mybir.AluOpType
    Act = mybir.ActivationFunctionType

    nc = bacc.Bacc("TRN2", target_bir_lowering=False, debug=False)

    xT_d = nc.declare_dram_parameter("xT", [D, S], BF16, isOutput=False)
    wq_d = nc.declare_dram_parameter("wq", [D, D], BF16, isOutput=False)
    wk_d = nc.declare_dram_parameter("wk", [D, D], BF16, isOutput=False)
    wv_d = nc.declare_dram_parameter("wv", [D, D], BF16, isOutput=False)
    bq_d = nc.declare_dram_parameter("bq", [D], F32, isOutput=False)
    bk_d = nc.declare_dram_parameter("bk", [D], F32, isOutput=False)
    bv_d = nc.declare_dram_parameter("bv", [D], F32, isOutput=False)
    gamma_d = nc.declare_dram_parameter("gamma", [D], F32, isOutput=False)
    beta_d = nc.declare_dram_parameter("beta", [D], F32, isOutput=False)
    out_d = nc.declare_dram_parameter("out", [S, D], F32, isOutput=True)

    import concourse.bass as bass

    def bcast(param_ap, parts=P):
        # [N] dram vector -> [parts, N] partition-broadcast AP
        return bass.AP(
            tensor=param_ap.tensor,
            offset=param_ap.offset,
            ap=[[0, parts]] + list(param_ap.ap),
        )

    with TileContext(nc) as tc:
        with (
            tc.tile_pool(name="pers", bufs=1) as pers,
            tc.tile_pool(name="attnp", bufs=3) as attnp,
            tc.tile_pool(name="aTp", bufs=8) as aTp,
            tc.tile_pool(name="work", bufs=3) as work,
            tc.tile_pool(name="small", bufs=4) as small,
            tc.tile_pool(name="psA", bufs=5, space="PSUM") as psA,
            tc.tile_pool(name="psB", bufs=1, space="PSUM") as psB,
            tc.tile_pool(name="psT", bufs=2, space="PSUM") as psT,
        ):
            # ---- persistent loads (per d-chunk so compute starts early;
            # ordered so the first projection's operands land first) ----
            w_sbs = {
                nm: pers.tile([P, NC_D, D], BF16, tag=nm, name=nm)
                for nm in ("wq", "wk", "wv")
            }
            xT_sb = pers.tile([P, NC_D, S], BF16, tag="xT")
            # wq first (first ldweights), then all of x (the qT/kT psum
            # groups need every d-chunk), then wk/wv (needed later). Few
            # large DMAs — each dma_start costs ~0.4us of queue overhead.
            nc.sync.dma_start(out=w_sbs["wq"][:, 0, :], in_=wq_d.ap()[0:P, :])
            nc.sync.dma_start(out=xT_sb[:, 0, :], in_=xT_d.ap()[0:P, :])
            nc.sync.dma_start(
                out=w_sbs["wq"][:, 1:, :],
                in_=wq_d.ap()[P:, :].rearrange("(c p) n -> p c n", p=P),
            )
            for c in range(1, NC_D):
                nc.sync.dma_start(
                    out=xT_sb[:, c, :], in_=xT_d.ap()[c * P : (c + 1) * P, :]
                )
            for nm, wd in (("wk", wk_d), ("wv", wv_d)):
                nc.sync.dma_start(
                    out=w_sbs[nm], in_=wd.ap().rearrange("(c p) n -> p c n", p=P)
                )
            bq_sb = pers.tile([P, NC_D], F32, tag="bq")
            nc.sync.dma_start(out=bq_sb, in_=bq_d.ap().rearrange("(c p) -> p c", p=P))
            bk_sb = pers.tile([P, NC_D], F32, tag="bk")
            nc.sync.dma_start(out=bk_sb, in_=bk_d.ap().rearrange("(c p) -> p c", p=P))
            bv_bc = pers.tile([P, D], F32, tag="bv")
            nc.sync.dma_start(out=bv_bc, in_=bcast(bv_d.ap()))
            gamma_bc = pers.tile([P, D], F32, tag="gamma")
            nc.sync.dma_start(out=gamma_bc, in_=bcast(gamma_d.ap()))
            beta_bc = pers.tile([P, D], F32, tag="beta")
            nc.sync.dma_start(out=beta_bc, in_=bcast(beta_d.ap()))
            ident = pers.tile([P, P], BF16, tag="ident")
            make_identity(nc, ident)
            eps_sb = pers.tile([P, 1], F32, tag="eps")
            nc.vector.memset(eps_sb, EPS)
            # dummy activation right at kernel start: pulls the one-time
            # 1.28us act-table load (ln+exp set) off the first eviction's
            # critical path — it runs concurrently with the input DMAs
            warm = pers.tile([P, 1], F32, tag="warm")
            nc.scalar.activation(out=warm, in_=eps_sb, func=Act.Exp)

            # ---- phase 1: projections ----
            # qT[d',s], kT[d',s]: stationary = W chunk [d, d'-block],
            # moving = xT [d, s-seg]; accumulate over 4 d-chunks.
            qT_sb = pers.tile([P, NC_D, S], BF16, tag="qT")
            kT_sb = pers.tile([P, NC_D, S], BF16, tag="kT")
            for w_sb, dst, b_sb in ((w_sbs["wq"], qT_sb, bq_sb), (w_sbs["wk"], kT_sb, bk_sb)):
                for m in range(NC_D):
                    # borrow psB's bank (idle until phase 2) for every 6th
                    # group: 6 projection groups in flight instead of 5
                    pss = [
                        (
                            psB.tile([P, D], mybir.dt.float32, tag="out", name=f"pjpb{g}")
                            if (m * NSEG + g) % 6 == 5
                            else psA.tile(
                                [P, SEG], mybir.dt.float32, tag="mm", name=f"pjps{g}"
                            )
                        )
                        for g in range(NSEG)
                    ]
                    for c in range(NC_D):
                        for g in range(NSEG):
                            nc.tensor.matmul(
                                pss[g],
                                w_sb[:, c, m * P : (m + 1) * P],
                                xT_sb[:, c, g * SEG : (g + 1) * SEG],
                                start=(c == 0),
                                stop=(c == NC_D - 1),
                            )
                    for g in range(NSEG):
                        # evict + per-partition bias + cast to bf16;
                        # alternate ACT/DVE so the post-accumulation burst
                        # drains two PSUM banks at once
                        if g % 2 == 0:
                            nc.scalar.activation(
                                out=dst[:, m, g * SEG : (g + 1) * SEG],
                                in_=pss[g],
                                func=Act.Identity,
                                bias=b_sb[:, m : m + 1],
                                scale=1.0,
                            )
                        else:
                            nc.vector.tensor_scalar(
                                out=dst[:, m, g * SEG : (g + 1) * SEG],
                                in0=pss[g],
                                scalar1=b_sb[:, m : m + 1],
                                scalar2=None,
                                op0=Alu.add,
                            )
            # v[s,d']: stationary = xT block [d, s-block], moving = Wv [d, d']
            v_sb = pers.tile([P, NBLK, D], BF16, tag="v")
            for j in range(NBLK):
                ps = psA.tile([P, D], mybir.dt.float32, tag="mm")
                for c in range(NC_D):
                    nc.tensor.matmul(
                        ps,
                        xT_sb[:, c, j * P : (j + 1) * P],
                        w_sbs["wv"][:, c, :],
                        start=(c == 0),
                        stop=(c == NC_D - 1),
                    )
                # evict + bias along free dim + cast
                nc.vector.tensor_add(v_sb[:, j, :], ps, bv_bc)

            # ---- phase 2: attention + layernorm, per 128-row q chunk ----
            # Software-pipelined: produce chunk m (logits+exp) before
            # consuming chunk m-1 (transpose, attn@v, LN epilogue), so the
            # PE never waits on the ACT exp latency.
            def produce(m):
                lps = [
                    psA.tile([P, SEG], mybir.dt.float32, tag="mm", name=f"lgps{g}")
                    for g in range(NSEG)
                ]
                attn = attnp.tile([P, S], BF16, tag="attn")
                sums4 = small.tile([P, NSEG], mybir.dt.float32, tag="sums4")
                for c in range(NC_D):
                    for g in range(NSEG):
                        nc.tensor.matmul(
                            lps[g],
                            qT_sb[:, c, m * P : (m + 1) * P],
                            kT_sb[:, c, g * SEG : (g + 1) * SEG],
                            start=(c == 0),
                            stop=(c == NC_D - 1),
                        )
                        if c == NC_D - 1:
                            # exp(logits) with fused row-sum, emitted right
                            # after each segment's accumulation completes;
                            # no max subtraction (|logits| < ~2.5 for this
                            # problem's distribution)
                            nc.scalar.activation(
                                out=attn[:, g * SEG : (g + 1) * SEG],
                                in_=lps[g],
                                func=Act.Exp,
                                accum_out=sums4[:, g : g + 1],
                            )
                return attn, sums4

            def consume(m, attn, sums4):
                out_ps = psB.tile([P, D], mybir.dt.float32, tag="out")
                for g in range(NSEG):
                    pst = psT.tile([P, 4, P], BF16, tag="pst")
                    for jj in range(4):
                        blk = g * 4 + jj
                        nc.tensor.transpose(
                            pst[:, jj, :],
                            attn[:, blk * P : (blk + 1) * P],
                            ident,
                        )
                    aT = aTp.tile([P, 4, P], BF16, tag="aT")
                    nc.vector.tensor_copy(out=aT, in_=pst)
                    for jj in range(4):
                        blk = g * 4 + jj
                        nc.tensor.matmul(
                            out_ps,
                            aT[:, jj, :],
                            v_sb[:, blk, :],
                            start=(blk == 0),
                            stop=(blk == NBLK - 1),
                        )

                # ---- epilogue: softmax normalization folded into LN ----
                # raw = attn_unnorm @ v; t = raw * r  (r = 1/sums)
                # mean(t) = r*mean(raw); var(t) = r^2*var(raw)
                # out = (raw - mean_raw) * c1 * gamma + beta,
                #   c1 = r / sqrt(r^2*var_raw + eps)
                # rstd = (r^2*var+eps)^-0.5 computed as Exp(-0.5*Ln(.)) so the
                # ACT engine stays on the single ln+exp function table (a
                # Sqrt would force a 1.3us table reload twice per chunk).
                sums = small.tile([P, 1], mybir.dt.float32, tag="sums")
                nc.vector.reduce_sum(out=sums, in_=sums4, axis=mybir.AxisListType.X)
                recip = small.tile([P, 1], mybir.dt.float32, tag="recip")
                nc.vector.reciprocal(out=recip, in_=sums)
                bst = small.tile([P, 6], mybir.dt.float32, tag="bst")
                nc.vector.bn_stats(out=bst, in_=out_ps)
                mv = small.tile([P, 2], mybir.dt.float32, tag="mv")
                nc.vector.bn_aggr(out=mv, in_=bst)
                r2 = small.tile([P, 1], mybir.dt.float32, tag="r2")
                nc.vector.tensor_scalar_mul(r2, recip, recip)
                lnv = small.tile([P, 1], mybir.dt.float32, tag="lnv")
                nc.scalar.activation(
                    out=lnv, in_=mv[:, 1:2], func=Act.Ln, bias=eps_sb, scale=r2
                )
                rstd = small.tile([P, 1], mybir.dt.float32, tag="rstd")
                nc.scalar.activation(out=rstd, in_=lnv, func=Act.Exp, scale=-0.5)
                c1 = small.tile([P, 1], mybir.dt.float32, tag="c1")
                nc.vector.tensor_scalar_mul(c1, recip, rstd)

                # Last chunk: column-split the remaining passes + output DMA
                # so the final DVE work overlaps the final DMA (tail shave).
                halves = 2 if m == NBLK - 1 else 1
                hw_ = D // halves
                for h in range(halves):
                    cols = slice(h * hw_, (h + 1) * hw_)
                    y = work.tile([P, hw_], mybir.dt.float32, tag=f"y{h}")
                    nc.vector.tensor_scalar(
                        out=y,
                        in0=out_ps[:, cols],
                        scalar1=mv[:, 0:1],
                        scalar2=c1,
                        op0=Alu.subtract,
                        op1=Alu.mult,
                    )
                    o1 = work.tile([P, hw_], mybir.dt.float32, tag=f"o1{h}")
                    nc.vector.tensor_mul(o1, y, gamma_bc[:, cols])
                    o = work.tile([P, hw_], mybir.dt.float32, tag=f"o{h}")
                    nc.vector.tensor_add(o, o1, beta_bc[:, cols])
                    nc.sync.dma_start(
                        out=out_d.ap()[m * P : (m + 1) * P, cols], in_=o
                    )

            pending = None
            for m in range(NBLK):
                produced = produce(m)
                if pending is not None:
                    consume(m - 1, *pending)
                pending = produced
            consume(NBLK - 1, *pending)

    # Force every ACT instruction onto the one table set that contains all
    # functions we use ({exp, ln, identity} ⊆ natural_log_exp_and_others).
    # The default chooser picks the FIRST set containing each function
    # (exp→set0, ln→set5), inserting a 1.28us table reload twice per
    # chunk. Entries must keep their positions (act_func_set_id is the
    # index), so unwanted sets are emptied rather than removed.
    import concourse.bacc as bacc_mod

    orig_get_tables = bacc_mod.get_activation_tables

    def pinned_tables(arch):
        out = {}
        for name, funcs in orig_get_tables(arch).items():
            out[name] = funcs if name == "natural_log_exp_and_others" else set()
        return out

    bacc_mod.get_activation_tables = pinned_tables
    try:
        nc.compile()
    finally:
        bacc_mod.get_activation_tables = orig_get_tables
    return nc


def _numpy_fallback(query, mask, Wq, bq, Wk, bk, Wv, bv, gamma, beta):
    q = query @ Wq + bq
    k = query @ Wk + bk
    v = query @ Wv + bv
    scale = 1.0 / np.sqrt(np.float32(q.shape[-1]))
    logits = np.einsum("bqd,bkd->bqk", q, k) * scale
    m = np.swapaxes(mask, 1, 2)
    logits = np.where(m, logits, np.float32(-1e9))
    logits = logits - logits.max(axis=2, keepdims=True)
    attn = np.exp(logits)
    attn = attn / attn.sum(axis=2, keepdims=True)
    out = np.einsum("bqk,bkd->bqd", attn, v)
    mu = out.mean(axis=-1, keepdims=True)
    var = out.var(axis=-1, keepdims=True)
    return (out - mu) / np.sqrt(var + 1e-5) * gamma + beta


def kernel(query, mask, Wq, bq, Wk, bk, Wv, bv, gamma, beta):
    global _cached_nc, last_results
    from concourse.bass_utils import run_bass_kernel_spmd

    query = np.asarray(query, dtype=np.float32)
    mask = np.asarray(mask)
    Wq = np.asarray(Wq, dtype=np.float32)
    Wk = np.asarray(Wk, dtype=np.float32)
    Wv = np.asarray(Wv, dtype=np.float32)
    bq = np.asarray(bq, dtype=np.float32)
    bk = np.asarray(bk, dtype=np.float32)
    bv = np.asarray(bv, dtype=np.float32)
    gamma = np.asarray(gamma, dtype=np.float32)
    beta = np.asarray(beta, dtype=np.float32)

    if not mask.all():
        # General-mask path (never hit for this problem's all-ones mask).
        return _numpy_fallback(
            query, mask, Wq, bq, Wk, bk, Wv, bv, gamma, beta
        ).astype(np.float32)

    if _cached_nc is None:
        _cached_nc = _build_nc()
    nc = _cached_nc

    c = np.float32(1.0 / np.sqrt(D))
    wq_b = (Wq * c).astype(BF)
    wk_b = Wk.astype(BF)
    wv_b = Wv.astype(BF)
    bq_s = (bq * c).astype(np.float32)

    in_maps = []
    for b in range(B):
        in_maps.append(
            {
                "xT": np.ascontiguousarray(query[b].T).astype(BF),
                "wq": wq_b,
                "wk": wk_b,
                "wv": wv_b,
                "bq": bq_s,
                "bk": bk,
                "bv": bv,
                "gamma": gamma,
                "beta": beta,
            }
        )

    res = run_bass_kernel_spmd(nc, in_maps, core_ids=list(range(B)))
    last_results = res
    out = np.stack([res.results[b]["out"] for b in range(B)], axis=0)
    return out.astype(np.float32)



# revision 3
# speedup vs baseline: 1.1109x; 1.1109x over previous
"""Fused self-attention + LayerNorm kernel for Trainium2 (8 NeuronCores).

Problem: B=8, S=2048, D=512 dense transformer attention layer.
  q = x@Wq + bq; k = x@Wk + bk; v = x@Wv + bv
  logits = q @ k^T / sqrt(D); attn = softmax(logits)  (mask is all-ones)
  out = LayerNorm(attn @ v) * gamma + beta

Sharding: batch-data-parallel, one batch element per core, no collectives.

Per-core kernel (all matmuls bf16 with f32 PSUM accumulation):
  - host passes x pre-transposed (xT [D,S]) so no on-chip transposes of x
  - qT/kT computed directly in [D,S] layout (W as stationary operand);
    projections run seg-outer so the first 512-column slab of xT is enough
    to start the PE, with DMAs ordered/split to match (wq c-pieces, then
    xT seg-0 pieces, biases, wk, the rest of xT, wv)
  - logits computed TRANSPOSED, [k,q] per 128-k-block (stationary = kT
    block, moving = qT 256-column pair-chunk): exp(logitsT) is then
    directly the stationary operand of attn@v — no PE transposes at all
  - softmax row-sums via 1-row ones-matmuls sharing the attnT stationary
    (PE hwdecode makes the extra instructions ~free); normalization is
    folded into the LayerNorm epilogue analytically
  - attn@v accumulated over 16 k-blocks into one PSUM bank per 128-row
    q-chunk; exp on ACT; no max-subtraction (|logits| < ~2.5)
  - dummy PE matmuls during the initial DMA wait ramp the tensor engine
    to full clock before real work arrives
  - last pair runs its two q-chunks back-to-back (not interleaved) so the
    first chunk's epilogue+store overlaps the second chunk's matmuls, and
    the final store is column-quartered to pipeline DVE with DMA
"""

import sys

import numpy as np

_BASS_REPO = "/opt/trn_rl_repo"
if _BASS_REPO not in sys.path:
    sys.path.insert(0, _BASS_REPO)

import ml_dtypes  # noqa: E402

B, S, D = 8, 2048, 512
P = 128
NC_D = D // P  # 4 contraction chunks
SEG = 512
NSEG = S // SEG  # 4 free-dim segments
NBLK = S // P  # 16 k blocks
QP = 256  # q columns per produce (pair of 128-row chunks)
NPAIR = S // QP  # 8
EPS = 1e-5
BF = ml_dtypes.bfloat16
WARMUP_MM = 22  # dummy PE matmuls issued during the initial DMA wait

_cached_nc = {}
last_results = None  # BassKernelResults of the most recent run (for test.py)


def _build_nc(g1b0):
    import concourse.mybir as mybir
    from concourse import bacc
    from concourse.tile import TileContext

    BF16 = mybir.dt.bfloat16
    F32 = mybir.dt.float32
    Alu = mybir.AluOpType
    Act = mybir.ActivationFunctionType

    nc = bacc.Bacc("TRN2", target_bir_lowering=False, debug=False)

    xT_d = nc.declare_dram_parameter("xT", [D, S], BF16, isOutput=False)
    wq_d = nc.declare_dram_parameter("wq", [D, D], BF16, isOutput=False)
    wk_d = nc.declare_dram_parameter("wk", [D, D], BF16, isOutput=False)
    wv_d = nc.declare_dram_parameter("wv", [D, D], BF16, isOutput=False)
    bq_d = nc.declare_dram_parameter("bq", [D], F32, isOutput=False)
    bk_d = nc.declare_dram_parameter("bk", [D], F32, isOutput=False)
    bv_d = nc.declare_dram_parameter("bv", [D], F32, isOutput=False)
    if not g1b0:
        gamma_d = nc.declare_dram_parameter("gamma", [D], F32, isOutput=False)
        beta_d = nc.declare_dram_parameter("beta", [D], F32, isOutput=False)
    out_d = nc.declare_dram_parameter("out", [S, D], F32, isOutput=True)

    import concourse.bass as bass

    def bcast(param_ap, parts=P):
        # [N] dram vector -> [parts, N] partition-broadcast AP
        return bass.AP(
            tensor=param_ap.tensor,
            offset=param_ap.offset,
            ap=[[0, parts]] + list(param_ap.ap),
        )

    with TileContext(nc) as tc:
        with (
            tc.tile_pool(name="pers", bufs=1) as pers,
            tc.tile_pool(name="attnp", bufs=2) as attnp,
            tc.tile_pool(name="work", bufs=4) as work,
            tc.tile_pool(name="small", bufs=6) as small,
            tc.tile_pool(name="psA", bufs=5, space="PSUM") as psA,
            tc.tile_pool(name="psO", bufs=2, space="PSUM") as psO,
            tc.tile_pool(name="psS", bufs=1, space="PSUM") as psS,
        ):
            # ---- persistent tiles ----
            w_sbs = {
                nm: pers.tile([P, NC_D, D], BF16, tag=nm, name=nm)
                for nm in ("wq", "wk", "wv")
            }
            xT_sb = pers.tile([P, NC_D, S], BF16, tag="xT")

            # ---- input DMAs, ordered so PE work can start ~2.5us in ----
            # wq c-pieces first (first stationaries), then the seg-0 slab of
            # xT (first movings), then biases (needed by first evictions),
            # then wk, the rest of xT, and finally wv + epilogue vectors.
            for c in range(NC_D):
                nc.sync.dma_start(
                    out=w_sbs["wq"][:, c, :], in_=wq_d.ap()[c * P : (c + 1) * P, :]
                )
            for c in range(NC_D):
                nc.sync.dma_start(
                    out=xT_sb[:, c, 0:SEG], in_=xT_d.ap()[c * P : (c + 1) * P, 0:SEG]
                )
            bq_sb = pers.tile([P, NC_D], F32, tag="bq")
            nc.sync.dma_start(out=bq_sb, in_=bq_d.ap().rearrange("(c p) -> p c", p=P))
            bk_sb = pers.tile([P, NC_D], F32, tag="bk")
            nc.sync.dma_start(out=bk_sb, in_=bk_d.ap().rearrange("(c p) -> p c", p=P))
            for c in range(NC_D):
                nc.sync.dma_start(
                    out=w_sbs["wk"][:, c, :], in_=wk_d.ap()[c * P : (c + 1) * P, :]
                )
            for c in range(NC_D):
                nc.sync.dma_start(
                    out=xT_sb[:, c, SEG:S], in_=xT_d.ap()[c * P : (c + 1) * P, SEG:S]
                )
            nc.sync.dma_start(
                out=w_sbs["wv"], in_=wv_d.ap().rearrange("(c p) n -> p c n", p=P)
            )
            bv_bc = pers.tile([P, D], F32, tag="bv")
            nc.sync.dma_start(out=bv_bc, in_=bcast(bv_d.ap()))
            if not g1b0:
                gamma_bc = pers.tile([P, D], F32, tag="gamma")
                nc.sync.dma_start(out=gamma_bc, in_=bcast(gamma_d.ap()))
                beta_bc = pers.tile([P, D], F32, tag="beta")
                nc.sync.dma_start(out=beta_bc, in_=bcast(beta_d.ap()))

            eps_sb = pers.tile([P, 1], F32, tag="eps")
            nc.vector.memset(eps_sb, EPS)
            ones_sb = pers.tile([P, 1], BF16, tag="ones")
            nc.vector.memset(ones_sb, 1.0)
            # dummy activation right at kernel start: pulls the one-time
            # 1.28us act-table load (ln+exp set) off the first eviction's
            # critical path — it runs concurrently with the input DMAs
            warm = pers.tile([P, 1], F32, tag="warm")
            nc.scalar.activation(out=warm, in_=eps_sb, func=Act.Exp)

            # PE clock warmup: the tensor engine ramps to full speed only
            # after ~3us of continuous execution. Chew through dummy 128-row
            # matmuls on a zeroed tile while the first input DMAs land.
            wz = pers.tile([P, P], BF16, tag="wz")
            nc.vector.memset(wz, 0.0)
            if WARMUP_MM:
                wps = psA.tile([P, SEG], F32, tag="mm", name="warmps")
                for _ in range(WARMUP_MM):
                    nc.tensor.matmul(wps[:, 0:P], wz, wz, start=True, stop=True)

            # ---- phase 1: projections, seg-outer ----
            # qT[d',s], kT[d',s]: stationary = W chunk [d, d'-block],
            # moving = xT [d, s-seg]; accumulate over 4 d-chunks. seg-outer
            # so only xT's first 512 columns gate the start of compute.
            qT_sb = pers.tile([P, NC_D, S], BF16, tag="qT")
            kT_sb = pers.tile([P, NC_D, S], BF16, tag="kT")
            for g in range(NSEG):
                for w_sb, dst, b_sb in (
                    (w_sbs["wq"], qT_sb, bq_sb),
                    (w_sbs["wk"], kT_sb, bk_sb),
                ):
                    pss = [
                        psA.tile([P, SEG], F32, tag="mm", name=f"pj{m}")
                        for m in range(NC_D)
                    ]
                    for c in range(NC_D):
                        for m in range(NC_D):
                            nc.tensor.matmul(
                                pss[m],
                                w_sb[:, c, m * P : (m + 1) * P],
                                xT_sb[:, c, g * SEG : (g + 1) * SEG],
                                start=(c == 0),
                                stop=(c == NC_D - 1),
                            )
                    for m in range(NC_D):
                        # evict + per-partition bias + cast to bf16;
                        # alternate ACT/DVE so two engines drain PSUM
                        if m % 2 == 0:
                            nc.scalar.activation(
                                out=dst[:, m, g * SEG : (g + 1) * SEG],
                                in_=pss[m],
                                func=Act.Identity,
                                bias=b_sb[:, m : m + 1],
                                scale=1.0,
                            )
                        else:
                            nc.vector.tensor_scalar(
                                out=dst[:, m, g * SEG : (g + 1) * SEG],
                                in0=pss[m],
                                scalar1=b_sb[:, m : m + 1],
                                scalar2=None,
                                op0=Alu.add,
                            )
            # v[s,d']: stationary = xT block [d, s-block], moving = Wv [d, d']
            v_sb = pers.tile([P, NBLK, D], BF16, tag="v")
            for j in range(NBLK):
                ps = psA.tile([P, SEG], F32, tag="mm", name="vps")
                for c in range(NC_D):
                    nc.tensor.matmul(
                        ps,
                        xT_sb[:, c, j * P : (j + 1) * P],
                        w_sbs["wv"][:, c, :],
                        start=(c == 0),
                        stop=(c == NC_D - 1),
                    )
                # evict + bias along free dim + cast
                nc.vector.tensor_add(v_sb[:, j, :], ps, bv_bc)

            # ---- phase 2: attention + layernorm, per 256-column q pair ----
            # Software-pipelined: produce pair p+1 (logitsT+exp) before
            # consuming pair p (attn@v + LN epilogue), so the PE never waits
            # on the ACT exp latency.
            def produce(p):
                # logitsT[k, q] per 128-k-block: stationary = kT block,
                # moving = qT pair-chunk. exp(logitsT) lands in attnT ready
                # to be the stationary operand of attn@v — no transposes.
                attnT = attnp.tile([P, NBLK, QP], BF16, tag="attnT")
                for kb in range(NBLK):
                    lg = psA.tile([P, SEG], F32, tag="mm", name=f"lg{kb % 5}")
                    for c in range(NC_D):
                        nc.tensor.matmul(
                            lg[:, 0:QP],
                            kT_sb[:, c, kb * P : (kb + 1) * P],
                            qT_sb[:, c, p * QP : (p + 1) * QP],
                            start=(c == 0),
                            stop=(c == NC_D - 1),
                        )
                    # no max subtraction (|logits| < ~2.5 for this problem)
                    nc.scalar.activation(
                        out=attnT[:, kb, :], in_=lg[:, 0:QP], func=Act.Exp
                    )
                return attnT

            def epilogue(p, j, out_ps, sums, split):
                # ---- softmax normalization folded into LN ----
                # raw = attn_unnorm @ v; t = raw * r  (r = 1/rowsum)
                # out = (raw - mean_raw) * c1 * gamma + beta,
                #   c1 = r / sqrt(r^2*var_raw + eps)
                # rstd = (r^2*var+eps)^-0.5 computed as Exp(-0.5*Ln(.)) so the
                # ACT engine stays on the single ln+exp function table (a
                # Sqrt would force a 1.3us table reload twice per chunk).
                recip = small.tile([P, 1], F32, tag="recip")
                nc.vector.reciprocal(out=recip, in_=sums[:, j : j + 1])
                bst = small.tile([P, 6], F32, tag="bst")
                nc.vector.bn_stats(out=bst, in_=out_ps)
                mv = small.tile([P, 2], F32, tag="mv")
                nc.vector.bn_aggr(out=mv, in_=bst)
                r2 = small.tile([P, 1], F32, tag="r2")
                nc.vector.tensor_scalar_mul(r2, recip, recip)
                lnv = small.tile([P, 1], F32, tag="lnv")
                nc.scalar.activation(
                    out=lnv, in_=mv[:, 1:2], func=Act.Ln, bias=eps_sb, scale=r2
                )
                rstd = small.tile([P, 1], F32, tag="rstd")
                nc.scalar.activation(out=rstd, in_=lnv, func=Act.Exp, scale=-0.5)
                c1 = small.tile([P, 1], F32, tag="c1")
                nc.vector.tensor_scalar_mul(c1, recip, rstd)

                row = (p * 2 + j) * P
                hw_ = D // split
                for h in range(split):
                    cols = slice(h * hw_, (h + 1) * hw_)
                    y = work.tile([P, hw_], F32, tag=f"y{h}", name=f"y{h}")
                    nc.vector.tensor_scalar(
                        out=y,
                        in0=out_ps[:, cols],
                        scalar1=mv[:, 0:1],
                        scalar2=c1,
                        op0=Alu.subtract,
                        op1=Alu.mult,
                    )
                    if not g1b0:
                        o1 = work.tile([P, hw_], F32, tag=f"o1{h}", name=f"o1{h}")
                        nc.vector.tensor_mul(o1, y, gamma_bc[:, cols])
                        y = work.tile([P, hw_], F32, tag=f"o{h}", name=f"o{h}")
                        nc.vector.tensor_add(y, o1, beta_bc[:, cols])
                    nc.sync.dma_start(out=out_d.ap()[row : row + P, cols], in_=y)

            def consume(p, attnT, last):
                outps = [
                    psO.tile([P, D], F32, tag="out", name=f"out{j}") for j in (0, 1)
                ]
                sums = psS.tile([P, 2], F32, tag="s")
                if not last:
                    for kb in range(NBLK):
                        for j in (0, 1):
                            st = attnT[:, kb, j * P : (j + 1) * P]
                            nc.tensor.matmul(
                                outps[j],
                                st,
                                v_sb[:, kb, :],
                                start=(kb == 0),
                                stop=(kb == NBLK - 1),
                            )
                            # 1-row matmul reusing the stationary: rowsum of
                            # the exact bf16 attn weights used above
                            nc.tensor.matmul(
                                sums[:, j : j + 1],
                                st,
                                ones_sb,
                                start=(kb == 0),
                                stop=(kb == NBLK - 1),
                            )
                    for j in (0, 1):
                        epilogue(p, j, outps[j], sums, split=1)
                else:
                    # tail shave: run the two q-chunks back-to-back so chunk
                    # j=1's epilogue+store overlaps chunk j=0's matmuls, and
                    # quarter the very last store to pipeline DVE with DMA
                    for j in (1, 0):
                        for kb in range(NBLK):
                            st = attnT[:, kb, j * P : (j + 1) * P]
                            nc.tensor.matmul(
                                outps[j],
                                st,
                                v_sb[:, kb, :],
                                start=(kb == 0),
                                stop=(kb == NBLK - 1),
                            )
                            nc.tensor.matmul(
                                sums[:, j : j + 1],
                                st,
                                ones_sb,
                                start=(kb == 0),
                                stop=(kb == NBLK - 1),
                            )
                        epilogue(p, j, outps[j], sums, split=(2 if j else 4))

            pending = None
            for p in range(NPAIR):
                produced = produce(p)
                if pending is not None:
                    consume(p - 1, pending, last=False)
                pending = produced
            consume(NPAIR - 1, pending, last=True)

    # Force every ACT instruction onto the one table set that contains all
    # functions we use ({exp, ln, identity} ⊆ natural_log_exp_and_others).
    # The default chooser picks the FIRST set containing each function
    # (exp→set0, ln→set5), inserting a 1.28us table reload twice per
    # chunk. Entries must keep their positions (act_func_set_id is the
    # index), so unwanted sets are emptied rather than removed.
    import concourse.bacc as bacc_mod

    orig_get_tables = bacc_mod.get_activation_tables

    def pinned_tables(arch):
        out = {}
        for name, funcs in orig_get_tables(arch).items():
            out[name] = funcs if name == "natural_log_exp_and_others" else set()
        return out

    bacc_mod.get_activation_tables = pinned_tables
    try:
        nc.compile()
    finally:
        bacc_mod.get_activation_tables = orig_get_tables
    return nc


def _numpy_fallback(query, mask, Wq, bq, Wk, bk, Wv, bv, gamma, beta):
    q = query @ Wq + bq
    k = query @ Wk + bk
    v = query @ Wv + bv
    scale = 1.0 / np.sqrt(np.float32(q.shape[-1]))
    logits = np.einsum("bqd,bkd->bqk", q, k) * scale
    m = np.swapaxes(mask, 1, 2)
    logits = np.where(m, logits, np.float32(-1e9))
    logits = logits - logits.max(axis=2, keepdims=True)
    attn = np.exp(logits)
    attn = attn / attn.sum(axis=2, keepdims=True)
    out = np.einsum("bqk,bkd->bqd", attn, v)
    mu = out.mean(axis=-1, keepdims=True)
    var = out.var(axis=-1, keepdims=True)
    return (out - mu) / np.sqrt(var + 1e-5) * gamma + beta


def kernel(query, mask, Wq, bq, Wk, bk, Wv, bv, gamma, beta):
    global last_results
    from concourse.bass_utils import run_bass_kernel_spmd

    query = np.asarray(query, dtype=np.float32)
    mask = np.asarray(mask)
    Wq = np.asarray(Wq, dtype=np.float32)
    Wk = np.asarray(Wk, dtype=np.float32)
    Wv = np.asarray(Wv, dtype=np.float32)
    bq = np.asarray(bq, dtype=np.float32)
    bk = np.asarray(bk, dtype=np.float32)
    bv = np.asarray(bv, dtype=np.float32)
    gamma = np.asarray(gamma, dtype=np.float32)
    beta = np.asarray(beta, dtype=np.float32)

    if not mask.all():
        # General-mask path (never hit for this problem's all-ones mask).
        return _numpy_fallback(
            query, mask, Wq, bq, Wk, bk, Wv, bv, gamma, beta
        ).astype(np.float32)

    g1b0 = bool((gamma == 1.0).all() and (beta == 0.0).all())
    if g1b0 not in _cached_nc:
        _cached_nc[g1b0] = _build_nc(g1b0)
    nc = _cached_nc[g1b0]

    c = np.float32(1.0 / np.sqrt(D))
    wq_b = (Wq * c).astype(BF)
    wk_b = Wk.astype(BF)
    wv_b = Wv.astype(BF)
    bq_s = (bq * c).astype(np.float32)

    in_maps = []
    for b in range(B):
        m = {
            "xT": np.ascontiguousarray(query[b].T).astype(BF),
            "wq": wq_b,
            "wk": wk_b,
            "wv": wv_b,
            "bq": bq_s,
            "bk": bk,
            "bv": bv,
        }
        if not g1b0:
            m["gamma"] = gamma
            m["beta"] = beta
        in_maps.append(m)

    res = run_bass_kernel_spmd(nc, in_maps, core_ids=list(range(B)))
    last_results = res
    out = np.stack([res.results[b]["out"] for b in range(B)], axis=0)
    return out.astype(np.float32)


# revision 12
# speedup vs baseline: 1.1181x; 1.0065x over previous
"""Fused self-attention + LayerNorm kernel for Trainium2 (8 NeuronCores).

Problem: B=8, S=2048, D=512 dense transformer attention layer.
  q = x@Wq + bq; k = x@Wk + bk; v = x@Wv + bv
  logits = q @ k^T / sqrt(D); attn = softmax(logits)  (mask is all-ones)
  out = LayerNorm(attn @ v) * gamma + beta

Sharding: batch-data-parallel, one batch element per core, no collectives.

Per-core kernel (all matmuls bf16 with f32 PSUM accumulation):
  - host passes x pre-transposed (xT [D,S]) so no on-chip transposes of x
  - qT/kT computed directly in [D,S] layout (W as stationary operand);
    projections run seg-outer so the first 512-column slab of xT is enough
    to start the PE, with DMAs ordered/split to match (wq c-pieces, then
    xT seg-0 pieces, biases, wk, the rest of xT, wv)
  - logits computed TRANSPOSED, [k,q] per 128-k-block (stationary = kT
    block, moving = qT 256-column pair-chunk): exp(logitsT) is then
    directly the stationary operand of attn@v — no PE transposes at all
  - softmax row-sums via 1-row ones-matmuls sharing the attnT stationary
    (PE hwdecode makes the extra instructions ~free); normalization is
    folded into the LayerNorm epilogue analytically
  - attn@v accumulated over 16 k-blocks into one PSUM bank per 128-row
    q-chunk; exp on ACT; no max-subtraction (|logits| < ~2.5)
  - dummy PE matmuls during the initial DMA wait ramp the tensor engine
    to full clock before real work arrives
  - last pair runs its two q-chunks back-to-back (not interleaved) so the
    first chunk's epilogue+store overlaps the second chunk's matmuls, and
    the final store is column-quartered to pipeline DVE with DMA
"""

import sys

import numpy as np

_BASS_REPO = "/opt/trn_rl_repo"
if _BASS_REPO not in sys.path:
    sys.path.insert(0, _BASS_REPO)

import ml_dtypes  # noqa: E402

B, S, D = 8, 2048, 512
P = 128
NC_D = D // P  # 4 contraction chunks
SEG = 512
NSEG = S // SEG  # 4 free-dim segments
NBLK = S // P  # 16 k blocks
QP = 256  # q columns per produce (pair of 128-row chunks)
NPAIR = S // QP  # 8
EPS = 1e-5
BF = ml_dtypes.bfloat16
WARMUP_MM = 32  # dummy PE matmuls issued during the initial DMA wait

_cached_nc = {}
last_results = None  # BassKernelResults of the most recent run (for test.py)


def _build_nc(g1b0):
    import concourse.mybir as mybir
    from concourse import bacc
    from concourse.tile import TileContext

    BF16 = mybir.dt.bfloat16
    F32 = mybir.dt.float32
    Alu = mybir.AluOpType
    Act = mybir.ActivationFunctionType

    nc = bacc.Bacc("TRN2", target_bir_lowering=False, debug=False)

    xT_d = nc.declare_dram_parameter("xT", [D, S], BF16, isOutput=False)
    wq_d = nc.declare_dram_parameter("wq", [D, D], BF16, isOutput=False)
    wk_d = nc.declare_dram_parameter("wk", [D, D], BF16, isOutput=False)
    wv_d = nc.declare_dram_parameter("wv", [D, D], BF16, isOutput=False)
    bq_d = nc.declare_dram_parameter("bq", [D], F32, isOutput=False)
    bk_d = nc.declare_dram_parameter("bk", [D], F32, isOutput=False)
    bv_d = nc.declare_dram_parameter("bv", [D], F32, isOutput=False)
    if not g1b0:
        gamma_d = nc.declare_dram_parameter("gamma", [D], F32, isOutput=False)
        beta_d = nc.declare_dram_parameter("beta", [D], F32, isOutput=False)
    out_d = nc.declare_dram_parameter("out", [S, D], F32, isOutput=True)

    import concourse.bass as bass

    def bcast(param_ap, parts=P):
        # [N] dram vector -> [parts, N] partition-broadcast AP
        return bass.AP(
            tensor=param_ap.tensor,
            offset=param_ap.offset,
            ap=[[0, parts]] + list(param_ap.ap),
        )

    with TileContext(nc) as tc:
        with (
            tc.tile_pool(name="pers", bufs=1) as pers,
            tc.tile_pool(name="attnp", bufs=2) as attnp,
            tc.tile_pool(name="work", bufs=4) as work,
            tc.tile_pool(name="small", bufs=6) as small,
            tc.tile_pool(name="psA", bufs=5, space="PSUM") as psA,
            tc.tile_pool(name="psO", bufs=2, space="PSUM") as psO,
            tc.tile_pool(name="psS", bufs=1, space="PSUM") as psS,
        ):
            # ---- persistent tiles ----
            w_sbs = {
                nm: pers.tile([P, NC_D, D], BF16, tag=nm, name=nm)
                for nm in ("wq", "wk", "wv")
            }
            xT_sb = pers.tile([P, NC_D, S], BF16, tag="xT")

            # ---- input DMAs, ordered so PE work can start ~4us in ----
            # Interleave wq c-pieces with xT seg-0 c-pieces to match the
            # first projection batch's (c-major) consumption order, then
            # biases, wk, the rest of xT, and finally wv + epilogue vectors.
            for c in range(NC_D):
                nc.sync.dma_start(
                    out=w_sbs["wq"][:, c, :], in_=wq_d.ap()[c * P : (c + 1) * P, :]
                )
                nc.sync.dma_start(
                    out=xT_sb[:, c, 0:SEG], in_=xT_d.ap()[c * P : (c + 1) * P, 0:SEG]
                )
            bq_sb = pers.tile([P, NC_D], F32, tag="bq")
            nc.sync.dma_start(out=bq_sb, in_=bq_d.ap().rearrange("(c p) -> p c", p=P))
            bk_sb = pers.tile([P, NC_D], F32, tag="bk")
            nc.sync.dma_start(out=bk_sb, in_=bk_d.ap().rearrange("(c p) -> p c", p=P))
            for c in range(NC_D):
                nc.sync.dma_start(
                    out=w_sbs["wk"][:, c, :], in_=wk_d.ap()[c * P : (c + 1) * P, :]
                )
            for c in range(NC_D):
                nc.sync.dma_start(
                    out=xT_sb[:, c, SEG:S], in_=xT_d.ap()[c * P : (c + 1) * P, SEG:S]
                )
            nc.sync.dma_start(
                out=w_sbs["wv"], in_=wv_d.ap().rearrange("(c p) n -> p c n", p=P)
            )
            bv_bc = pers.tile([P, D], F32, tag="bv")
            nc.sync.dma_start(out=bv_bc, in_=bcast(bv_d.ap()))
            if not g1b0:
                gamma_bc = pers.tile([P, D], F32, tag="gamma")
                nc.sync.dma_start(out=gamma_bc, in_=bcast(gamma_d.ap()))
                beta_bc = pers.tile([P, D], F32, tag="beta")
                nc.sync.dma_start(out=beta_bc, in_=bcast(beta_d.ap()))

            # PE clock warmup: the tensor engine ramps to full speed only
            # after ~3us of continuous execution. Chew through dummy 128-row
            # matmuls on a zeroed tile while the first input DMAs land.
            # wz's memset is the first DVE instruction so warmup starts early.
            wz = pers.tile([P, P], BF16, tag="wz")
            nc.vector.memset(wz, 0.0)
            eps_sb = pers.tile([P, 1], F32, tag="eps")
            nc.vector.memset(eps_sb, EPS)
            ones_sb = pers.tile([P, 1], BF16, tag="ones")
            nc.vector.memset(ones_sb, 1.0)
            # dummy activation right at kernel start: pulls the one-time
            # 1.28us act-table load off the first exp eviction's critical
            # path — it runs concurrently with the input DMAs
            warm = pers.tile([P, 1], F32, tag="warm")
            nc.scalar.activation(out=warm, in_=eps_sb, func=Act.Exp)

            if WARMUP_MM:
                wps = psA.tile([P, SEG], F32, tag="mm", name="warmps")
                for _ in range(WARMUP_MM):
                    nc.tensor.matmul(wps[:, 0:P], wz, wz, start=True, stop=True)

            # ---- phase 1: projections, seg-outer ----
            # qT[d',s], kT[d',s]: stationary = W chunk [d, d'-block],
            # moving = xT [d, s-seg]; accumulate over 4 d-chunks. seg-outer
            # so only xT's first 512 columns gate the start of compute.
            qT_sb = pers.tile([P, NC_D, S], BF16, tag="qT")
            kT_sb = pers.tile([P, NC_D, S], BF16, tag="kT")
            for g in range(NSEG):
                for w_sb, dst, b_sb in (
                    (w_sbs["wq"], qT_sb, bq_sb),
                    (w_sbs["wk"], kT_sb, bk_sb),
                ):
                    pss = [
                        psA.tile([P, SEG], F32, tag="mm", name=f"pj{m}")
                        for m in range(NC_D)
                    ]
                    for c in range(NC_D):
                        for m in range(NC_D):
                            nc.tensor.matmul(
                                pss[m],
                                w_sb[:, c, m * P : (m + 1) * P],
                                xT_sb[:, c, g * SEG : (g + 1) * SEG],
                                start=(c == 0),
                                stop=(c == NC_D - 1),
                            )
                    for m in range(NC_D):
                        # evict + per-partition bias + cast to bf16;
                        # alternate ACT/DVE so two engines drain PSUM
                        if m % 2 == 0:
                            nc.scalar.activation(
                                out=dst[:, m, g * SEG : (g + 1) * SEG],
                                in_=pss[m],
                                func=Act.Identity,
                                bias=b_sb[:, m : m + 1],
                                scale=1.0,
                            )
                        else:
                            nc.vector.tensor_scalar(
                                out=dst[:, m, g * SEG : (g + 1) * SEG],
                                in0=pss[m],
                                scalar1=b_sb[:, m : m + 1],
                                scalar2=None,
                                op0=Alu.add,
                            )
            # v[s,d']: stationary = xT block [d, s-block], moving = Wv [d, d']
            v_sb = pers.tile([P, NBLK, D], BF16, tag="v")
            for j in range(NBLK):
                ps = psA.tile([P, SEG], F32, tag="mm", name="vps")
                for c in range(NC_D):
                    nc.tensor.matmul(
                        ps,
                        xT_sb[:, c, j * P : (j + 1) * P],
                        w_sbs["wv"][:, c, :],
                        start=(c == 0),
                        stop=(c == NC_D - 1),
                    )
                # evict + bias along free dim + cast
                nc.vector.tensor_add(v_sb[:, j, :], ps, bv_bc)

            # ---- phase 2: attention + layernorm, per 256-column q pair ----
            # Software-pipelined: produce pair p+1 (logitsT+exp) before
            # consuming pair p (attn@v + LN epilogue), so the PE never waits
            # on the ACT exp latency.
            def produce(p):
                # logitsT[k, q] per 128-k-block: stationary = kT block,
                # moving = qT pair-chunk. exp(logitsT) lands in attnT ready
                # to be the stationary operand of attn@v — no transposes.
                attnT = attnp.tile([P, NBLK, QP], BF16, tag="attnT")
                for kb in range(NBLK):
                    lg = psA.tile([P, SEG], F32, tag="mm", name=f"lg{kb % 5}")
                    for c in range(NC_D):
                        nc.tensor.matmul(
                            lg[:, 0:QP],
                            kT_sb[:, c, kb * P : (kb + 1) * P],
                            qT_sb[:, c, p * QP : (p + 1) * QP],
                            start=(c == 0),
                            stop=(c == NC_D - 1),
                        )
                    # no max subtraction (|logits| < ~2.5 for this problem)
                    nc.scalar.activation(
                        out=attnT[:, kb, :], in_=lg[:, 0:QP], func=Act.Exp
                    )
                return attnT

            # ---- epilogue, split in two stages ----
            # softmax normalization folded into LN:
            #   raw = attn_unnorm @ v; normalized x = raw / rowsum
            #   out = (raw - mean_raw) * c1 * gamma + beta, where
            #   c1 = (var_raw + eps*rowsum^2)^-0.5
            # (equals rstd(x)/rowsum analytically; eps*rowsum^2 keeps the
            # torch eps semantics). Stage A (DVE stats) is emitted with the
            # consume; stage B (ACT rsqrt via Exp(-0.5*Ln), final pass,
            # store) is deferred until after the NEXT produce so the ACT
            # FIFO never blocks that pair's exp evictions behind a
            # DVE-dependent Ln.
            def epi_a(p, j, out_ps, sums):
                sc = small.tile([P, 1], F32, tag="sc")
                nc.vector.tensor_copy(out=sc, in_=sums[:, j : j + 1])
                bst = small.tile([P, 6], F32, tag="bst")
                nc.vector.bn_stats(out=bst, in_=out_ps)
                mv = small.tile([P, 2], F32, tag="mv")
                nc.vector.bn_aggr(out=mv, in_=bst)
                t = small.tile([P, 1], F32, tag="t")
                nc.vector.tensor_scalar(
                    out=t,
                    in0=sc,
                    scalar1=sc,
                    scalar2=float(EPS),
                    op0=Alu.mult,
                    op1=Alu.mult,
                )
                return mv, t

            def epi_b(p, j, out_ps, mv, t, split, alt_queue=False):
                # rstd = (var + eps*s^2)^-0.5 as Exp(-0.5*Ln(.)) — the ACT
                # engine stays on the single ln+exp function table (a Sqrt
                # would force a 1.3us table reload twice per chunk)
                lnv = small.tile([P, 1], F32, tag="lnv")
                nc.scalar.activation(
                    out=lnv, in_=mv[:, 1:2], func=Act.Ln, bias=t, scale=1.0
                )
                c1 = small.tile([P, 1], F32, tag="c1")
                nc.scalar.activation(out=c1, in_=lnv, func=Act.Exp, scale=-0.5)

                row = (p * 2 + j) * P
                hw_ = D // split
                for h in range(split):
                    cols = slice(h * hw_, (h + 1) * hw_)
                    y = work.tile([P, hw_], F32, tag=f"y{h}", name=f"y{h}")
                    nc.vector.tensor_scalar(
                        out=y,
                        in0=out_ps[:, cols],
                        scalar1=mv[:, 0:1],
                        scalar2=c1,
                        op0=Alu.subtract,
                        op1=Alu.mult,
                    )
                    if not g1b0:
                        o1 = work.tile([P, hw_], F32, tag=f"o1{h}", name=f"o1{h}")
                        nc.vector.tensor_mul(o1, y, gamma_bc[:, cols])
                        y = work.tile([P, hw_], F32, tag=f"o{h}", name=f"o{h}")
                        nc.vector.tensor_add(y, o1, beta_bc[:, cols])
                    # alternate trigger queues on the tail so the final
                    # stores issue in parallel instead of serializing on SP
                    eng = nc.scalar if (alt_queue and h % 2 == 1) else nc.sync
                    eng.dma_start(out=out_d.ap()[row : row + P, cols], in_=y)

            # one persistent sums bank, column-region double-buffered by pair
            # parity so consecutive pairs' rowsum accumulations never share a
            # WAR dependency on the epilogue's read
            sums_all = psS.tile([P, 4], F32, tag="s")

            def consume_mm(p, attnT, outps, sums, j):
                for kb in range(NBLK):
                    st = attnT[:, kb, j * P : (j + 1) * P]
                    nc.tensor.matmul(
                        outps[j],
                        st,
                        v_sb[:, kb, :],
                        start=(kb == 0),
                        stop=(kb == NBLK - 1),
                    )
                    # 1-row matmul reusing the stationary: rowsum of the
                    # exact bf16 attn weights used above
                    nc.tensor.matmul(
                        sums[:, j : j + 1],
                        st,
                        ones_sb,
                        start=(kb == 0),
                        stop=(kb == NBLK - 1),
                    )

            def consume_a(p, attnT):
                outps = [
                    psO.tile([P, D], F32, tag="out", name=f"out{j}") for j in (0, 1)
                ]
                sums = sums_all[:, (p % 2) * 2 : (p % 2) * 2 + 2]
                for kb in range(NBLK):
                    for j in (0, 1):
                        st = attnT[:, kb, j * P : (j + 1) * P]
                        nc.tensor.matmul(
                            outps[j],
                            st,
                            v_sb[:, kb, :],
                            start=(kb == 0),
                            stop=(kb == NBLK - 1),
                        )
                        nc.tensor.matmul(
                            sums[:, j : j + 1],
                            st,
                            ones_sb,
                            start=(kb == 0),
                            stop=(kb == NBLK - 1),
                        )
                state = []
                for j in (0, 1):
                    mv, t = epi_a(p, j, outps[j], sums)
                    state.append((outps[j], mv, t))
                return state

            pend_attn = None  # produce(p) awaiting consume
            pend_epi = None  # (p, state) awaiting epi_b
            for p in range(NPAIR):
                produced = produce(p)
                if pend_epi is not None:
                    ep, st = pend_epi
                    for j in (0, 1):
                        epi_b(ep, j, st[j][0], st[j][1], st[j][2], split=1)
                if pend_attn is not None:
                    pend_epi = (p - 1, consume_a(p - 1, pend_attn))
                pend_attn = produced
            ep, st = pend_epi
            for j in (0, 1):
                epi_b(ep, j, st[j][0], st[j][1], st[j][2], split=1)

            # last pair: run the two q-chunks back-to-back so chunk j=1's
            # full epilogue+store overlaps chunk j=0's matmuls, and split
            # the final stores across two trigger queues
            pl = NPAIR - 1
            outps = [psO.tile([P, D], F32, tag="out", name=f"lout{j}") for j in (0, 1)]
            sums = sums_all[:, (pl % 2) * 2 : (pl % 2) * 2 + 2]
            for j in (1, 0):
                consume_mm(pl, pend_attn, outps, sums, j)
                mv, t = epi_a(pl, j, outps[j], sums)
                epi_b(pl, j, outps[j], mv, t, split=2, alt_queue=(j == 0))

    # Force every ACT instruction onto the one table set that contains all
    # functions we use ({exp, ln, identity} ⊆ natural_log_exp_and_others).
    # The default chooser picks the FIRST set containing each function
    # (exp→set0, ln→set5), inserting a 1.28us table reload twice per
    # chunk. Entries must keep their positions (act_func_set_id is the
    # index), so unwanted sets are emptied rather than removed.
    import concourse.bacc as bacc_mod

    orig_get_tables = bacc_mod.get_activation_tables

    def pinned_tables(arch):
        out = {}
        for name, funcs in orig_get_tables(arch).items():
            out[name] = funcs if name == "natural_log_exp_and_others" else set()
        return out

    bacc_mod.get_activation_tables = pinned_tables
    try:
        nc.compile()
    finally:
        bacc_mod.get_activation_tables = orig_get_tables
    return nc


def _numpy_fallback(query, mask, Wq, bq, Wk, bk, Wv, bv, gamma, beta):
    q = query @ Wq + bq
    k = query @ Wk + bk
    v = query @ Wv + bv
    scale = 1.0 / np.sqrt(np.float32(q.shape[-1]))
    logits = np.einsum("bqd,bkd->bqk", q, k) * scale
    m = np.swapaxes(mask, 1, 2)
    logits = np.where(m, logits, np.float32(-1e9))
    logits = logits - logits.max(axis=2, keepdims=True)
    attn = np.exp(logits)
    attn = attn / attn.sum(axis=2, keepdims=True)
    out = np.einsum("bqk,bkd->bqd", attn, v)
    mu = out.mean(axis=-1, keepdims=True)
    var = out.var(axis=-1, keepdims=True)
    return (out - mu) / np.sqrt(var + 1e-5) * gamma + beta


def kernel(query, mask, Wq, bq, Wk, bk, Wv, bv, gamma, beta):
    global last_results
    from concourse.bass_utils import run_bass_kernel_spmd

    query = np.asarray(query, dtype=np.float32)
    mask = np.asarray(mask)
    Wq = np.asarray(Wq, dtype=np.float32)
    Wk = np.asarray(Wk, dtype=np.float32)
    Wv = np.asarray(Wv, dtype=np.float32)
    bq = np.asarray(bq, dtype=np.float32)
    bk = np.asarray(bk, dtype=np.float32)
    bv = np.asarray(bv, dtype=np.float32)
    gamma = np.asarray(gamma, dtype=np.float32)
    beta = np.asarray(beta, dtype=np.float32)

    if not mask.all():
        # General-mask path (never hit for this problem's all-ones mask).
        return _numpy_fallback(
            query, mask, Wq, bq, Wk, bk, Wv, bv, gamma, beta
        ).astype(np.float32)

    g1b0 = bool((gamma == 1.0).all() and (beta == 0.0).all())
    if g1b0 not in _cached_nc:
        _cached_nc[g1b0] = _build_nc(g1b0)
    nc = _cached_nc[g1b0]

    c = np.float32(1.0 / np.sqrt(D))
    wq_b = (Wq * c).astype(BF)
    wk_b = Wk.astype(BF)
    wv_b = Wv.astype(BF)
    bq_s = (bq * c).astype(np.float32)

    in_maps = []
    for b in range(B):
        m = {
            "xT": np.ascontiguousarray(query[b].T).astype(BF),
            "wq": wq_b,
            "wk": wk_b,
            "wv": wv_b,
            "bq": bq_s,
            "bk": bk,
            "bv": bv,
        }
        if not g1b0:
            m["gamma"] = gamma
            m["beta"] = beta
        in_maps.append(m)

    res = run_bass_kernel_spmd(nc, in_maps, core_ids=list(range(B)))
    last_results = res
    out = np.stack([res.results[b]["out"] for b in range(B)], axis=0)
    return out.astype(np.float32)


# revision 16
# speedup vs baseline: 1.1354x; 1.0155x over previous
"""Fused self-attention + LayerNorm kernel for Trainium2 (8 NeuronCores).

Problem: B=8, S=2048, D=512 dense transformer attention layer.
  q = x@Wq + bq; k = x@Wk + bk; v = x@Wv + bv
  logits = q @ k^T / sqrt(D); attn = softmax(logits)  (mask is all-ones)
  out = LayerNorm(attn @ v) * gamma + beta

Sharding: batch-data-parallel, one batch element per core, no collectives.

Per-core kernel (all matmuls bf16 with f32 PSUM accumulation):
  - host passes x pre-transposed (xT [D,S]) so no on-chip transposes of x
  - qT/kT computed directly in [D,S] layout (W as stationary operand);
    projections run seg-outer so the first 512-column slab of xT is enough
    to start the PE, with DMAs ordered/split to match (wq c-pieces, then
    xT seg-0 pieces, biases, wk, the rest of xT, wv)
  - logits computed TRANSPOSED, [k,q] per 128-k-block (stationary = kT
    block, moving = qT 256-column pair-chunk): exp(logitsT) is then
    directly the stationary operand of attn@v — no PE transposes at all
  - softmax row-sums via 1-row ones-matmuls sharing the attnT stationary
    (PE hwdecode makes the extra instructions ~free); normalization is
    folded into the LayerNorm epilogue analytically
  - attn@v accumulated over 16 k-blocks into one PSUM bank per 128-row
    q-chunk; exp on ACT; no max-subtraction (|logits| < ~2.5)
  - dummy PE matmuls during the initial DMA wait ramp the tensor engine
    to full clock before real work arrives
  - last pair runs its two q-chunks back-to-back (not interleaved) so the
    first chunk's epilogue+store overlaps the second chunk's matmuls, and
    the final store is column-quartered to pipeline DVE with DMA
"""

import sys

import numpy as np

_BASS_REPO = "/opt/trn_rl_repo"
if _BASS_REPO not in sys.path:
    sys.path.insert(0, _BASS_REPO)

import ml_dtypes  # noqa: E402

B, S, D = 8, 2048, 512
P = 128
NC_D = D // P  # 4 contraction chunks
SEG = 512
NSEG = S // SEG  # 4 free-dim segments
NBLK = S // P  # 16 k blocks
QP = 256  # q columns per produce (pair of 128-row chunks)
NPAIR = S // QP  # 8
EPS = 1e-5
BF = ml_dtypes.bfloat16
WARMUP_MM = 56  # dummy PE matmuls issued during the initial DMA wait

_cached_nc = {}
last_results = None  # BassKernelResults of the most recent run (for test.py)


def _build_nc(g1b0):
    import concourse.mybir as mybir
    from concourse import bacc
    from concourse.tile import TileContext

    BF16 = mybir.dt.bfloat16
    F32 = mybir.dt.float32
    Alu = mybir.AluOpType
    Act = mybir.ActivationFunctionType

    nc = bacc.Bacc("TRN2", target_bir_lowering=False, debug=False)

    xT_d = nc.declare_dram_parameter("xT", [D, S], BF16, isOutput=False)
    wq_d = nc.declare_dram_parameter("wq", [D, D], BF16, isOutput=False)
    wk_d = nc.declare_dram_parameter("wk", [D, D], BF16, isOutput=False)
    wv_d = nc.declare_dram_parameter("wv", [D, D], BF16, isOutput=False)
    bq_d = nc.declare_dram_parameter("bq", [D], F32, isOutput=False)
    bk_d = nc.declare_dram_parameter("bk", [D], F32, isOutput=False)
    bv_d = nc.declare_dram_parameter("bv", [D], F32, isOutput=False)
    if not g1b0:
        gamma_d = nc.declare_dram_parameter("gamma", [D], F32, isOutput=False)
        beta_d = nc.declare_dram_parameter("beta", [D], F32, isOutput=False)
    out_d = nc.declare_dram_parameter("out", [S, D], F32, isOutput=True)

    import concourse.bass as bass

    def bcast(param_ap, parts=P):
        # [N] dram vector -> [parts, N] partition-broadcast AP
        return bass.AP(
            tensor=param_ap.tensor,
            offset=param_ap.offset,
            ap=[[0, parts]] + list(param_ap.ap),
        )

    with TileContext(nc) as tc:
        with (
            tc.tile_pool(name="pers", bufs=1) as pers,
            tc.tile_pool(name="attnp", bufs=2) as attnp,
            tc.tile_pool(name="work", bufs=4) as work,
            tc.tile_pool(name="small", bufs=6) as small,
            tc.tile_pool(name="psA", bufs=5, space="PSUM") as psA,
            tc.tile_pool(name="psO", bufs=2, space="PSUM") as psO,
            tc.tile_pool(name="psS", bufs=1, space="PSUM") as psS,
        ):
            # ---- persistent tiles ----
            w_sbs = {
                nm: pers.tile([P, NC_D, D], BF16, tag=nm, name=nm)
                for nm in ("wq", "wk", "wv")
            }
            xT_sb = pers.tile([P, NC_D, S], BF16, tag="xT")

            # ---- input DMAs, ordered around the single HWDGE queue ----
            # Issue serializes at ~625ns/DMA and transfers serialize on the
            # DMA engines, so: few DMAs, ordered to match PE consumption.
            # wq's first c-chunk, then all of xT seg 0 (one rearranged DMA),
            # then the rest of wq, wk, biases, remaining xT segs, wv.
            nc.sync.dma_start(out=w_sbs["wq"][:, 0, :], in_=wq_d.ap()[0:P, :])
            nc.sync.dma_start(
                out=xT_sb[:, :, 0:SEG],
                in_=xT_d.ap()[:, 0:SEG].rearrange("(c p) n -> p c n", p=P),
            )
            for c in range(1, NC_D):
                nc.sync.dma_start(
                    out=w_sbs["wq"][:, c, :], in_=wq_d.ap()[c * P : (c + 1) * P, :]
                )
            nc.sync.dma_start(
                out=w_sbs["wk"], in_=wk_d.ap().rearrange("(c p) n -> p c n", p=P)
            )
            bq_sb = pers.tile([P, NC_D], F32, tag="bq")
            nc.sync.dma_start(out=bq_sb, in_=bq_d.ap().rearrange("(c p) -> p c", p=P))
            bk_sb = pers.tile([P, NC_D], F32, tag="bk")
            nc.sync.dma_start(out=bk_sb, in_=bk_d.ap().rearrange("(c p) -> p c", p=P))
            for g in range(1, NSEG):
                nc.sync.dma_start(
                    out=xT_sb[:, :, g * SEG : (g + 1) * SEG],
                    in_=xT_d.ap()[:, g * SEG : (g + 1) * SEG].rearrange(
                        "(c p) n -> p c n", p=P
                    ),
                )
            nc.sync.dma_start(
                out=w_sbs["wv"], in_=wv_d.ap().rearrange("(c p) n -> p c n", p=P)
            )
            bv_bc = pers.tile([P, D], F32, tag="bv")
            nc.sync.dma_start(out=bv_bc, in_=bcast(bv_d.ap()))
            if not g1b0:
                gamma_bc = pers.tile([P, D], F32, tag="gamma")
                nc.sync.dma_start(out=gamma_bc, in_=bcast(gamma_d.ap()))
                beta_bc = pers.tile([P, D], F32, tag="beta")
                nc.sync.dma_start(out=beta_bc, in_=bcast(beta_d.ap()))

            # PE clock warmup: the tensor engine ramps to full speed only
            # after ~3us of continuous execution. Chew through dummy 128-row
            # matmuls on a zeroed tile while the first input DMAs land.
            # wz's memset is the first DVE instruction so warmup starts early.
            wz = pers.tile([P, P], BF16, tag="wz")
            nc.vector.memset(wz, 0.0)
            eps_sb = pers.tile([P, 1], F32, tag="eps")
            nc.vector.memset(eps_sb, EPS)
            ones_sb = pers.tile([P, 1], BF16, tag="ones")
            nc.vector.memset(ones_sb, 1.0)
            # dummy activation right at kernel start: pulls the one-time
            # 1.28us act-table load off the first exp eviction's critical
            # path — it runs concurrently with the input DMAs
            warm = pers.tile([P, 1], F32, tag="warm")
            nc.scalar.activation(out=warm, in_=eps_sb, func=Act.Exp)

            if WARMUP_MM:
                wps = psA.tile([P, SEG], F32, tag="mm", name="warmps")
                for _ in range(WARMUP_MM):
                    nc.tensor.matmul(wps[:, 0:P], wz, wz, start=True, stop=True)

            # ---- phase 1: projections, seg-outer ----
            # qT[d',s], kT[d',s]: stationary = W chunk [d, d'-block],
            # moving = xT [d, s-seg]; accumulate over 4 d-chunks. seg-outer
            # so only xT's first 512 columns gate the start of compute.
            qT_sb = pers.tile([P, NC_D, S], BF16, tag="qT")
            kT_sb = pers.tile([P, NC_D, S], BF16, tag="kT")
            for g in range(NSEG):
                for w_sb, dst, b_sb in (
                    (w_sbs["wq"], qT_sb, bq_sb),
                    (w_sbs["wk"], kT_sb, bk_sb),
                ):
                    pss = [
                        psA.tile([P, SEG], F32, tag="mm", name=f"pj{m}")
                        for m in range(NC_D)
                    ]
                    for c in range(NC_D):
                        for m in range(NC_D):
                            nc.tensor.matmul(
                                pss[m],
                                w_sb[:, c, m * P : (m + 1) * P],
                                xT_sb[:, c, g * SEG : (g + 1) * SEG],
                                start=(c == 0),
                                stop=(c == NC_D - 1),
                            )
                    for m in range(NC_D):
                        # evict + per-partition bias + cast to bf16;
                        # alternate ACT/DVE so two engines drain PSUM
                        if m % 2 == 0:
                            nc.scalar.activation(
                                out=dst[:, m, g * SEG : (g + 1) * SEG],
                                in_=pss[m],
                                func=Act.Identity,
                                bias=b_sb[:, m : m + 1],
                                scale=1.0,
                            )
                        else:
                            nc.vector.tensor_scalar(
                                out=dst[:, m, g * SEG : (g + 1) * SEG],
                                in0=pss[m],
                                scalar1=b_sb[:, m : m + 1],
                                scalar2=None,
                                op0=Alu.add,
                            )
            # v[s,d']: stationary = xT block [d, s-block], moving = Wv [d, d']
            v_sb = pers.tile([P, NBLK, D], BF16, tag="v")
            for j in range(NBLK):
                ps = psA.tile([P, SEG], F32, tag="mm", name="vps")
                for c in range(NC_D):
                    nc.tensor.matmul(
                        ps,
                        xT_sb[:, c, j * P : (j + 1) * P],
                        w_sbs["wv"][:, c, :],
                        start=(c == 0),
                        stop=(c == NC_D - 1),
                    )
                # evict + bias along free dim + cast
                nc.vector.tensor_add(v_sb[:, j, :], ps, bv_bc)

            # ---- phase 2: attention + layernorm, per 256-column q pair ----
            # Software-pipelined: produce pair p+1 (logitsT+exp) before
            # consuming pair p (attn@v + LN epilogue), so the PE never waits
            # on the ACT exp latency.
            def produce(p):
                # logitsT[k, q] per 128-k-block: stationary = kT block,
                # moving = qT pair-chunk. exp(logitsT) lands in attnT ready
                # to be the stationary operand of attn@v — no transposes.
                attnT = attnp.tile([P, NBLK, QP], BF16, tag="attnT")
                for kb in range(NBLK):
                    lg = psA.tile([P, SEG], F32, tag="mm", name=f"lg{kb % 5}")
                    for c in range(NC_D):
                        nc.tensor.matmul(
                            lg[:, 0:QP],
                            kT_sb[:, c, kb * P : (kb + 1) * P],
                            qT_sb[:, c, p * QP : (p + 1) * QP],
                            start=(c == 0),
                            stop=(c == NC_D - 1),
                        )
                    # no max subtraction (|logits| < ~2.5 for this problem)
                    nc.scalar.activation(
                        out=attnT[:, kb, :], in_=lg[:, 0:QP], func=Act.Exp
                    )
                return attnT

            # ---- epilogue, split in two stages ----
            # softmax normalization folded into LN:
            #   raw = attn_unnorm @ v; normalized x = raw / rowsum
            #   out = (raw - mean_raw) * c1 * gamma + beta, where
            #   c1 = (var_raw + eps*rowsum^2)^-0.5
            # (equals rstd(x)/rowsum analytically; eps*rowsum^2 keeps the
            # torch eps semantics). Stage A (DVE stats) is emitted with the
            # consume; stage B (ACT rsqrt via Exp(-0.5*Ln), final pass,
            # store) is deferred until after the NEXT produce so the ACT
            # FIFO never blocks that pair's exp evictions behind a
            # DVE-dependent Ln.
            def epi_a(p, j, out_ps, sums):
                sc = small.tile([P, 1], F32, tag="sc")
                nc.vector.tensor_copy(out=sc, in_=sums[:, j : j + 1])
                bst = small.tile([P, 6], F32, tag="bst")
                nc.vector.bn_stats(out=bst, in_=out_ps)
                mv = small.tile([P, 2], F32, tag="mv")
                nc.vector.bn_aggr(out=mv, in_=bst)
                t = small.tile([P, 1], F32, tag="t")
                nc.vector.tensor_scalar(
                    out=t,
                    in0=sc,
                    scalar1=sc,
                    scalar2=float(EPS),
                    op0=Alu.mult,
                    op1=Alu.mult,
                )
                return mv, t

            def epi_b(p, j, out_ps, mv, t, split, alt_queue=False):
                # rstd = (var + eps*s^2)^-0.5 as Exp(-0.5*Ln(.)) — the ACT
                # engine stays on the single ln+exp function table (a Sqrt
                # would force a 1.3us table reload twice per chunk)
                lnv = small.tile([P, 1], F32, tag="lnv")
                nc.scalar.activation(
                    out=lnv, in_=mv[:, 1:2], func=Act.Ln, bias=t, scale=1.0
                )
                c1 = small.tile([P, 1], F32, tag="c1")
                nc.scalar.activation(out=c1, in_=lnv, func=Act.Exp, scale=-0.5)

                row = (p * 2 + j) * P
                hw_ = D // split
                for h in range(split):
                    cols = slice(h * hw_, (h + 1) * hw_)
                    y = work.tile([P, hw_], F32, tag=f"y{h}", name=f"y{h}")
                    nc.vector.tensor_scalar(
                        out=y,
                        in0=out_ps[:, cols],
                        scalar1=mv[:, 0:1],
                        scalar2=c1,
                        op0=Alu.subtract,
                        op1=Alu.mult,
                    )
                    if not g1b0:
                        o1 = work.tile([P, hw_], F32, tag=f"o1{h}", name=f"o1{h}")
                        nc.vector.tensor_mul(o1, y, gamma_bc[:, cols])
                        y = work.tile([P, hw_], F32, tag=f"o{h}", name=f"o{h}")
                        nc.vector.tensor_add(y, o1, beta_bc[:, cols])
                    # alternate trigger queues on the tail so the final
                    # stores issue in parallel instead of serializing on SP
                    eng = nc.scalar if (alt_queue and h % 2 == 1) else nc.sync
                    eng.dma_start(out=out_d.ap()[row : row + P, cols], in_=y)

            # one persistent sums bank, column-region double-buffered by pair
            # parity so consecutive pairs' rowsum accumulations never share a
            # WAR dependency on the epilogue's read
            sums_all = psS.tile([P, 4], F32, tag="s")

            def consume_mm(p, attnT, outps, sums, j):
                for kb in range(NBLK):
                    st = attnT[:, kb, j * P : (j + 1) * P]
                    nc.tensor.matmul(
                        outps[j],
                        st,
                        v_sb[:, kb, :],
                        start=(kb == 0),
                        stop=(kb == NBLK - 1),
                    )
                    # 1-row matmul reusing the stationary: rowsum of the
                    # exact bf16 attn weights used above
                    nc.tensor.matmul(
                        sums[:, j : j + 1],
                        st,
                        ones_sb,
                        start=(kb == 0),
                        stop=(kb == NBLK - 1),
                    )

            def consume_a(p, attnT):
                outps = [
                    psO.tile([P, D], F32, tag="out", name=f"out{j}") for j in (0, 1)
                ]
                sums = sums_all[:, (p % 2) * 2 : (p % 2) * 2 + 2]
                for kb in range(NBLK):
                    for j in (0, 1):
                        st = attnT[:, kb, j * P : (j + 1) * P]
                        nc.tensor.matmul(
                            outps[j],
                            st,
                            v_sb[:, kb, :],
                            start=(kb == 0),
                            stop=(kb == NBLK - 1),
                        )
                        nc.tensor.matmul(
                            sums[:, j : j + 1],
                            st,
                            ones_sb,
                            start=(kb == 0),
                            stop=(kb == NBLK - 1),
                        )
                state = []
                for j in (0, 1):
                    mv, t = epi_a(p, j, outps[j], sums)
                    state.append((outps[j], mv, t))
                return state

            pend_attn = None  # produce(p) awaiting consume
            pend_epi = None  # (p, state) awaiting epi_b
            for p in range(NPAIR):
                produced = produce(p)
                if pend_epi is not None:
                    ep, st = pend_epi
                    for j in (0, 1):
                        epi_b(ep, j, st[j][0], st[j][1], st[j][2], split=1)
                if pend_attn is not None:
                    pend_epi = (p - 1, consume_a(p - 1, pend_attn))
                pend_attn = produced
            ep, st = pend_epi
            for j in (0, 1):
                epi_b(ep, j, st[j][0], st[j][1], st[j][2], split=1)

            # last pair: accumulate into now-idle psA banks (no WAR against
            # the previous pair's psO epilogue reads), run the two q-chunks
            # back-to-back so chunk j=1's full epilogue+store overlaps chunk
            # j=0's matmuls, and column-halve j=0's accumulation so its
            # bn_stats mostly overlaps the final matmuls
            pl = NPAIR - 1
            attnT = pend_attn
            sums = sums_all[:, (pl % 2) * 2 : (pl % 2) * 2 + 2]
            lout1 = psA.tile([P, D], F32, tag="mm", name="lout1")
            consume_mm(pl, attnT, {1: lout1}, sums, 1)
            mv, t = epi_a(pl, 1, lout1, sums)
            epi_b(pl, 1, lout1, mv, t, split=2)

            lout0 = psA.tile([P, D], F32, tag="mm", name="lout0")
            HB = D // 2
            bst2 = small.tile([P, 12], F32, tag="bst2")
            for h in (0, 1):
                cols = slice(h * HB, (h + 1) * HB)
                for kb in range(NBLK):
                    st = attnT[:, kb, 0:P]
                    nc.tensor.matmul(
                        lout0[:, cols],
                        st,
                        v_sb[:, kb, cols],
                        start=(kb == 0),
                        stop=(kb == NBLK - 1),
                    )
                    if h == 0:
                        nc.tensor.matmul(
                            sums[:, 0:1],
                            st,
                            ones_sb,
                            start=(kb == 0),
                            stop=(kb == NBLK - 1),
                        )
                if h == 0:
                    sc = small.tile([P, 1], F32, tag="sc")
                    nc.vector.tensor_copy(out=sc, in_=sums[:, 0:1])
                    t = small.tile([P, 1], F32, tag="t")
                    nc.vector.tensor_scalar(
                        out=t,
                        in0=sc,
                        scalar1=sc,
                        scalar2=float(EPS),
                        op0=Alu.mult,
                        op1=Alu.mult,
                    )
                nc.vector.bn_stats(out=bst2[:, h * 6 : (h + 1) * 6], in_=lout0[:, cols])
            mv = small.tile([P, 2], F32, tag="mv")
            nc.vector.bn_aggr(out=mv, in_=bst2)
            epi_b(pl, 0, lout0, mv, t, split=2, alt_queue=True)

    # Force every ACT instruction onto the one table set that contains all
    # functions we use ({exp, ln, identity} ⊆ natural_log_exp_and_others).
    # The default chooser picks the FIRST set containing each function
    # (exp→set0, ln→set5), inserting a 1.28us table reload twice per
    # chunk. Entries must keep their positions (act_func_set_id is the
    # index), so unwanted sets are emptied rather than removed.
    import concourse.bacc as bacc_mod

    orig_get_tables = bacc_mod.get_activation_tables

    def pinned_tables(arch):
        out = {}
        for name, funcs in orig_get_tables(arch).items():
            out[name] = funcs if name == "natural_log_exp_and_others" else set()
        return out

    bacc_mod.get_activation_tables = pinned_tables
    try:
        nc.compile()
    finally:
        bacc_mod.get_activation_tables = orig_get_tables
    return nc


def _numpy_fallback(query, mask, Wq, bq, Wk, bk, Wv, bv, gamma, beta):
    q = query @ Wq + bq
    k = query @ Wk + bk
    v = query @ Wv + bv
    scale = 1.0 / np.sqrt(np.float32(q.shape[-1]))
    logits = np.einsum("bqd,bkd->bqk", q, k) * scale
    m = np.swapaxes(mask, 1, 2)
    logits = np.where(m, logits, np.float32(-1e9))
    logits = logits - logits.max(axis=2, keepdims=True)
    attn = np.exp(logits)
    attn = attn / attn.sum(axis=2, keepdims=True)
    out = np.einsum("bqk,bkd->bqd", attn, v)
    mu = out.mean(axis=-1, keepdims=True)
    var = out.var(axis=-1, keepdims=True)
    return (out - mu) / np.sqrt(var + 1e-5) * gamma + beta


def kernel(query, mask, Wq, bq, Wk, bk, Wv, bv, gamma, beta):
    global last_results
    from concourse.bass_utils import run_bass_kernel_spmd

    query = np.asarray(query, dtype=np.float32)
    mask = np.asarray(mask)
    Wq = np.asarray(Wq, dtype=np.float32)
    Wk = np.asarray(Wk, dtype=np.float32)
    Wv = np.asarray(Wv, dtype=np.float32)
    bq = np.asarray(bq, dtype=np.float32)
    bk = np.asarray(bk, dtype=np.float32)
    bv = np.asarray(bv, dtype=np.float32)
    gamma = np.asarray(gamma, dtype=np.float32)
    beta = np.asarray(beta, dtype=np.float32)

    if not mask.all():
        # General-mask path (never hit for this problem's all-ones mask).
        return _numpy_fallback(
            query, mask, Wq, bq, Wk, bk, Wv, bv, gamma, beta
        ).astype(np.float32)

    g1b0 = bool((gamma == 1.0).all() and (beta == 0.0).all())
    if g1b0 not in _cached_nc:
        _cached_nc[g1b0] = _build_nc(g1b0)
    nc = _cached_nc[g1b0]

    c = np.float32(1.0 / np.sqrt(D))
    wq_b = (Wq * c).astype(BF)
    wk_b = Wk.astype(BF)
    wv_b = Wv.astype(BF)
    bq_s = (bq * c).astype(np.float32)

    in_maps = []
    for b in range(B):
        m = {
            "xT": np.ascontiguousarray(query[b].T).astype(BF),
            "wq": wq_b,
            "wk": wk_b,
            "wv": wv_b,
            "bq": bq_s,
            "bk": bk,
            "bv": bv,
        }
        if not g1b0:
            m["gamma"] = gamma
            m["beta"] = beta
        in_maps.append(m)

    res = run_bass_kernel_spmd(nc, in_maps, core_ids=list(range(B)))
    last_results = res
    out = np.stack([res.results[b]["out"] for b in range(B)], axis=0)
    return out.astype(np.float32)


# revision 17
# speedup vs baseline: 1.1394x; 1.0035x over previous
"""Fused self-attention + LayerNorm kernel for Trainium2 (8 NeuronCores).

Problem: B=8, S=2048, D=512 dense transformer attention layer.
  q = x@Wq + bq; k = x@Wk + bk; v = x@Wv + bv
  logits = q @ k^T / sqrt(D); attn = softmax(logits)  (mask is all-ones)
  out = LayerNorm(attn @ v) * gamma + beta

Sharding: batch-data-parallel, one batch element per core, no collectives.

Per-core kernel (all matmuls bf16 with f32 PSUM accumulation):
  - host passes x pre-transposed (xT [D,S]) so no on-chip transposes of x
  - qT/kT computed directly in [D,S] layout (W as stationary operand);
    projections run seg-outer so the first 512-column slab of xT is enough
    to start the PE, with DMAs ordered/split to match (wq c-pieces, then
    xT seg-0 pieces, biases, wk, the rest of xT, wv)
  - logits computed TRANSPOSED, [k,q] per 128-k-block (stationary = kT
    block, moving = qT 256-column pair-chunk): exp(logitsT) is then
    directly the stationary operand of attn@v — no PE transposes at all
  - softmax row-sums via 1-row ones-matmuls sharing the attnT stationary
    (PE hwdecode makes the extra instructions ~free); normalization is
    folded into the LayerNorm epilogue analytically
  - attn@v accumulated over 16 k-blocks into one PSUM bank per 128-row
    q-chunk; exp on ACT; no max-subtraction (|logits| < ~2.5)
  - dummy PE matmuls during the initial DMA wait ramp the tensor engine
    to full clock before real work arrives
  - last pair runs its two q-chunks back-to-back (not interleaved) so the
    first chunk's epilogue+store overlaps the second chunk's matmuls, and
    the final store is column-quartered to pipeline DVE with DMA
"""

import sys

import numpy as np

_BASS_REPO = "/opt/trn_rl_repo"
if _BASS_REPO not in sys.path:
    sys.path.insert(0, _BASS_REPO)

import ml_dtypes  # noqa: E402

B, S, D = 8, 2048, 512
P = 128
NC_D = D // P  # 4 contraction chunks
SEG = 512
NSEG = S // SEG  # 4 free-dim segments
NBLK = S // P  # 16 k blocks
QP = 256  # q columns per produce (pair of 128-row chunks)
NPAIR = S // QP  # 8
EPS = 1e-5
BF = ml_dtypes.bfloat16
WARMUP_MM = 46  # dummy PE matmuls issued during the initial DMA wait

_cached_nc = {}
last_results = None  # BassKernelResults of the most recent run (for test.py)


def _build_nc(g1b0):
    import concourse.mybir as mybir
    from concourse import bacc
    from concourse.tile import TileContext

    BF16 = mybir.dt.bfloat16
    F32 = mybir.dt.float32
    Alu = mybir.AluOpType
    Act = mybir.ActivationFunctionType

    nc = bacc.Bacc("TRN2", target_bir_lowering=False, debug=False)

    xT_d = nc.declare_dram_parameter("xT", [D, S], BF16, isOutput=False)
    wq_d = nc.declare_dram_parameter("wq", [D, D], BF16, isOutput=False)
    wk_d = nc.declare_dram_parameter("wk", [D, D], BF16, isOutput=False)
    wv_d = nc.declare_dram_parameter("wv", [D, D], BF16, isOutput=False)
    bq_d = nc.declare_dram_parameter("bq", [D], F32, isOutput=False)
    bk_d = nc.declare_dram_parameter("bk", [D], F32, isOutput=False)
    bv_d = nc.declare_dram_parameter("bv", [D], F32, isOutput=False)
    if not g1b0:
        gamma_d = nc.declare_dram_parameter("gamma", [D], F32, isOutput=False)
        beta_d = nc.declare_dram_parameter("beta", [D], F32, isOutput=False)
    out_d = nc.declare_dram_parameter("out", [S, D], F32, isOutput=True)

    import concourse.bass as bass

    def bcast(param_ap, parts=P):
        # [N] dram vector -> [parts, N] partition-broadcast AP
        return bass.AP(
            tensor=param_ap.tensor,
            offset=param_ap.offset,
            ap=[[0, parts]] + list(param_ap.ap),
        )

    with TileContext(nc) as tc:
        with (
            tc.tile_pool(name="pers", bufs=1) as pers,
            tc.tile_pool(name="attnp", bufs=2) as attnp,
            tc.tile_pool(name="work", bufs=4) as work,
            tc.tile_pool(name="small", bufs=6) as small,
            tc.tile_pool(name="psA", bufs=5, space="PSUM") as psA,
            tc.tile_pool(name="psO", bufs=2, space="PSUM") as psO,
            tc.tile_pool(name="psS", bufs=1, space="PSUM") as psS,
        ):
            # ---- persistent tiles ----
            w_sbs = {
                nm: pers.tile([P, NC_D, D], BF16, tag=nm, name=nm)
                for nm in ("wq", "wk", "wv")
            }
            xT_sb = pers.tile([P, NC_D, S], BF16, tag="xT")

            # ---- input DMAs, ordered around the single HWDGE queue ----
            # Issue serializes at ~625ns/DMA and transfers serialize on the
            # DMA engines, so: few DMAs, ordered to match PE consumption.
            # wq's first c-chunk, then all of xT seg 0 (one rearranged DMA),
            # then the rest of wq, wk, biases, remaining xT segs, wv.
            nc.sync.dma_start(out=w_sbs["wq"][:, 0, :], in_=wq_d.ap()[0:P, :])
            nc.sync.dma_start(
                out=xT_sb[:, :, 0:SEG],
                in_=xT_d.ap()[:, 0:SEG].rearrange("(c p) n -> p c n", p=P),
            )
            for c in range(1, NC_D):
                nc.sync.dma_start(
                    out=w_sbs["wq"][:, c, :], in_=wq_d.ap()[c * P : (c + 1) * P, :]
                )
            nc.sync.dma_start(
                out=w_sbs["wk"], in_=wk_d.ap().rearrange("(c p) n -> p c n", p=P)
            )
            bq_sb = pers.tile([P, NC_D], F32, tag="bq")
            nc.sync.dma_start(out=bq_sb, in_=bq_d.ap().rearrange("(c p) -> p c", p=P))
            bk_sb = pers.tile([P, NC_D], F32, tag="bk")
            nc.sync.dma_start(out=bk_sb, in_=bk_d.ap().rearrange("(c p) -> p c", p=P))
            for g in range(1, NSEG):
                nc.sync.dma_start(
                    out=xT_sb[:, :, g * SEG : (g + 1) * SEG],
                    in_=xT_d.ap()[:, g * SEG : (g + 1) * SEG].rearrange(
                        "(c p) n -> p c n", p=P
                    ),
                )
            nc.sync.dma_start(
                out=w_sbs["wv"], in_=wv_d.ap().rearrange("(c p) n -> p c n", p=P)
            )
            bv_bc = pers.tile([P, D], F32, tag="bv")
            nc.sync.dma_start(out=bv_bc, in_=bcast(bv_d.ap()))
            if not g1b0:
                gamma_bc = pers.tile([P, D], F32, tag="gamma")
                nc.sync.dma_start(out=gamma_bc, in_=bcast(gamma_d.ap()))
                beta_bc = pers.tile([P, D], F32, tag="beta")
                nc.sync.dma_start(out=beta_bc, in_=bcast(beta_d.ap()))

            # PE clock warmup: the tensor engine ramps to full speed only
            # after ~3us of continuous execution. Chew through dummy 128-row
            # matmuls on a zeroed tile while the first input DMAs land.
            # wz's memset is the first DVE instruction so warmup starts early.
            wz = pers.tile([P, P], BF16, tag="wz")
            nc.vector.memset(wz, 0.0)
            eps_sb = pers.tile([P, 1], F32, tag="eps")
            nc.vector.memset(eps_sb, EPS)
            ones_sb = pers.tile([P, 1], BF16, tag="ones")
            nc.vector.memset(ones_sb, 1.0)
            # dummy activation right at kernel start: pulls the one-time
            # 1.28us act-table load off the first exp eviction's critical
            # path — it runs concurrently with the input DMAs
            warm = pers.tile([P, 1], F32, tag="warm")
            nc.scalar.activation(out=warm, in_=eps_sb, func=Act.Exp)

            if WARMUP_MM:
                wps = psA.tile([P, SEG], F32, tag="mm", name="warmps")
                for _ in range(WARMUP_MM):
                    nc.tensor.matmul(wps[:, 0:P], wz, wz, start=True, stop=True)

            # ---- phase 1: projections, seg-outer ----
            # qT[d',s], kT[d',s]: stationary = W chunk [d, d'-block],
            # moving = xT [d, s-seg]; accumulate over 4 d-chunks. seg-outer
            # so only xT's first 512 columns gate the start of compute.
            qT_sb = pers.tile([P, NC_D, S], BF16, tag="qT")
            kT_sb = pers.tile([P, NC_D, S], BF16, tag="kT")
            for g in range(NSEG):
                for w_sb, dst, b_sb in (
                    (w_sbs["wq"], qT_sb, bq_sb),
                    (w_sbs["wk"], kT_sb, bk_sb),
                ):
                    pss = [
                        psA.tile([P, SEG], F32, tag="mm", name=f"pj{m}")
                        for m in range(NC_D)
                    ]
                    for c in range(NC_D):
                        for m in range(NC_D):
                            nc.tensor.matmul(
                                pss[m],
                                w_sb[:, c, m * P : (m + 1) * P],
                                xT_sb[:, c, g * SEG : (g + 1) * SEG],
                                start=(c == 0),
                                stop=(c == NC_D - 1),
                            )
                    for m in range(NC_D):
                        # evict + per-partition bias + cast to bf16;
                        # alternate ACT/DVE so two engines drain PSUM
                        if m % 2 == 0:
                            nc.scalar.activation(
                                out=dst[:, m, g * SEG : (g + 1) * SEG],
                                in_=pss[m],
                                func=Act.Identity,
                                bias=b_sb[:, m : m + 1],
                                scale=1.0,
                            )
                        else:
                            nc.vector.tensor_scalar(
                                out=dst[:, m, g * SEG : (g + 1) * SEG],
                                in0=pss[m],
                                scalar1=b_sb[:, m : m + 1],
                                scalar2=None,
                                op0=Alu.add,
                            )
            # v[s,d']: stationary = xT block [d, s-block], moving = Wv [d, d']
            v_sb = pers.tile([P, NBLK, D], BF16, tag="v")
            for j in range(NBLK):
                ps = psA.tile([P, SEG], F32, tag="mm", name="vps")
                for c in range(NC_D):
                    nc.tensor.matmul(
                        ps,
                        xT_sb[:, c, j * P : (j + 1) * P],
                        w_sbs["wv"][:, c, :],
                        start=(c == 0),
                        stop=(c == NC_D - 1),
                    )
                # evict + bias along free dim + cast
                nc.vector.tensor_add(v_sb[:, j, :], ps, bv_bc)

            # ---- phase 2: attention + layernorm, per 256-column q pair ----
            # Software-pipelined: produce pair p+1 (logitsT+exp) before
            # consuming pair p (attn@v + LN epilogue), so the PE never waits
            # on the ACT exp latency.
            def produce(p):
                # logitsT[k, q] per 128-k-block: stationary = kT block,
                # moving = qT pair-chunk. exp(logitsT) lands in attnT ready
                # to be the stationary operand of attn@v — no transposes.
                attnT = attnp.tile([P, NBLK, QP], BF16, tag="attnT")
                for kb in range(NBLK):
                    lg = psA.tile([P, SEG], F32, tag="mm", name=f"lg{kb % 5}")
                    for c in range(NC_D):
                        nc.tensor.matmul(
                            lg[:, 0:QP],
                            kT_sb[:, c, kb * P : (kb + 1) * P],
                            qT_sb[:, c, p * QP : (p + 1) * QP],
                            start=(c == 0),
                            stop=(c == NC_D - 1),
                        )
                    # no max subtraction (|logits| < ~2.5 for this problem)
                    nc.scalar.activation(
                        out=attnT[:, kb, :], in_=lg[:, 0:QP], func=Act.Exp
                    )
                return attnT

            # ---- epilogue, split in two stages ----
            # softmax normalization folded into LN:
            #   raw = attn_unnorm @ v; normalized x = raw / rowsum
            #   out = (raw - mean_raw) * c1 * gamma + beta, where
            #   c1 = (var_raw + eps*rowsum^2)^-0.5
            # (equals rstd(x)/rowsum analytically; eps*rowsum^2 keeps the
            # torch eps semantics). Stage A (DVE stats) is emitted with the
            # consume; stage B (ACT rsqrt via Exp(-0.5*Ln), final pass,
            # store) is deferred until after the NEXT produce so the ACT
            # FIFO never blocks that pair's exp evictions behind a
            # DVE-dependent Ln.
            def epi_a(p, j, out_ps, sums):
                sc = small.tile([P, 1], F32, tag="sc")
                nc.vector.tensor_copy(out=sc, in_=sums[:, j : j + 1])
                bst = small.tile([P, 6], F32, tag="bst")
                nc.vector.bn_stats(out=bst, in_=out_ps)
                mv = small.tile([P, 2], F32, tag="mv")
                nc.vector.bn_aggr(out=mv, in_=bst)
                t = small.tile([P, 1], F32, tag="t")
                nc.vector.tensor_scalar(
                    out=t,
                    in0=sc,
                    scalar1=sc,
                    scalar2=float(EPS),
                    op0=Alu.mult,
                    op1=Alu.mult,
                )
                return mv, t

            def epi_b(p, j, out_ps, mv, t, split, alt_queue=False):
                # rstd = (var + eps*s^2)^-0.5 as Exp(-0.5*Ln(.)) — the ACT
                # engine stays on the single ln+exp function table (a Sqrt
                # would force a 1.3us table reload twice per chunk)
                lnv = small.tile([P, 1], F32, tag="lnv")
                nc.scalar.activation(
                    out=lnv, in_=mv[:, 1:2], func=Act.Ln, bias=t, scale=1.0
                )
                c1 = small.tile([P, 1], F32, tag="c1")
                nc.scalar.activation(out=c1, in_=lnv, func=Act.Exp, scale=-0.5)

                row = (p * 2 + j) * P
                hw_ = D // split
                for h in range(split):
                    cols = slice(h * hw_, (h + 1) * hw_)
                    y = work.tile([P, hw_], F32, tag=f"y{h}", name=f"y{h}")
                    nc.vector.tensor_scalar(
                        out=y,
                        in0=out_ps[:, cols],
                        scalar1=mv[:, 0:1],
                        scalar2=c1,
                        op0=Alu.subtract,
                        op1=Alu.mult,
                    )
                    if not g1b0:
                        o1 = work.tile([P, hw_], F32, tag=f"o1{h}", name=f"o1{h}")
                        nc.vector.tensor_mul(o1, y, gamma_bc[:, cols])
                        y = work.tile([P, hw_], F32, tag=f"o{h}", name=f"o{h}")
                        nc.vector.tensor_add(y, o1, beta_bc[:, cols])
                    # alternate trigger queues on the tail so the final
                    # stores issue in parallel instead of serializing on SP
                    eng = nc.scalar if (alt_queue and h % 2 == 1) else nc.sync
                    eng.dma_start(out=out_d.ap()[row : row + P, cols], in_=y)

            # one persistent sums bank, column-region double-buffered by pair
            # parity so consecutive pairs' rowsum accumulations never share a
            # WAR dependency on the epilogue's read
            sums_all = psS.tile([P, 4], F32, tag="s")

            def consume_mm(p, attnT, outps, sums, j):
                for kb in range(NBLK):
                    st = attnT[:, kb, j * P : (j + 1) * P]
                    nc.tensor.matmul(
                        outps[j],
                        st,
                        v_sb[:, kb, :],
                        start=(kb == 0),
                        stop=(kb == NBLK - 1),
                    )
                    # 1-row matmul reusing the stationary: rowsum of the
                    # exact bf16 attn weights used above
                    nc.tensor.matmul(
                        sums[:, j : j + 1],
                        st,
                        ones_sb,
                        start=(kb == 0),
                        stop=(kb == NBLK - 1),
                    )

            def consume_a(p, attnT):
                outps = [
                    psO.tile([P, D], F32, tag="out", name=f"out{j}") for j in (0, 1)
                ]
                sums = sums_all[:, (p % 2) * 2 : (p % 2) * 2 + 2]
                for kb in range(NBLK):
                    for j in (0, 1):
                        st = attnT[:, kb, j * P : (j + 1) * P]
                        nc.tensor.matmul(
                            outps[j],
                            st,
                            v_sb[:, kb, :],
                            start=(kb == 0),
                            stop=(kb == NBLK - 1),
                        )
                        nc.tensor.matmul(
                            sums[:, j : j + 1],
                            st,
                            ones_sb,
                            start=(kb == 0),
                            stop=(kb == NBLK - 1),
                        )
                state = []
                for j in (0, 1):
                    mv, t = epi_a(p, j, outps[j], sums)
                    state.append((outps[j], mv, t))
                return state

            pend_attn = None  # produce(p) awaiting consume
            pend_epi = None  # (p, state) awaiting epi_b
            for p in range(NPAIR):
                produced = produce(p)
                if pend_epi is not None:
                    ep, st = pend_epi
                    for j in (0, 1):
                        epi_b(ep, j, st[j][0], st[j][1], st[j][2], split=1)
                if pend_attn is not None:
                    pend_epi = (p - 1, consume_a(p - 1, pend_attn))
                pend_attn = produced
            ep, st = pend_epi
            for j in (0, 1):
                epi_b(ep, j, st[j][0], st[j][1], st[j][2], split=1)

            # last pair: accumulate into now-idle psA banks (no WAR against
            # the previous pair's psO epilogue reads), run the two q-chunks
            # back-to-back so chunk j=1's full epilogue+store overlaps chunk
            # j=0's matmuls, and column-halve j=0's accumulation so its
            # bn_stats mostly overlaps the final matmuls
            pl = NPAIR - 1
            attnT = pend_attn
            sums = sums_all[:, (pl % 2) * 2 : (pl % 2) * 2 + 2]
            lout1 = psA.tile([P, D], F32, tag="mm", name="lout1")
            consume_mm(pl, attnT, {1: lout1}, sums, 1)
            mv, t = epi_a(pl, 1, lout1, sums)
            epi_b(pl, 1, lout1, mv, t, split=2)

            lout0 = psA.tile([P, D], F32, tag="mm", name="lout0")
            HB = D // 2
            bst2 = small.tile([P, 12], F32, tag="bst2")
            for h in (0, 1):
                cols = slice(h * HB, (h + 1) * HB)
                for kb in range(NBLK):
                    st = attnT[:, kb, 0:P]
                    nc.tensor.matmul(
                        lout0[:, cols],
                        st,
                        v_sb[:, kb, cols],
                        start=(kb == 0),
                        stop=(kb == NBLK - 1),
                    )
                    if h == 0:
                        nc.tensor.matmul(
                            sums[:, 0:1],
                            st,
                            ones_sb,
                            start=(kb == 0),
                            stop=(kb == NBLK - 1),
                        )
                if h == 0:
                    sc = small.tile([P, 1], F32, tag="sc")
                    nc.vector.tensor_copy(out=sc, in_=sums[:, 0:1])
                    t = small.tile([P, 1], F32, tag="t")
                    nc.vector.tensor_scalar(
                        out=t,
                        in0=sc,
                        scalar1=sc,
                        scalar2=float(EPS),
                        op0=Alu.mult,
                        op1=Alu.mult,
                    )
                nc.vector.bn_stats(out=bst2[:, h * 6 : (h + 1) * 6], in_=lout0[:, cols])
            mv = small.tile([P, 2], F32, tag="mv")
            nc.vector.bn_aggr(out=mv, in_=bst2)
            epi_b(pl, 0, lout0, mv, t, split=2, alt_queue=True)

    # Force every ACT instruction onto the one table set that contains all
    # functions we use ({exp, ln, identity} ⊆ natural_log_exp_and_others).
    # The default chooser picks the FIRST set containing each function
    # (exp→set0, ln→set5), inserting a 1.28us table reload twice per
    # chunk. Entries must keep their positions (act_func_set_id is the
    # index), so unwanted sets are emptied rather than removed.
    import concourse.bacc as bacc_mod

    orig_get_tables = bacc_mod.get_activation_tables

    def pinned_tables(arch):
        out = {}
        for name, funcs in orig_get_tables(arch).items():
            out[name] = funcs if name == "natural_log_exp_and_others" else set()
        return out

    bacc_mod.get_activation_tables = pinned_tables
    try:
        nc.compile()
    finally:
        bacc_mod.get_activation_tables = orig_get_tables
    return nc


def _numpy_fallback(query, mask, Wq, bq, Wk, bk, Wv, bv, gamma, beta):
    q = query @ Wq + bq
    k = query @ Wk + bk
    v = query @ Wv + bv
    scale = 1.0 / np.sqrt(np.float32(q.shape[-1]))
    logits = np.einsum("bqd,bkd->bqk", q, k) * scale
    m = np.swapaxes(mask, 1, 2)
    logits = np.where(m, logits, np.float32(-1e9))
    logits = logits - logits.max(axis=2, keepdims=True)
    attn = np.exp(logits)
    attn = attn / attn.sum(axis=2, keepdims=True)
    out = np.einsum("bqk,bkd->bqd", attn, v)
    mu = out.mean(axis=-1, keepdims=True)
    var = out.var(axis=-1, keepdims=True)
    return (out - mu) / np.sqrt(var + 1e-5) * gamma + beta


def kernel(query, mask, Wq, bq, Wk, bk, Wv, bv, gamma, beta):
    global last_results
    from concourse.bass_utils import run_bass_kernel_spmd

    query = np.asarray(query, dtype=np.float32)
    mask = np.asarray(mask)
    Wq = np.asarray(Wq, dtype=np.float32)
    Wk = np.asarray(Wk, dtype=np.float32)
    Wv = np.asarray(Wv, dtype=np.float32)
    bq = np.asarray(bq, dtype=np.float32)
    bk = np.asarray(bk, dtype=np.float32)
    bv = np.asarray(bv, dtype=np.float32)
    gamma = np.asarray(gamma, dtype=np.float32)
    beta = np.asarray(beta, dtype=np.float32)

    if not mask.all():
        # General-mask path (never hit for this problem's all-ones mask).
        return _numpy_fallback(
            query, mask, Wq, bq, Wk, bk, Wv, bv, gamma, beta
        ).astype(np.float32)

    g1b0 = bool((gamma == 1.0).all() and (beta == 0.0).all())
    if g1b0 not in _cached_nc:
        _cached_nc[g1b0] = _build_nc(g1b0)
    nc = _cached_nc[g1b0]

    c = np.float32(1.0 / np.sqrt(D))
    wq_b = (Wq * c).astype(BF)
    wk_b = Wk.astype(BF)
    wv_b = Wv.astype(BF)
    bq_s = (bq * c).astype(np.float32)

    in_maps = []
    for b in range(B):
        m = {
            "xT": np.ascontiguousarray(query[b].T).astype(BF),
            "wq": wq_b,
            "wk": wk_b,
            "wv": wv_b,
            "bq": bq_s,
            "bk": bk,
            "bv": bv,
        }
        if not g1b0:
            m["gamma"] = gamma
            m["beta"] = beta
        in_maps.append(m)

    res = run_bass_kernel_spmd(nc, in_maps, core_ids=list(range(B)))
    last_results = res
    out = np.stack([res.results[b]["out"] for b in range(B)], axis=0)
    return out.astype(np.float32)


# revision 20
# speedup vs baseline: 1.1396x; 1.0001x over previous
"""Fused self-attention + LayerNorm kernel for Trainium2 (8 NeuronCores).

Problem: B=8, S=2048, D=512 dense transformer attention layer.
  q = x@Wq + bq; k = x@Wk + bk; v = x@Wv + bv
  logits = q @ k^T / sqrt(D); attn = softmax(logits)  (mask is all-ones)
  out = LayerNorm(attn @ v) * gamma + beta

Sharding: batch-data-parallel, one batch element per core, no collectives.

Per-core kernel (all matmuls bf16 with f32 PSUM accumulation):
  - host passes x pre-transposed (xT [D,S]) so no on-chip transposes of x
  - qT/kT computed directly in [D,S] layout (W as stationary operand);
    projections run seg-outer so the first 512-column slab of xT is enough
    to start the PE, with DMAs ordered/split to match (wq c-pieces, then
    xT seg-0 pieces, biases, wk, the rest of xT, wv)
  - logits computed TRANSPOSED, [k,q] per 128-k-block (stationary = kT
    block, moving = qT 256-column pair-chunk): exp(logitsT) is then
    directly the stationary operand of attn@v — no PE transposes at all
  - softmax row-sums via 1-row ones-matmuls sharing the attnT stationary
    (PE hwdecode makes the extra instructions ~free); normalization is
    folded into the LayerNorm epilogue analytically
  - attn@v accumulated over 16 k-blocks into one PSUM bank per 128-row
    q-chunk; exp on ACT; no max-subtraction (|logits| < ~2.5)
  - dummy PE matmuls during the initial DMA wait ramp the tensor engine
    to full clock before real work arrives
  - last pair runs its two q-chunks back-to-back (not interleaved) so the
    first chunk's epilogue+store overlaps the second chunk's matmuls, and
    the final store is column-quartered to pipeline DVE with DMA
"""

import sys

import numpy as np

_BASS_REPO = "/opt/trn_rl_repo"
if _BASS_REPO not in sys.path:
    sys.path.insert(0, _BASS_REPO)

import ml_dtypes  # noqa: E402

B, S, D = 8, 2048, 512
P = 128
NC_D = D // P  # 4 contraction chunks
SEG = 512
NSEG = S // SEG  # 4 free-dim segments
NBLK = S // P  # 16 k blocks
QP = 256  # q columns per produce (pair of 128-row chunks)
NPAIR = S // QP  # 8
EPS = 1e-5
BF = ml_dtypes.bfloat16
WARMUP_MM = 46  # dummy PE matmuls issued during the initial DMA wait

_cached_nc = {}
last_results = None  # BassKernelResults of the most recent run (for test.py)


def _build_nc(g1b0):
    import concourse.mybir as mybir
    from concourse import bacc
    from concourse.tile import TileContext

    BF16 = mybir.dt.bfloat16
    F32 = mybir.dt.float32
    Alu = mybir.AluOpType
    Act = mybir.ActivationFunctionType

    nc = bacc.Bacc("TRN2", target_bir_lowering=False, debug=False)

    xT_d = nc.declare_dram_parameter("xT", [D, S], BF16, isOutput=False)
    wq_d = nc.declare_dram_parameter("wq", [D, D], BF16, isOutput=False)
    wk_d = nc.declare_dram_parameter("wk", [D, D], BF16, isOutput=False)
    wv_d = nc.declare_dram_parameter("wv", [D, D], BF16, isOutput=False)
    bq_d = nc.declare_dram_parameter("bq", [D], F32, isOutput=False)
    bk_d = nc.declare_dram_parameter("bk", [D], F32, isOutput=False)
    bv_d = nc.declare_dram_parameter("bv", [D], F32, isOutput=False)
    if not g1b0:
        gamma_d = nc.declare_dram_parameter("gamma", [D], F32, isOutput=False)
        beta_d = nc.declare_dram_parameter("beta", [D], F32, isOutput=False)
    out_d = nc.declare_dram_parameter("out", [S, D], F32, isOutput=True)

    import concourse.bass as bass

    def bcast(param_ap, parts=P):
        # [N] dram vector -> [parts, N] partition-broadcast AP
        return bass.AP(
            tensor=param_ap.tensor,
            offset=param_ap.offset,
            ap=[[0, parts]] + list(param_ap.ap),
        )

    with TileContext(nc) as tc:
        with (
            tc.tile_pool(name="pers", bufs=1) as pers,
            tc.tile_pool(name="attnp", bufs=2) as attnp,
            tc.tile_pool(name="work", bufs=4) as work,
            tc.tile_pool(name="small", bufs=6) as small,
            tc.tile_pool(name="psA", bufs=5, space="PSUM") as psA,
            tc.tile_pool(name="psO", bufs=2, space="PSUM") as psO,
            tc.tile_pool(name="psS", bufs=1, space="PSUM") as psS,
        ):
            # ---- persistent tiles ----
            w_sbs = {
                nm: pers.tile([P, NC_D, D], BF16, tag=nm, name=nm)
                for nm in ("wq", "wk", "wv")
            }
            xT_sb = pers.tile([P, NC_D, S], BF16, tag="xT")

            # ---- input DMAs, ordered around the single HWDGE queue ----
            # Issue serializes at ~625ns/DMA and transfers serialize on the
            # DMA engines, so: few DMAs, ordered to match PE consumption.
            # wq's first c-chunk, then all of xT seg 0 (one rearranged DMA),
            # then the rest of wq, wk, biases, remaining xT segs, wv.
            nc.sync.dma_start(out=w_sbs["wq"][:, 0, :], in_=wq_d.ap()[0:P, :])
            nc.sync.dma_start(
                out=xT_sb[:, :, 0:SEG],
                in_=xT_d.ap()[:, 0:SEG].rearrange("(c p) n -> p c n", p=P),
            )
            for c in range(1, NC_D):
                nc.sync.dma_start(
                    out=w_sbs["wq"][:, c, :], in_=wq_d.ap()[c * P : (c + 1) * P, :]
                )
            nc.sync.dma_start(
                out=w_sbs["wk"], in_=wk_d.ap().rearrange("(c p) n -> p c n", p=P)
            )
            bq_sb = pers.tile([P, NC_D], F32, tag="bq")
            nc.sync.dma_start(out=bq_sb, in_=bq_d.ap().rearrange("(c p) -> p c", p=P))
            bk_sb = pers.tile([P, NC_D], F32, tag="bk")
            nc.sync.dma_start(out=bk_sb, in_=bk_d.ap().rearrange("(c p) -> p c", p=P))
            for g in range(1, NSEG):
                nc.sync.dma_start(
                    out=xT_sb[:, :, g * SEG : (g + 1) * SEG],
                    in_=xT_d.ap()[:, g * SEG : (g + 1) * SEG].rearrange(
                        "(c p) n -> p c n", p=P
                    ),
                )
            nc.sync.dma_start(
                out=w_sbs["wv"], in_=wv_d.ap().rearrange("(c p) n -> p c n", p=P)
            )
            bv_bc = pers.tile([P, D], F32, tag="bv")
            nc.sync.dma_start(out=bv_bc, in_=bcast(bv_d.ap()))
            if not g1b0:
                gamma_bc = pers.tile([P, D], F32, tag="gamma")
                nc.sync.dma_start(out=gamma_bc, in_=bcast(gamma_d.ap()))
                beta_bc = pers.tile([P, D], F32, tag="beta")
                nc.sync.dma_start(out=beta_bc, in_=bcast(beta_d.ap()))

            # PE clock warmup: the tensor engine ramps to full speed only
            # after ~3us of continuous execution. Chew through dummy 128-row
            # matmuls on a zeroed tile while the first input DMAs land.
            # wz is memset on GPSIMD (idle, short preamble) so warmup can
            # start ~0.5us in instead of waiting out the DVE preamble.
            wz = pers.tile([P, P], BF16, tag="wz")
            nc.gpsimd.memset(wz, 0.0)
            eps_sb = pers.tile([P, 1], F32, tag="eps")
            nc.vector.memset(eps_sb, EPS)
            # module-init const, ready at t=0 with no engine dependency
            ones_sb = nc.const_aps.tensor(1.0, (P, 1), BF16)
            # dummy activation right at kernel start: pulls the one-time
            # 1.28us act-table load off the first exp eviction's critical
            # path — it runs concurrently with the input DMAs
            warm = pers.tile([P, 1], F32, tag="warm")
            nc.scalar.activation(out=warm, in_=eps_sb, func=Act.Exp)

            if WARMUP_MM:
                wps = psA.tile([P, SEG], F32, tag="mm", name="warmps")
                for _ in range(WARMUP_MM):
                    nc.tensor.matmul(wps[:, 0:P], wz, wz, start=True, stop=True)

            # ---- phase 1: projections, seg-outer ----
            # qT[d',s], kT[d',s]: stationary = W chunk [d, d'-block],
            # moving = xT [d, s-seg]; accumulate over 4 d-chunks. seg-outer
            # so only xT's first 512 columns gate the start of compute.
            qT_sb = pers.tile([P, NC_D, S], BF16, tag="qT")
            kT_sb = pers.tile([P, NC_D, S], BF16, tag="kT")
            for g in range(NSEG):
                for w_sb, dst, b_sb in (
                    (w_sbs["wq"], qT_sb, bq_sb),
                    (w_sbs["wk"], kT_sb, bk_sb),
                ):
                    pss = [
                        psA.tile([P, SEG], F32, tag="mm", name=f"pj{m}")
                        for m in range(NC_D)
                    ]
                    for c in range(NC_D):
                        for m in range(NC_D):
                            nc.tensor.matmul(
                                pss[m],
                                w_sb[:, c, m * P : (m + 1) * P],
                                xT_sb[:, c, g * SEG : (g + 1) * SEG],
                                start=(c == 0),
                                stop=(c == NC_D - 1),
                            )
                    for m in range(NC_D):
                        # evict + per-partition bias + cast to bf16;
                        # alternate ACT/DVE so two engines drain PSUM
                        if m % 2 == 0:
                            nc.scalar.activation(
                                out=dst[:, m, g * SEG : (g + 1) * SEG],
                                in_=pss[m],
                                func=Act.Identity,
                                bias=b_sb[:, m : m + 1],
                                scale=1.0,
                            )
                        else:
                            nc.vector.tensor_scalar(
                                out=dst[:, m, g * SEG : (g + 1) * SEG],
                                in0=pss[m],
                                scalar1=b_sb[:, m : m + 1],
                                scalar2=None,
                                op0=Alu.add,
                            )
            # v[s,d']: stationary = xT block [d, s-block], moving = Wv [d, d']
            v_sb = pers.tile([P, NBLK, D], BF16, tag="v")
            for j in range(NBLK):
                ps = psA.tile([P, SEG], F32, tag="mm", name="vps")
                for c in range(NC_D):
                    nc.tensor.matmul(
                        ps,
                        xT_sb[:, c, j * P : (j + 1) * P],
                        w_sbs["wv"][:, c, :],
                        start=(c == 0),
                        stop=(c == NC_D - 1),
                    )
                # evict + bias along free dim + cast
                nc.vector.tensor_add(v_sb[:, j, :], ps, bv_bc)

            # ---- phase 2: attention + layernorm, per 256-column q pair ----
            # Software-pipelined: produce pair p+1 (logitsT+exp) before
            # consuming pair p (attn@v + LN epilogue), so the PE never waits
            # on the ACT exp latency.
            def produce(p):
                # logitsT[k, q] per 128-k-block: stationary = kT block,
                # moving = qT pair-chunk. exp(logitsT) lands in attnT ready
                # to be the stationary operand of attn@v — no transposes.
                attnT = attnp.tile([P, NBLK, QP], BF16, tag="attnT")
                for kb in range(NBLK):
                    lg = psA.tile([P, SEG], F32, tag="mm", name=f"lg{kb % 5}")
                    for c in range(NC_D):
                        nc.tensor.matmul(
                            lg[:, 0:QP],
                            kT_sb[:, c, kb * P : (kb + 1) * P],
                            qT_sb[:, c, p * QP : (p + 1) * QP],
                            start=(c == 0),
                            stop=(c == NC_D - 1),
                        )
                    # no max subtraction (|logits| < ~2.5 for this problem)
                    nc.scalar.activation(
                        out=attnT[:, kb, :], in_=lg[:, 0:QP], func=Act.Exp
                    )
                return attnT

            # ---- epilogue, split in two stages ----
            # softmax normalization folded into LN:
            #   raw = attn_unnorm @ v; normalized x = raw / rowsum
            #   out = (raw - mean_raw) * c1 * gamma + beta, where
            #   c1 = (var_raw + eps*rowsum^2)^-0.5
            # (equals rstd(x)/rowsum analytically; eps*rowsum^2 keeps the
            # torch eps semantics). Stage A (DVE stats) is emitted with the
            # consume; stage B (ACT rsqrt via Exp(-0.5*Ln), final pass,
            # store) is deferred until after the NEXT produce so the ACT
            # FIFO never blocks that pair's exp evictions behind a
            # DVE-dependent Ln.
            def epi_a(p, j, out_ps, sums):
                sc = small.tile([P, 1], F32, tag="sc")
                nc.vector.tensor_copy(out=sc, in_=sums[:, j : j + 1])
                bst = small.tile([P, 6], F32, tag="bst")
                nc.vector.bn_stats(out=bst, in_=out_ps)
                mv = small.tile([P, 2], F32, tag="mv")
                nc.vector.bn_aggr(out=mv, in_=bst)
                t = small.tile([P, 1], F32, tag="t")
                nc.vector.tensor_scalar(
                    out=t,
                    in0=sc,
                    scalar1=sc,
                    scalar2=float(EPS),
                    op0=Alu.mult,
                    op1=Alu.mult,
                )
                return mv, t

            def epi_b(p, j, out_ps, mv, t, split, alt_queue=False):
                # rstd = (var + eps*s^2)^-0.5 as Exp(-0.5*Ln(.)) — the ACT
                # engine stays on the single ln+exp function table (a Sqrt
                # would force a 1.3us table reload twice per chunk)
                lnv = small.tile([P, 1], F32, tag="lnv")
                nc.scalar.activation(
                    out=lnv, in_=mv[:, 1:2], func=Act.Ln, bias=t, scale=1.0
                )
                c1 = small.tile([P, 1], F32, tag="c1")
                nc.scalar.activation(out=c1, in_=lnv, func=Act.Exp, scale=-0.5)

                row = (p * 2 + j) * P
                hw_ = D // split
                for h in range(split):
                    cols = slice(h * hw_, (h + 1) * hw_)
                    y = work.tile([P, hw_], F32, tag=f"y{h}", name=f"y{h}")
                    nc.vector.tensor_scalar(
                        out=y,
                        in0=out_ps[:, cols],
                        scalar1=mv[:, 0:1],
                        scalar2=c1,
                        op0=Alu.subtract,
                        op1=Alu.mult,
                    )
                    if not g1b0:
                        o1 = work.tile([P, hw_], F32, tag=f"o1{h}", name=f"o1{h}")
                        nc.vector.tensor_mul(o1, y, gamma_bc[:, cols])
                        y = work.tile([P, hw_], F32, tag=f"o{h}", name=f"o{h}")
                        nc.vector.tensor_add(y, o1, beta_bc[:, cols])
                    # alternate trigger queues on the tail so the final
                    # stores issue in parallel instead of serializing on SP
                    eng = nc.scalar if (alt_queue and h % 2 == 1) else nc.sync
                    eng.dma_start(out=out_d.ap()[row : row + P, cols], in_=y)

            # one persistent sums bank, column-region double-buffered by pair
            # parity so consecutive pairs' rowsum accumulations never share a
            # WAR dependency on the epilogue's read
            sums_all = psS.tile([P, 6], F32, tag="s")

            def consume_mm(p, attnT, outps, sums, j):
                for kb in range(NBLK):
                    st = attnT[:, kb, j * P : (j + 1) * P]
                    nc.tensor.matmul(
                        outps[j],
                        st,
                        v_sb[:, kb, :],
                        start=(kb == 0),
                        stop=(kb == NBLK - 1),
                    )
                    # 1-row matmul reusing the stationary: rowsum of the
                    # exact bf16 attn weights used above
                    nc.tensor.matmul(
                        sums[:, j : j + 1],
                        st,
                        ones_sb,
                        start=(kb == 0),
                        stop=(kb == NBLK - 1),
                    )

            def consume_a(p, attnT):
                outps = [
                    psO.tile([P, D], F32, tag="out", name=f"out{j}") for j in (0, 1)
                ]
                sums = sums_all[:, (p % 2) * 2 : (p % 2) * 2 + 2]
                for kb in range(NBLK):
                    for j in (0, 1):
                        st = attnT[:, kb, j * P : (j + 1) * P]
                        nc.tensor.matmul(
                            outps[j],
                            st,
                            v_sb[:, kb, :],
                            start=(kb == 0),
                            stop=(kb == NBLK - 1),
                        )
                        nc.tensor.matmul(
                            sums[:, j : j + 1],
                            st,
                            ones_sb,
                            start=(kb == 0),
                            stop=(kb == NBLK - 1),
                        )
                state = []
                for j in (0, 1):
                    mv, t = epi_a(p, j, outps[j], sums)
                    state.append((outps[j], mv, t))
                return state

            pend_attn = None  # produce(p) awaiting consume
            pend_epi = None  # (p, state) awaiting epi_b
            for p in range(NPAIR):
                produced = produce(p)
                if pend_epi is not None:
                    ep, st = pend_epi
                    for j in (0, 1):
                        epi_b(ep, j, st[j][0], st[j][1], st[j][2], split=1)
                if pend_attn is not None:
                    pend_epi = (p - 1, consume_a(p - 1, pend_attn))
                pend_attn = produced
            ep, st = pend_epi
            for j in (0, 1):
                epi_b(ep, j, st[j][0], st[j][1], st[j][2], split=1)

            # last pair: accumulate into now-idle psA banks (no WAR against
            # the previous pair's psO epilogue reads), run the two q-chunks
            # back-to-back so chunk j=1's full epilogue+store overlaps chunk
            # j=0's matmuls, and column-halve j=0's accumulation so its
            # bn_stats mostly overlaps the final matmuls
            pl = NPAIR - 1
            attnT = pend_attn
            sums = sums_all[:, (pl % 2) * 2 : (pl % 2) * 2 + 2]
            lout1 = psA.tile([P, D], F32, tag="mm", name="lout1")
            consume_mm(pl, attnT, {1: lout1}, sums, 1)
            mv, t = epi_a(pl, 1, lout1, sums)
            epi_b(pl, 1, lout1, mv, t, split=2)

            lout0 = psA.tile([P, D], F32, tag="mm", name="lout0")
            lsums = sums_all[:, 4:5]  # untouched column: no tracked deps
            HB = D // 2
            bst2 = small.tile([P, 12], F32, tag="bst2")
            for h in (0, 1):
                cols = slice(h * HB, (h + 1) * HB)
                for kb in range(NBLK):
                    st = attnT[:, kb, 0:P]
                    nc.tensor.matmul(
                        lout0[:, cols],
                        st,
                        v_sb[:, kb, cols],
                        start=(kb == 0),
                        stop=(kb == NBLK - 1),
                    )
                    if h == 0:
                        nc.tensor.matmul(
                            lsums,
                            st,
                            ones_sb,
                            start=(kb == 0),
                            stop=(kb == NBLK - 1),
                        )
                if h == 0:
                    sc = small.tile([P, 1], F32, tag="sc")
                    nc.vector.tensor_copy(out=sc, in_=lsums)
                    t = small.tile([P, 1], F32, tag="t")
                    nc.vector.tensor_scalar(
                        out=t,
                        in0=sc,
                        scalar1=sc,
                        scalar2=float(EPS),
                        op0=Alu.mult,
                        op1=Alu.mult,
                    )
                nc.vector.bn_stats(out=bst2[:, h * 6 : (h + 1) * 6], in_=lout0[:, cols])
            mv = small.tile([P, 2], F32, tag="mv")
            nc.vector.bn_aggr(out=mv, in_=bst2)
            epi_b(pl, 0, lout0, mv, t, split=2, alt_queue=True)

    # Force every ACT instruction onto the one table set that contains all
    # functions we use ({exp, ln, identity} ⊆ natural_log_exp_and_others).
    # The default chooser picks the FIRST set containing each function
    # (exp→set0, ln→set5), inserting a 1.28us table reload twice per
    # chunk. Entries must keep their positions (act_func_set_id is the
    # index), so unwanted sets are emptied rather than removed.
    import concourse.bacc as bacc_mod

    orig_get_tables = bacc_mod.get_activation_tables

    def pinned_tables(arch):
        out = {}
        for name, funcs in orig_get_tables(arch).items():
            out[name] = funcs if name == "natural_log_exp_and_others" else set()
        return out

    bacc_mod.get_activation_tables = pinned_tables
    try:
        nc.compile()
    finally:
        bacc_mod.get_activation_tables = orig_get_tables
    return nc


def _numpy_fallback(query, mask, Wq, bq, Wk, bk, Wv, bv, gamma, beta):
    q = query @ Wq + bq
    k = query @ Wk + bk
    v = query @ Wv + bv
    scale = 1.0 / np.sqrt(np.float32(q.shape[-1]))
    logits = np.einsum("bqd,bkd->bqk", q, k) * scale
    m = np.swapaxes(mask, 1, 2)
    logits = np.where(m, logits, np.float32(-1e9))
    logits = logits - logits.max(axis=2, keepdims=True)
    attn = np.exp(logits)
    attn = attn / attn.sum(axis=2, keepdims=True)
    out = np.einsum("bqk,bkd->bqd", attn, v)
    mu = out.mean(axis=-1, keepdims=True)
    var = out.var(axis=-1, keepdims=True)
    return (out - mu) / np.sqrt(var + 1e-5) * gamma + beta


def kernel(query, mask, Wq, bq, Wk, bk, Wv, bv, gamma, beta):
    global last_results
    from concourse.bass_utils import run_bass_kernel_spmd

    query = np.asarray(query, dtype=np.float32)
    mask = np.asarray(mask)
    Wq = np.asarray(Wq, dtype=np.float32)
    Wk = np.asarray(Wk, dtype=np.float32)
    Wv = np.asarray(Wv, dtype=np.float32)
    bq = np.asarray(bq, dtype=np.float32)
    bk = np.asarray(bk, dtype=np.float32)
    bv = np.asarray(bv, dtype=np.float32)
    gamma = np.asarray(gamma, dtype=np.float32)
    beta = np.asarray(beta, dtype=np.float32)

    if not mask.all():
        # General-mask path (never hit for this problem's all-ones mask).
        return _numpy_fallback(
            query, mask, Wq, bq, Wk, bk, Wv, bv, gamma, beta
        ).astype(np.float32)

    g1b0 = bool((gamma == 1.0).all() and (beta == 0.0).all())
    if g1b0 not in _cached_nc:
        _cached_nc[g1b0] = _build_nc(g1b0)
    nc = _cached_nc[g1b0]

    c = np.float32(1.0 / np.sqrt(D))
    wq_b = (Wq * c).astype(BF)
    wk_b = Wk.astype(BF)
    wv_b = Wv.astype(BF)
    bq_s = (bq * c).astype(np.float32)

    in_maps = []
    for b in range(B):
        m = {
            "xT": np.ascontiguousarray(query[b].T).astype(BF),
            "wq": wq_b,
            "wk": wk_b,
            "wv": wv_b,
            "bq": bq_s,
            "bk": bk,
            "bv": bv,
        }
        if not g1b0:
            m["gamma"] = gamma
            m["beta"] = beta
        in_maps.append(m)

    res = run_bass_kernel_spmd(nc, in_maps, core_ids=list(range(B)))
    last_results = res
    out = np.stack([res.results[b]["out"] for b in range(B)], axis=0)
    return out.astype(np.float32)


# revision 21
# speedup vs baseline: 1.1452x; 1.0049x over previous
"""Fused self-attention + LayerNorm kernel for Trainium2 (8 NeuronCores).

Problem: B=8, S=2048, D=512 dense transformer attention layer.
  q = x@Wq + bq; k = x@Wk + bk; v = x@Wv + bv
  logits = q @ k^T / sqrt(D); attn = softmax(logits)  (mask is all-ones)
  out = LayerNorm(attn @ v) * gamma + beta

Sharding: batch-data-parallel, one batch element per core, no collectives.

Per-core kernel (all matmuls bf16 with f32 PSUM accumulation):
  - host passes x pre-transposed (xT [D,S]) so no on-chip transposes of x
  - qT/kT computed directly in [D,S] layout (W as stationary operand);
    projections run seg-outer so the first 512-column slab of xT is enough
    to start the PE, with DMAs ordered/split to match (wq c-pieces, then
    xT seg-0 pieces, biases, wk, the rest of xT, wv)
  - logits computed TRANSPOSED, [k,q] per 128-k-block (stationary = kT
    block, moving = qT 256-column pair-chunk): exp(logitsT) is then
    directly the stationary operand of attn@v — no PE transposes at all
  - softmax row-sums via 1-row ones-matmuls sharing the attnT stationary
    (PE hwdecode makes the extra instructions ~free); normalization is
    folded into the LayerNorm epilogue analytically
  - attn@v accumulated over 16 k-blocks into one PSUM bank per 128-row
    q-chunk; exp on ACT; no max-subtraction (|logits| < ~2.5)
  - dummy PE matmuls during the initial DMA wait ramp the tensor engine
    to full clock before real work arrives
  - last pair runs its two q-chunks back-to-back (not interleaved) so the
    first chunk's epilogue+store overlaps the second chunk's matmuls, and
    the final store is column-quartered to pipeline DVE with DMA
"""

import sys

import numpy as np

_BASS_REPO = "/opt/trn_rl_repo"
if _BASS_REPO not in sys.path:
    sys.path.insert(0, _BASS_REPO)

import ml_dtypes  # noqa: E402

B, S, D = 8, 2048, 512
P = 128
NC_D = D // P  # 4 contraction chunks
SEG = 512
NSEG = S // SEG  # 4 free-dim segments
NBLK = S // P  # 16 k blocks
QP = 256  # q columns per produce (pair of 128-row chunks)
NPAIR = S // QP  # 8
EPS = 1e-5
BF = ml_dtypes.bfloat16
WARMUP_MM = 46  # dummy PE matmuls issued during the initial DMA wait

_cached_nc = {}
last_results = None  # BassKernelResults of the most recent run (for test.py)


def _build_nc(g1b0):
    import concourse.mybir as mybir
    from concourse import bacc
    from concourse.tile import TileContext

    BF16 = mybir.dt.bfloat16
    F32 = mybir.dt.float32
    Alu = mybir.AluOpType
    Act = mybir.ActivationFunctionType

    nc = bacc.Bacc("TRN2", target_bir_lowering=False, debug=False)

    xT_d = nc.declare_dram_parameter("xT", [D, S], BF16, isOutput=False)
    wq_d = nc.declare_dram_parameter("wq", [D, D], BF16, isOutput=False)
    wk_d = nc.declare_dram_parameter("wk", [D, D], BF16, isOutput=False)
    wv_d = nc.declare_dram_parameter("wv", [D, D], BF16, isOutput=False)
    bq_d = nc.declare_dram_parameter("bq", [D], F32, isOutput=False)
    bk_d = nc.declare_dram_parameter("bk", [D], F32, isOutput=False)
    bv_d = nc.declare_dram_parameter("bv", [D], F32, isOutput=False)
    if not g1b0:
        gamma_d = nc.declare_dram_parameter("gamma", [D], F32, isOutput=False)
        beta_d = nc.declare_dram_parameter("beta", [D], F32, isOutput=False)
    out_d = nc.declare_dram_parameter("out", [S, D], F32, isOutput=True)

    import concourse.bass as bass

    def bcast(param_ap, parts=P):
        # [N] dram vector -> [parts, N] partition-broadcast AP
        return bass.AP(
            tensor=param_ap.tensor,
            offset=param_ap.offset,
            ap=[[0, parts]] + list(param_ap.ap),
        )

    with TileContext(nc) as tc:
        with (
            tc.tile_pool(name="pers", bufs=1) as pers,
            tc.tile_pool(name="attnp", bufs=2) as attnp,
            tc.tile_pool(name="work", bufs=4) as work,
            tc.tile_pool(name="small", bufs=6) as small,
            tc.tile_pool(name="psA", bufs=5, space="PSUM") as psA,
            tc.tile_pool(name="psO", bufs=2, space="PSUM") as psO,
            tc.tile_pool(name="psS", bufs=1, space="PSUM") as psS,
        ):
            # ---- persistent tiles ----
            w_sbs = {
                nm: pers.tile([P, NC_D, D], BF16, tag=nm, name=nm)
                for nm in ("wq", "wk", "wv")
            }
            xT_sb = pers.tile([P, NC_D, S], BF16, tag="xT")

            # ---- input DMAs, ordered around the single HWDGE queue ----
            # Issue serializes at ~625ns/DMA and transfers serialize on the
            # DMA engines, so: few DMAs, ordered to match PE consumption.
            # wq's first c-chunk, then all of xT seg 0 (one rearranged DMA),
            # then the rest of wq, wk, biases, remaining xT segs, wv.
            nc.sync.dma_start(out=w_sbs["wq"][:, 0, :], in_=wq_d.ap()[0:P, :])
            nc.sync.dma_start(
                out=xT_sb[:, :, 0:SEG],
                in_=xT_d.ap()[:, 0:SEG].rearrange("(c p) n -> p c n", p=P),
            )
            for c in range(1, NC_D):
                nc.sync.dma_start(
                    out=w_sbs["wq"][:, c, :], in_=wq_d.ap()[c * P : (c + 1) * P, :]
                )
            nc.sync.dma_start(
                out=w_sbs["wk"], in_=wk_d.ap().rearrange("(c p) n -> p c n", p=P)
            )
            bq_sb = pers.tile([P, NC_D], F32, tag="bq")
            nc.sync.dma_start(out=bq_sb, in_=bq_d.ap().rearrange("(c p) -> p c", p=P))
            bk_sb = pers.tile([P, NC_D], F32, tag="bk")
            nc.sync.dma_start(out=bk_sb, in_=bk_d.ap().rearrange("(c p) -> p c", p=P))
            for g in range(1, NSEG):
                nc.sync.dma_start(
                    out=xT_sb[:, :, g * SEG : (g + 1) * SEG],
                    in_=xT_d.ap()[:, g * SEG : (g + 1) * SEG].rearrange(
                        "(c p) n -> p c n", p=P
                    ),
                )
            nc.sync.dma_start(
                out=w_sbs["wv"], in_=wv_d.ap().rearrange("(c p) n -> p c n", p=P)
            )
            bv_bc = pers.tile([P, D], F32, tag="bv")
            nc.sync.dma_start(out=bv_bc, in_=bcast(bv_d.ap()))
            if not g1b0:
                gamma_bc = pers.tile([P, D], F32, tag="gamma")
                nc.sync.dma_start(out=gamma_bc, in_=bcast(gamma_d.ap()))
                beta_bc = pers.tile([P, D], F32, tag="beta")
                nc.sync.dma_start(out=beta_bc, in_=bcast(beta_d.ap()))

            # PE clock warmup: the tensor engine ramps to full speed only
            # after ~3us of continuous execution. Chew through dummy 128-row
            # matmuls on a zeroed tile while the first input DMAs land.
            # wz is memset on GPSIMD (idle, short preamble) so warmup can
            # start ~0.5us in instead of waiting out the DVE preamble.
            wz = pers.tile([P, P], BF16, tag="wz")
            nc.gpsimd.memset(wz, 0.0)
            eps_sb = pers.tile([P, 1], F32, tag="eps")
            nc.vector.memset(eps_sb, EPS)
            # module-init const, ready at t=0 with no engine dependency
            ones_sb = nc.const_aps.tensor(1.0, (P, 1), BF16)
            # dummy activation right at kernel start: pulls the one-time
            # 1.28us act-table load off the first exp eviction's critical
            # path — it runs concurrently with the input DMAs
            warm = pers.tile([P, 1], F32, tag="warm")
            nc.scalar.activation(out=warm, in_=eps_sb, func=Act.Exp)

            if WARMUP_MM:
                wps = psA.tile([P, SEG], F32, tag="mm", name="warmps")
                for _ in range(WARMUP_MM):
                    nc.tensor.matmul(wps[:, 0:P], wz, wz, start=True, stop=True)

            # ---- phase 1: projections, seg-outer ----
            # qT[d',s], kT[d',s]: stationary = W chunk [d, d'-block],
            # moving = xT [d, s-seg]; accumulate over 4 d-chunks. seg-outer
            # so only xT's first 512 columns gate the start of compute.
            qT_sb = pers.tile([P, NC_D, S], BF16, tag="qT")
            kT_sb = pers.tile([P, NC_D, S], BF16, tag="kT")
            for g in range(NSEG):
                for w_sb, dst, b_sb in (
                    (w_sbs["wq"], qT_sb, bq_sb),
                    (w_sbs["wk"], kT_sb, bk_sb),
                ):
                    pss = [
                        psA.tile([P, SEG], F32, tag="mm", name=f"pj{m}")
                        for m in range(NC_D)
                    ]
                    for c in range(NC_D):
                        for m in range(NC_D):
                            nc.tensor.matmul(
                                pss[m],
                                w_sb[:, c, m * P : (m + 1) * P],
                                xT_sb[:, c, g * SEG : (g + 1) * SEG],
                                start=(c == 0),
                                stop=(c == NC_D - 1),
                            )
                    for m in range(NC_D):
                        # evict + per-partition bias + cast to bf16;
                        # alternate ACT/DVE so two engines drain PSUM
                        if m % 2 == 0:
                            nc.scalar.activation(
                                out=dst[:, m, g * SEG : (g + 1) * SEG],
                                in_=pss[m],
                                func=Act.Identity,
                                bias=b_sb[:, m : m + 1],
                                scale=1.0,
                            )
                        else:
                            nc.vector.tensor_scalar(
                                out=dst[:, m, g * SEG : (g + 1) * SEG],
                                in0=pss[m],
                                scalar1=b_sb[:, m : m + 1],
                                scalar2=None,
                                op0=Alu.add,
                            )
            # v[s,d']: stationary = xT block [d, s-block], moving = Wv [d, d']
            v_sb = pers.tile([P, NBLK, D], BF16, tag="v")
            for j in range(NBLK):
                ps = psA.tile([P, SEG], F32, tag="mm", name="vps")
                for c in range(NC_D):
                    nc.tensor.matmul(
                        ps,
                        xT_sb[:, c, j * P : (j + 1) * P],
                        w_sbs["wv"][:, c, :],
                        start=(c == 0),
                        stop=(c == NC_D - 1),
                    )
                # evict + bias along free dim + cast
                nc.vector.tensor_add(v_sb[:, j, :], ps, bv_bc)

            # ---- phase 2: attention + layernorm, per 256-column q pair ----
            # Software-pipelined: produce pair p+1 (logitsT+exp) before
            # consuming pair p (attn@v + LN epilogue), so the PE never waits
            # on the ACT exp latency.
            def produce(p):
                # logitsT[k, q] per 128-k-block: stationary = kT block,
                # moving = qT pair-chunk. exp(logitsT) lands in attnT ready
                # to be the stationary operand of attn@v — no transposes.
                attnT = attnp.tile([P, NBLK, QP], BF16, tag="attnT")
                for kb in range(NBLK):
                    lg = psA.tile([P, SEG], F32, tag="mm", name=f"lg{kb % 5}")
                    for c in range(NC_D):
                        nc.tensor.matmul(
                            lg[:, 0:QP],
                            kT_sb[:, c, kb * P : (kb + 1) * P],
                            qT_sb[:, c, p * QP : (p + 1) * QP],
                            start=(c == 0),
                            stop=(c == NC_D - 1),
                        )
                    # no max subtraction (|logits| < ~2.5 for this problem)
                    nc.scalar.activation(
                        out=attnT[:, kb, :], in_=lg[:, 0:QP], func=Act.Exp
                    )
                return attnT

            # ---- epilogue, split in two stages ----
            # softmax normalization folded into LN:
            #   raw = attn_unnorm @ v; normalized x = raw / rowsum
            #   out = (raw - mean_raw) * c1 * gamma + beta, where
            #   c1 = (var_raw + eps*rowsum^2)^-0.5
            # (equals rstd(x)/rowsum analytically; eps*rowsum^2 keeps the
            # torch eps semantics). Stage A (DVE stats) is emitted with the
            # consume; stage B (ACT rsqrt via Exp(-0.5*Ln), final pass,
            # store) is deferred until after the NEXT produce so the ACT
            # FIFO never blocks that pair's exp evictions behind a
            # DVE-dependent Ln.
            def epi_a(p, j, out_ps, sums):
                sc = small.tile([P, 1], F32, tag="sc")
                nc.vector.tensor_copy(out=sc, in_=sums[:, j : j + 1])
                bst = small.tile([P, 6], F32, tag="bst")
                nc.vector.bn_stats(out=bst, in_=out_ps)
                mv = small.tile([P, 2], F32, tag="mv")
                nc.vector.bn_aggr(out=mv, in_=bst)
                t = small.tile([P, 1], F32, tag="t")
                nc.vector.tensor_scalar(
                    out=t,
                    in0=sc,
                    scalar1=sc,
                    scalar2=float(EPS),
                    op0=Alu.mult,
                    op1=Alu.mult,
                )
                return mv, t

            def epi_b(p, j, out_ps, mv, t, split, alt_queue=False):
                # rstd = (var + eps*s^2)^-0.5 as Exp(-0.5*Ln(.)) — the ACT
                # engine stays on the single ln+exp function table (a Sqrt
                # would force a 1.3us table reload twice per chunk)
                lnv = small.tile([P, 1], F32, tag="lnv")
                nc.scalar.activation(
                    out=lnv, in_=mv[:, 1:2], func=Act.Ln, bias=t, scale=1.0
                )
                c1 = small.tile([P, 1], F32, tag="c1")
                nc.scalar.activation(out=c1, in_=lnv, func=Act.Exp, scale=-0.5)

                row = (p * 2 + j) * P
                hw_ = D // split
                for h in range(split):
                    cols = slice(h * hw_, (h + 1) * hw_)
                    y = work.tile([P, hw_], F32, tag=f"y{h}", name=f"y{h}")
                    nc.vector.tensor_scalar(
                        out=y,
                        in0=out_ps[:, cols],
                        scalar1=mv[:, 0:1],
                        scalar2=c1,
                        op0=Alu.subtract,
                        op1=Alu.mult,
                    )
                    if not g1b0:
                        o1 = work.tile([P, hw_], F32, tag=f"o1{h}", name=f"o1{h}")
                        nc.vector.tensor_mul(o1, y, gamma_bc[:, cols])
                        y = work.tile([P, hw_], F32, tag=f"o{h}", name=f"o{h}")
                        nc.vector.tensor_add(y, o1, beta_bc[:, cols])
                    # alternate trigger queues on the tail so the final
                    # stores issue in parallel instead of serializing on SP
                    eng = nc.scalar if (alt_queue and h % 2 == 1) else nc.sync
                    eng.dma_start(out=out_d.ap()[row : row + P, cols], in_=y)

            # one persistent sums bank, column-region double-buffered by pair
            # parity so consecutive pairs' rowsum accumulations never share a
            # WAR dependency on the epilogue's read
            sums_all = psS.tile([P, 6], F32, tag="s")

            def consume_mm(p, attnT, outps, sums, j):
                for kb in range(NBLK):
                    st = attnT[:, kb, j * P : (j + 1) * P]
                    nc.tensor.matmul(
                        outps[j],
                        st,
                        v_sb[:, kb, :],
                        start=(kb == 0),
                        stop=(kb == NBLK - 1),
                    )
                    # 1-row matmul reusing the stationary: rowsum of the
                    # exact bf16 attn weights used above
                    nc.tensor.matmul(
                        sums[:, j : j + 1],
                        st,
                        ones_sb,
                        start=(kb == 0),
                        stop=(kb == NBLK - 1),
                    )

            def consume_a(p, attnT):
                outps = [
                    psO.tile([P, D], F32, tag="out", name=f"out{j}") for j in (0, 1)
                ]
                sums = sums_all[:, (p % 2) * 2 : (p % 2) * 2 + 2]
                for kb in range(NBLK):
                    for j in (0, 1):
                        st = attnT[:, kb, j * P : (j + 1) * P]
                        nc.tensor.matmul(
                            outps[j],
                            st,
                            v_sb[:, kb, :],
                            start=(kb == 0),
                            stop=(kb == NBLK - 1),
                        )
                        nc.tensor.matmul(
                            sums[:, j : j + 1],
                            st,
                            ones_sb,
                            start=(kb == 0),
                            stop=(kb == NBLK - 1),
                        )
                state = []
                for j in (0, 1):
                    mv, t = epi_a(p, j, outps[j], sums)
                    state.append((outps[j], mv, t))
                return state

            pend_attn = None  # produce(p) awaiting consume
            pend_epi = None  # (p, state) awaiting epi_b
            for p in range(NPAIR):
                produced = produce(p)
                if pend_epi is not None:
                    ep, st = pend_epi
                    for j in (0, 1):
                        epi_b(ep, j, st[j][0], st[j][1], st[j][2], split=1)
                if pend_attn is not None:
                    pend_epi = (p - 1, consume_a(p - 1, pend_attn))
                pend_attn = produced
            ep, st = pend_epi
            for j in (0, 1):
                epi_b(ep, j, st[j][0], st[j][1], st[j][2], split=1)

            # last pair: accumulate into now-idle psA banks (no WAR against
            # the previous pair's psO epilogue reads), run the two q-chunks
            # back-to-back so chunk j=1's full epilogue+store overlaps chunk
            # j=0's matmuls, and column-halve j=0's accumulation so its
            # bn_stats mostly overlaps the final matmuls
            pl = NPAIR - 1
            attnT = pend_attn
            sums = sums_all[:, (pl % 2) * 2 : (pl % 2) * 2 + 2]
            lout1 = psA.tile([P, D], F32, tag="mm", name="lout1")
            consume_mm(pl, attnT, {1: lout1}, sums, 1)
            mv, t = epi_a(pl, 1, lout1, sums)
            epi_b(pl, 1, lout1, mv, t, split=2)

            # j=0 accumulates its two column halves into SEPARATE tiles:
            # tile-level dependency tracking would otherwise see the h0
            # bn_stats (emitted between the halves so it overlaps the h1
            # matmuls) as conflicting with the h1 writes and stall the PE
            lsums = sums_all[:, 4:5]  # untouched column: no tracked deps
            HB = D // 2
            louts = [
                psA.tile([P, HB], F32, tag="mm", name=f"l0h{h}") for h in (0, 1)
            ]
            bst2 = small.tile([P, 12], F32, tag="bst2")
            for h in (0, 1):
                cols = slice(h * HB, (h + 1) * HB)
                for kb in range(NBLK):
                    st = attnT[:, kb, 0:P]
                    nc.tensor.matmul(
                        louts[h],
                        st,
                        v_sb[:, kb, cols],
                        start=(kb == 0),
                        stop=(kb == NBLK - 1),
                    )
                    if h == 0:
                        nc.tensor.matmul(
                            lsums,
                            st,
                            ones_sb,
                            start=(kb == 0),
                            stop=(kb == NBLK - 1),
                        )
                if h == 0:
                    sc = small.tile([P, 1], F32, tag="sc")
                    nc.vector.tensor_copy(out=sc, in_=lsums)
                    t = small.tile([P, 1], F32, tag="t")
                    nc.vector.tensor_scalar(
                        out=t,
                        in0=sc,
                        scalar1=sc,
                        scalar2=float(EPS),
                        op0=Alu.mult,
                        op1=Alu.mult,
                    )
                nc.vector.bn_stats(out=bst2[:, h * 6 : (h + 1) * 6], in_=louts[h])
            mv = small.tile([P, 2], F32, tag="mv")
            nc.vector.bn_aggr(out=mv, in_=bst2)
            lnv = small.tile([P, 1], F32, tag="lnv")
            nc.scalar.activation(
                out=lnv, in_=mv[:, 1:2], func=Act.Ln, bias=t, scale=1.0
            )
            c1 = small.tile([P, 1], F32, tag="c1")
            nc.scalar.activation(out=c1, in_=lnv, func=Act.Exp, scale=-0.5)
            row = pl * 2 * P
            for h in (0, 1):
                cols = slice(h * HB, (h + 1) * HB)
                y = work.tile([P, HB], F32, tag=f"y{h}", name=f"ly{h}")
                nc.vector.tensor_scalar(
                    out=y,
                    in0=louts[h],
                    scalar1=mv[:, 0:1],
                    scalar2=c1,
                    op0=Alu.subtract,
                    op1=Alu.mult,
                )
                if not g1b0:
                    o1 = work.tile([P, HB], F32, tag=f"o1{h}", name=f"lo1{h}")
                    nc.vector.tensor_mul(o1, y, gamma_bc[:, cols])
                    y = work.tile([P, HB], F32, tag=f"o{h}", name=f"lo{h}")
                    nc.vector.tensor_add(y, o1, beta_bc[:, cols])
                eng = nc.scalar if h % 2 == 1 else nc.sync
                eng.dma_start(out=out_d.ap()[row : row + P, cols], in_=y)

    # Force every ACT instruction onto the one table set that contains all
    # functions we use ({exp, ln, identity} ⊆ natural_log_exp_and_others).
    # The default chooser picks the FIRST set containing each function
    # (exp→set0, ln→set5), inserting a 1.28us table reload twice per
    # chunk. Entries must keep their positions (act_func_set_id is the
    # index), so unwanted sets are emptied rather than removed.
    import concourse.bacc as bacc_mod

    orig_get_tables = bacc_mod.get_activation_tables

    def pinned_tables(arch):
        out = {}
        for name, funcs in orig_get_tables(arch).items():
            out[name] = funcs if name == "natural_log_exp_and_others" else set()
        return out

    bacc_mod.get_activation_tables = pinned_tables
    try:
        nc.compile()
    finally:
        bacc_mod.get_activation_tables = orig_get_tables
    return nc


def _numpy_fallback(query, mask, Wq, bq, Wk, bk, Wv, bv, gamma, beta):
    q = query @ Wq + bq
    k = query @ Wk + bk
    v = query @ Wv + bv
    scale = 1.0 / np.sqrt(np.float32(q.shape[-1]))
    logits = np.einsum("bqd,bkd->bqk", q, k) * scale
    m = np.swapaxes(mask, 1, 2)
    logits = np.where(m, logits, np.float32(-1e9))
    logits = logits - logits.max(axis=2, keepdims=True)
    attn = np.exp(logits)
    attn = attn / attn.sum(axis=2, keepdims=True)
    out = np.einsum("bqk,bkd->bqd", attn, v)
    mu = out.mean(axis=-1, keepdims=True)
    var = out.var(axis=-1, keepdims=True)
    return (out - mu) / np.sqrt(var + 1e-5) * gamma + beta


def kernel(query, mask, Wq, bq, Wk, bk, Wv, bv, gamma, beta):
    global last_results
    from concourse.bass_utils import run_bass_kernel_spmd

    query = np.asarray(query, dtype=np.float32)
    mask = np.asarray(mask)
    Wq = np.asarray(Wq, dtype=np.float32)
    Wk = np.asarray(Wk, dtype=np.float32)
    Wv = np.asarray(Wv, dtype=np.float32)
    bq = np.asarray(bq, dtype=np.float32)
    bk = np.asarray(bk, dtype=np.float32)
    bv = np.asarray(bv, dtype=np.float32)
    gamma = np.asarray(gamma, dtype=np.float32)
    beta = np.asarray(beta, dtype=np.float32)

    if not mask.all():
        # General-mask path (never hit for this problem's all-ones mask).
        return _numpy_fallback(
            query, mask, Wq, bq, Wk, bk, Wv, bv, gamma, beta
        ).astype(np.float32)

    g1b0 = bool((gamma == 1.0).all() and (beta == 0.0).all())
    if g1b0 not in _cached_nc:
        _cached_nc[g1b0] = _build_nc(g1b0)
    nc = _cached_nc[g1b0]

    c = np.float32(1.0 / np.sqrt(D))
    wq_b = (Wq * c).astype(BF)
    wk_b = Wk.astype(BF)
    wv_b = Wv.astype(BF)
    bq_s = (bq * c).astype(np.float32)

    in_maps = []
    for b in range(B):
        m = {
            "xT": np.ascontiguousarray(query[b].T).astype(BF),
            "wq": wq_b,
            "wk": wk_b,
            "wv": wv_b,
            "bq": bq_s,
            "bk": bk,
            "bv": bv,
        }
        if not g1b0:
            m["gamma"] = gamma
            m["beta"] = beta
        in_maps.append(m)

    res = run_bass_kernel_spmd(nc, in_maps, core_ids=list(range(B)))
    last_results = res
    out = np.stack([res.results[b]["out"] for b in range(B)], axis=0)
    return out.astype(np.float32)


# revision 28
# speedup vs baseline: 1.2489x; 1.0906x over previous
"""Fused self-attention + LayerNorm kernel for Trainium2 (8 NeuronCores).

Problem: B=8, S=2048, D=512 dense transformer attention layer.
  q = x@Wq + bq; k = x@Wk + bk; v = x@Wv + bv
  logits = q @ k^T / sqrt(D); attn = softmax(logits)  (mask is all-ones)
  out = LayerNorm(attn @ v) * gamma + beta

Sharding: batch-data-parallel, one batch element per core, no collectives.

Per-core kernel (all matmuls bf16 with f32 PSUM accumulation):
  - host passes x pre-transposed (xT [D,S]) so no on-chip transposes of x
  - qT/kT computed directly in [D,S] layout (W as stationary operand);
    projections run seg-outer so the first 512-column slab of xT is enough
    to start the PE, with DMAs ordered/split to match (wq c-pieces, then
    xT seg-0 pieces, biases, wk, the rest of xT, wv)
  - logits computed TRANSPOSED, [k,q] per 128-k-block (stationary = kT
    block, moving = qT 256-column pair-chunk): exp(logitsT) is then
    directly the stationary operand of attn@v — no PE transposes at all
  - softmax row-sums via 1-row ones-matmuls sharing the attnT stationary
    (PE hwdecode makes the extra instructions ~free); normalization is
    folded into the LayerNorm epilogue analytically
  - attn@v accumulated over 16 k-blocks into one PSUM bank per 128-row
    q-chunk; exp on ACT; no max-subtraction (|logits| < ~2.5)
  - dummy PE matmuls during the initial DMA wait ramp the tensor engine
    to full clock before real work arrives
  - last pair runs its two q-chunks back-to-back (not interleaved) so the
    first chunk's epilogue+store overlaps the second chunk's matmuls, and
    the final store is column-quartered to pipeline DVE with DMA
"""

import sys

import numpy as np

_BASS_REPO = "/opt/trn_rl_repo"
if _BASS_REPO not in sys.path:
    sys.path.insert(0, _BASS_REPO)

import ml_dtypes  # noqa: E402

B, S, D = 8, 2048, 512
P = 128
NC_D = D // P  # 4 contraction chunks
SEG = 512
NSEG = S // SEG  # 4 free-dim segments
NBLK = S // P  # 16 k blocks
QP = 256  # q columns per produce (pair of 128-row chunks)
NPAIR = S // QP  # 8
EPS = 1e-5
BF = ml_dtypes.bfloat16
WARMUP_MM = 46  # dummy PE matmuls issued during the initial DMA wait

_cached_nc = {}
last_results = None  # BassKernelResults of the most recent run (for test.py)


def _build_nc(g1b0):
    import concourse.mybir as mybir
    from concourse import bacc
    from concourse.tile import TileContext

    BF16 = mybir.dt.bfloat16
    F32 = mybir.dt.float32
    Alu = mybir.AluOpType
    Act = mybir.ActivationFunctionType

    nc = bacc.Bacc("TRN2", target_bir_lowering=False, debug=False)

    xT_d = nc.declare_dram_parameter("xT", [D, S], BF16, isOutput=False)
    # m = Wq @ Wk^T / sqrt(D) folded on host: logits = x @ m @ x^T needs one
    # projection (u = x@m) instead of two (q and k). gk = Wk @ bq / sqrt(D)
    # carries the only softmax-relevant bias term (per-k, added pre-exp);
    # the q-bias term is constant per row and softmax-invariant, and the
    # epilogue's c1 form is invariant to the resulting rowsum rescale.
    m_d = nc.declare_dram_parameter("m", [D, D], BF16, isOutput=False)
    gk_d = nc.declare_dram_parameter("gk", [D], BF16, isOutput=False)
    wv_d = nc.declare_dram_parameter("wv", [D, D], BF16, isOutput=False)
    bv_d = nc.declare_dram_parameter("bv", [D], F32, isOutput=False)
    if not g1b0:
        gamma_d = nc.declare_dram_parameter("gamma", [D], F32, isOutput=False)
        beta_d = nc.declare_dram_parameter("beta", [D], F32, isOutput=False)
    out_d = nc.declare_dram_parameter("out", [S, D], F32, isOutput=True)

    import concourse.bass as bass

    def bcast(param_ap, parts=P):
        # [N] dram vector -> [parts, N] partition-broadcast AP
        return bass.AP(
            tensor=param_ap.tensor,
            offset=param_ap.offset,
            ap=[[0, parts]] + list(param_ap.ap),
        )

    with TileContext(nc) as tc:
        with (
            tc.tile_pool(name="pers", bufs=1) as pers,
            tc.tile_pool(name="attnp", bufs=2) as attnp,
            tc.tile_pool(name="work", bufs=4) as work,
            tc.tile_pool(name="small", bufs=6) as small,
            tc.tile_pool(name="psA", bufs=5, space="PSUM") as psA,
            tc.tile_pool(name="psO", bufs=2, space="PSUM") as psO,
            tc.tile_pool(name="psS", bufs=1, space="PSUM") as psS,
        ):
            # ---- persistent tiles ----
            w_sbs = {
                nm: pers.tile([P, NC_D, D], BF16, tag=nm, name=nm)
                for nm in ("m", "wv")
            }
            xT_sb = pers.tile([P, NC_D, S], BF16, tag="xT")

            # ---- input DMAs, ordered around the single HWDGE queue ----
            # Issue serializes at ~625ns/DMA and transfers serialize on the
            # DMA engines, so: few DMAs, ordered to match PE consumption.
            # M's first c-chunk, then all of xT seg 0 (one rearranged DMA),
            # then the rest of M, gk, remaining xT segs, wv.
            nc.sync.dma_start(out=w_sbs["m"][:, 0, :], in_=m_d.ap()[0:P, :])
            nc.sync.dma_start(
                out=xT_sb[:, :, 0:SEG],
                in_=xT_d.ap()[:, 0:SEG].rearrange("(c p) n -> p c n", p=P),
            )
            for c in range(1, NC_D):
                nc.sync.dma_start(
                    out=w_sbs["m"][:, c, :], in_=m_d.ap()[c * P : (c + 1) * P, :]
                )
            gk_sb = pers.tile([P, NC_D], BF16, tag="gk")
            nc.sync.dma_start(out=gk_sb, in_=gk_d.ap().rearrange("(c p) -> p c", p=P))
            for g in range(1, NSEG):
                nc.sync.dma_start(
                    out=xT_sb[:, :, g * SEG : (g + 1) * SEG],
                    in_=xT_d.ap()[:, g * SEG : (g + 1) * SEG].rearrange(
                        "(c p) n -> p c n", p=P
                    ),
                )
            nc.sync.dma_start(
                out=w_sbs["wv"], in_=wv_d.ap().rearrange("(c p) n -> p c n", p=P)
            )
            bv_bc = pers.tile([P, D], F32, tag="bv")
            nc.sync.dma_start(out=bv_bc, in_=bcast(bv_d.ap()))
            if not g1b0:
                gamma_bc = pers.tile([P, D], F32, tag="gamma")
                nc.sync.dma_start(out=gamma_bc, in_=bcast(gamma_d.ap()))
                beta_bc = pers.tile([P, D], F32, tag="beta")
                nc.sync.dma_start(out=beta_bc, in_=bcast(beta_d.ap()))

            # PE clock warmup: the tensor engine ramps to full speed only
            # after ~3us of continuous execution. Chew through dummy 128-row
            # matmuls on a zeroed tile while the first input DMAs land.
            # wz is memset on GPSIMD (idle, short preamble) so warmup can
            # start ~0.5us in instead of waiting out the DVE preamble.
            wz = pers.tile([P, P], BF16, tag="wz")
            nc.gpsimd.memset(wz, 0.0)
            eps_sb = pers.tile([P, 1], F32, tag="eps")
            nc.vector.memset(eps_sb, EPS)
            # module-init const, ready at t=0 with no engine dependency
            ones_sb = nc.const_aps.tensor(1.0, (P, 1), BF16)
            # dummy activation right at kernel start: pulls the one-time
            # 1.28us act-table load off the first exp eviction's critical
            # path — it runs concurrently with the input DMAs
            warm = pers.tile([P, 1], F32, tag="warm")
            nc.scalar.activation(out=warm, in_=eps_sb, func=Act.Exp)

            if WARMUP_MM:
                wps = psA.tile([P, SEG], F32, tag="mm", name="warmps")
                for _ in range(WARMUP_MM):
                    nc.tensor.matmul(wps[:, 0:P], wz, wz, start=True, stop=True)

            # ---- phase 1: u projection + gamma matvec, seg-outer ----
            # uT[d',s] (u = x@m): stationary = m chunk [d, d'-block],
            # moving = xT [d, s-seg]; accumulate over 4 d-chunks. seg-outer
            # so only xT's first 512 columns gate the start of compute.
            uT_sb = pers.tile([P, NC_D, S], BF16, tag="uT")
            for g in range(NSEG):
                pss = [
                    psA.tile([P, SEG], F32, tag="mm", name=f"pj{m}")
                    for m in range(NC_D)
                ]
                for c in range(NC_D):
                    for m in range(NC_D):
                        nc.tensor.matmul(
                            pss[m],
                            w_sbs["m"][:, c, m * P : (m + 1) * P],
                            xT_sb[:, c, g * SEG : (g + 1) * SEG],
                            start=(c == 0),
                            stop=(c == NC_D - 1),
                        )
                for m in range(NC_D):
                    # evict + cast to bf16; alternate ACT/DVE so two
                    # engines drain PSUM
                    if m % 2 == 0:
                        nc.scalar.activation(
                            out=uT_sb[:, m, g * SEG : (g + 1) * SEG],
                            in_=pss[m],
                            func=Act.Identity,
                        )
                    else:
                        nc.vector.tensor_copy(
                            out=uT_sb[:, m, g * SEG : (g + 1) * SEG],
                            in_=pss[m],
                        )
            # gamma[k] = x[k,:] @ gk: 1-row matmuls, ~free on the hwdecode
            # PE; added per-partition as the exp eviction's bias
            gam_sb = pers.tile([P, NBLK], F32, tag="gam")
            gps = psA.tile([P, SEG], F32, tag="mm", name="gps")
            for kb in range(NBLK):
                for c in range(NC_D):
                    nc.tensor.matmul(
                        gps[:, kb : kb + 1],
                        xT_sb[:, c, kb * P : (kb + 1) * P],
                        gk_sb[:, c : c + 1],
                        start=(c == 0),
                        stop=(c == NC_D - 1),
                    )
            nc.vector.tensor_copy(out=gam_sb, in_=gps[:, 0:NBLK])
            # v[s,d']: stationary = xT block [d, s-block], moving = Wv [d, d']
            v_sb = pers.tile([P, NBLK, D], BF16, tag="v")
            for j in range(NBLK):
                ps = psA.tile([P, SEG], F32, tag="mm", name="vps")
                for c in range(NC_D):
                    nc.tensor.matmul(
                        ps,
                        xT_sb[:, c, j * P : (j + 1) * P],
                        w_sbs["wv"][:, c, :],
                        start=(c == 0),
                        stop=(c == NC_D - 1),
                    )
                # evict + bias along free dim + cast
                nc.vector.tensor_add(v_sb[:, j, :], ps, bv_bc)

            # ---- phase 2: attention + layernorm, per 256-column q pair ----
            # Software-pipelined: produce pair p+1 (logitsT+exp) before
            # consuming pair p (attn@v + LN epilogue), so the PE never waits
            # on the ACT exp latency.
            def produce(p):
                # logitsT[k, q] per 128-k-block: stationary = xT block,
                # moving = uT pair-chunk. exp(logitsT) lands in attnT ready
                # to be the stationary operand of attn@v — no transposes.
                attnT = attnp.tile([P, NBLK, QP], BF16, tag="attnT")
                for kb in range(NBLK):
                    lg = psA.tile([P, SEG], F32, tag="mm", name=f"lg{kb % 5}")
                    for c in range(NC_D):
                        nc.tensor.matmul(
                            lg[:, 0:QP],
                            xT_sb[:, c, kb * P : (kb + 1) * P],
                            uT_sb[:, c, p * QP : (p + 1) * QP],
                            start=(c == 0),
                            stop=(c == NC_D - 1),
                        )
                    # no max subtraction (|logits| < ~2.5 for this problem);
                    # gamma carries the k-bias term (zero for zero bq)
                    nc.scalar.activation(
                        out=attnT[:, kb, :],
                        in_=lg[:, 0:QP],
                        func=Act.Exp,
                        bias=gam_sb[:, kb : kb + 1],
                        scale=1.0,
                    )
                return attnT

            # ---- epilogue, split in two stages ----
            # softmax normalization folded into LN:
            #   raw = attn_unnorm @ v; normalized x = raw / rowsum
            #   out = (raw - mean_raw) * c1 * gamma + beta, where
            #   c1 = (var_raw + eps*rowsum^2)^-0.5
            # (equals rstd(x)/rowsum analytically; eps*rowsum^2 keeps the
            # torch eps semantics). Stage A (DVE stats) is emitted with the
            # consume; stage B (ACT rsqrt via Exp(-0.5*Ln), final pass,
            # store) is deferred until after the NEXT produce so the ACT
            # FIFO never blocks that pair's exp evictions behind a
            # DVE-dependent Ln.
            def epi_a(p, j, out_ps, sums):
                sc = small.tile([P, 1], F32, tag="sc")
                nc.vector.tensor_copy(out=sc, in_=sums[:, j : j + 1])
                bst = small.tile([P, 6], F32, tag="bst")
                nc.vector.bn_stats(out=bst, in_=out_ps)
                mv = small.tile([P, 2], F32, tag="mv")
                nc.vector.bn_aggr(out=mv, in_=bst)
                t = small.tile([P, 1], F32, tag="t")
                nc.vector.tensor_scalar(
                    out=t,
                    in0=sc,
                    scalar1=sc,
                    scalar2=float(EPS),
                    op0=Alu.mult,
                    op1=Alu.mult,
                )
                return mv, t

            def epi_b(p, j, out_ps, mv, t, split, alt_queue=False):
                # rstd = (var + eps*s^2)^-0.5 as Exp(-0.5*Ln(.)) — the ACT
                # engine stays on the single ln+exp function table (a Sqrt
                # would force a 1.3us table reload twice per chunk)
                lnv = small.tile([P, 1], F32, tag="lnv")
                nc.scalar.activation(
                    out=lnv, in_=mv[:, 1:2], func=Act.Ln, bias=t, scale=1.0
                )
                c1 = small.tile([P, 1], F32, tag="c1")
                nc.scalar.activation(out=c1, in_=lnv, func=Act.Exp, scale=-0.5)

                row = (p * 2 + j) * P
                hw_ = D // split
                for h in range(split):
                    cols = slice(h * hw_, (h + 1) * hw_)
                    y = work.tile([P, hw_], F32, tag=f"y{h}", name=f"y{h}")
                    nc.vector.tensor_scalar(
                        out=y,
                        in0=out_ps[:, cols],
                        scalar1=mv[:, 0:1],
                        scalar2=c1,
                        op0=Alu.subtract,
                        op1=Alu.mult,
                    )
                    if not g1b0:
                        o1 = work.tile([P, hw_], F32, tag=f"o1{h}", name=f"o1{h}")
                        nc.vector.tensor_mul(o1, y, gamma_bc[:, cols])
                        y = work.tile([P, hw_], F32, tag=f"o{h}", name=f"o{h}")
                        nc.vector.tensor_add(y, o1, beta_bc[:, cols])
                    # alternate trigger queues on the tail so the final
                    # stores issue in parallel instead of serializing on SP
                    eng = nc.scalar if (alt_queue and h % 2 == 1) else nc.sync
                    eng.dma_start(out=out_d.ap()[row : row + P, cols], in_=y)

            # one persistent sums bank, column-region double-buffered by pair
            # parity so consecutive pairs' rowsum accumulations never share a
            # WAR dependency on the epilogue's read
            sums_all = psS.tile([P, 6], F32, tag="s")

            def consume_mm(p, attnT, outps, sums, j):
                for kb in range(NBLK):
                    st = attnT[:, kb, j * P : (j + 1) * P]
                    nc.tensor.matmul(
                        outps[j],
                        st,
                        v_sb[:, kb, :],
                        start=(kb == 0),
                        stop=(kb == NBLK - 1),
                    )
                    # 1-row matmul reusing the stationary: rowsum of the
                    # exact bf16 attn weights used above
                    nc.tensor.matmul(
                        sums[:, j : j + 1],
                        st,
                        ones_sb,
                        start=(kb == 0),
                        stop=(kb == NBLK - 1),
                    )

            def consume_a(p, attnT):
                outps = [
                    psO.tile([P, D], F32, tag="out", name=f"out{j}") for j in (0, 1)
                ]
                sums = sums_all[:, (p % 2) * 2 : (p % 2) * 2 + 2]
                for kb in range(NBLK):
                    for j in (0, 1):
                        st = attnT[:, kb, j * P : (j + 1) * P]
                        nc.tensor.matmul(
                            outps[j],
                            st,
                            v_sb[:, kb, :],
                            start=(kb == 0),
                            stop=(kb == NBLK - 1),
                        )
                        nc.tensor.matmul(
                            sums[:, j : j + 1],
                            st,
                            ones_sb,
                            start=(kb == 0),
                            stop=(kb == NBLK - 1),
                        )
                state = []
                for j in (0, 1):
                    mv, t = epi_a(p, j, outps[j], sums)
                    state.append((outps[j], mv, t))
                return state

            pend_attn = None  # produce(p) awaiting consume
            pend_epi = None  # (p, state) awaiting epi_b
            for p in range(NPAIR):
                produced = produce(p)
                if pend_epi is not None:
                    ep, st = pend_epi
                    for j in (0, 1):
                        epi_b(ep, j, st[j][0], st[j][1], st[j][2], split=1)
                if pend_attn is not None:
                    pend_epi = (p - 1, consume_a(p - 1, pend_attn))
                pend_attn = produced
            ep, st = pend_epi
            for j in (0, 1):
                epi_b(ep, j, st[j][0], st[j][1], st[j][2], split=1)

            # last pair: accumulate into now-idle psA banks (no WAR against
            # the previous pair's psO epilogue reads), run the two q-chunks
            # back-to-back so chunk j=1's full epilogue+store overlaps chunk
            # j=0's matmuls, and column-halve j=0's accumulation so its
            # bn_stats mostly overlaps the final matmuls
            pl = NPAIR - 1
            attnT = pend_attn
            sums = sums_all[:, (pl % 2) * 2 : (pl % 2) * 2 + 2]
            lout1 = psA.tile([P, D], F32, tag="mm", name="lout1")
            consume_mm(pl, attnT, {1: lout1}, sums, 1)
            mv, t = epi_a(pl, 1, lout1, sums)
            epi_b(pl, 1, lout1, mv, t, split=2)

            # j=0 accumulates its two column halves into SEPARATE tiles:
            # tile-level dependency tracking would otherwise see the h0
            # bn_stats (emitted between the halves so it overlaps the h1
            # matmuls) as conflicting with the h1 writes and stall the PE
            lsums = sums_all[:, 4:5]  # untouched column: no tracked deps
            HB = D // 2
            louts = [
                psA.tile([P, HB], F32, tag="mm", name=f"l0h{h}") for h in (0, 1)
            ]
            bst2 = small.tile([P, 12], F32, tag="bst2")
            for h in (0, 1):
                cols = slice(h * HB, (h + 1) * HB)
                for kb in range(NBLK):
                    st = attnT[:, kb, 0:P]
                    nc.tensor.matmul(
                        louts[h],
                        st,
                        v_sb[:, kb, cols],
                        start=(kb == 0),
                        stop=(kb == NBLK - 1),
                    )
                    if h == 0:
                        nc.tensor.matmul(
                            lsums,
                            st,
                            ones_sb,
                            start=(kb == 0),
                            stop=(kb == NBLK - 1),
                        )
                if h == 0:
                    sc = small.tile([P, 1], F32, tag="sc")
                    nc.vector.tensor_copy(out=sc, in_=lsums)
                    t = small.tile([P, 1], F32, tag="t")
                    nc.vector.tensor_scalar(
                        out=t,
                        in0=sc,
                        scalar1=sc,
                        scalar2=float(EPS),
                        op0=Alu.mult,
                        op1=Alu.mult,
                    )
                nc.vector.bn_stats(out=bst2[:, h * 6 : (h + 1) * 6], in_=louts[h])
            mv = small.tile([P, 2], F32, tag="mv")
            nc.vector.bn_aggr(out=mv, in_=bst2)
            lnv = small.tile([P, 1], F32, tag="lnv")
            nc.scalar.activation(
                out=lnv, in_=mv[:, 1:2], func=Act.Ln, bias=t, scale=1.0
            )
            c1 = small.tile([P, 1], F32, tag="c1")
            nc.scalar.activation(out=c1, in_=lnv, func=Act.Exp, scale=-0.5)
            row = pl * 2 * P
            for h in (0, 1):
                cols = slice(h * HB, (h + 1) * HB)
                y = work.tile([P, HB], F32, tag=f"y{h}", name=f"ly{h}")
                nc.vector.tensor_scalar(
                    out=y,
                    in0=louts[h],
                    scalar1=mv[:, 0:1],
                    scalar2=c1,
                    op0=Alu.subtract,
                    op1=Alu.mult,
                )
                if not g1b0:
                    o1 = work.tile([P, HB], F32, tag=f"o1{h}", name=f"lo1{h}")
                    nc.vector.tensor_mul(o1, y, gamma_bc[:, cols])
                    y = work.tile([P, HB], F32, tag=f"o{h}", name=f"lo{h}")
                    nc.vector.tensor_add(y, o1, beta_bc[:, cols])
                eng = nc.scalar if h % 2 == 1 else nc.sync
                eng.dma_start(out=out_d.ap()[row : row + P, cols], in_=y)

    # Force every ACT instruction onto the one table set that contains all
    # functions we use ({exp, ln, identity} ⊆ natural_log_exp_and_others).
    # The default chooser picks the FIRST set containing each function
    # (exp→set0, ln→set5), inserting a 1.28us table reload twice per
    # chunk. Entries must keep their positions (act_func_set_id is the
    # index), so unwanted sets are emptied rather than removed.
    import concourse.bacc as bacc_mod

    orig_get_tables = bacc_mod.get_activation_tables

    def pinned_tables(arch):
        out = {}
        for name, funcs in orig_get_tables(arch).items():
            out[name] = funcs if name == "natural_log_exp_and_others" else set()
        return out

    bacc_mod.get_activation_tables = pinned_tables
    try:
        nc.compile()
    finally:
        bacc_mod.get_activation_tables = orig_get_tables
    return nc


def _numpy_fallback(query, mask, Wq, bq, Wk, bk, Wv, bv, gamma, beta):
    q = query @ Wq + bq
    k = query @ Wk + bk
    v = query @ Wv + bv
    scale = 1.0 / np.sqrt(np.float32(q.shape[-1]))
    logits = np.einsum("bqd,bkd->bqk", q, k) * scale
    m = np.swapaxes(mask, 1, 2)
    logits = np.where(m, logits, np.float32(-1e9))
    logits = logits - logits.max(axis=2, keepdims=True)
    attn = np.exp(logits)
    attn = attn / attn.sum(axis=2, keepdims=True)
    out = np.einsum("bqk,bkd->bqd", attn, v)
    mu = out.mean(axis=-1, keepdims=True)
    var = out.var(axis=-1, keepdims=True)
    return (out - mu) / np.sqrt(var + 1e-5) * gamma + beta


def kernel(query, mask, Wq, bq, Wk, bk, Wv, bv, gamma, beta):
    global last_results
    from concourse.bass_utils import run_bass_kernel_spmd

    query = np.asarray(query, dtype=np.float32)
    mask = np.asarray(mask)
    Wq = np.asarray(Wq, dtype=np.float32)
    Wk = np.asarray(Wk, dtype=np.float32)
    Wv = np.asarray(Wv, dtype=np.float32)
    bq = np.asarray(bq, dtype=np.float32)
    bk = np.asarray(bk, dtype=np.float32)
    bv = np.asarray(bv, dtype=np.float32)
    gamma = np.asarray(gamma, dtype=np.float32)
    beta = np.asarray(beta, dtype=np.float32)

    if not mask.all():
        # General-mask path (never hit for this problem's all-ones mask).
        return _numpy_fallback(
            query, mask, Wq, bq, Wk, bk, Wv, bv, gamma, beta
        ).astype(np.float32)

    g1b0 = bool((gamma == 1.0).all() and (beta == 0.0).all())
    if g1b0 not in _cached_nc:
        _cached_nc[g1b0] = _build_nc(g1b0)
    nc = _cached_nc[g1b0]

    scale = 1.0 / np.sqrt(np.float64(D))
    # fold the two q/k projections into one: logits = x @ m @ x^T + gk-term
    m_b = ((Wq.astype(np.float64) @ Wk.astype(np.float64).T) * scale).astype(BF)
    gk_b = ((Wk.astype(np.float64) @ bq.astype(np.float64)) * scale).astype(BF)
    wv_b = Wv.astype(BF)

    in_maps = []
    for b in range(B):
        m = {
            "xT": np.ascontiguousarray(query[b].T).astype(BF),
            "m": m_b,
            "gk": gk_b,
            "wv": wv_b,
            "bv": bv,
        }
        if not g1b0:
            m["gamma"] = gamma
            m["beta"] = beta
        in_maps.append(m)

    res = run_bass_kernel_spmd(nc, in_maps, core_ids=list(range(B)))
    last_results = res
    out = np.stack([res.results[b]["out"] for b in range(B)], axis=0)
    return out.astype(np.float32)


# revision 31
# speedup vs baseline: 1.2506x; 1.0013x over previous
"""Fused self-attention + LayerNorm kernel for Trainium2 (8 NeuronCores).

Problem: B=8, S=2048, D=512 dense transformer attention layer.
  q = x@Wq + bq; k = x@Wk + bk; v = x@Wv + bv
  logits = q @ k^T / sqrt(D); attn = softmax(logits)  (mask is all-ones)
  out = LayerNorm(attn @ v) * gamma + beta

Sharding: batch-data-parallel, one batch element per core, no collectives.

Per-core kernel (all matmuls bf16 with f32 PSUM accumulation):
  - host passes x pre-transposed (xT [D,S]) so no on-chip transposes of x
  - qT/kT computed directly in [D,S] layout (W as stationary operand);
    projections run seg-outer so the first 512-column slab of xT is enough
    to start the PE, with DMAs ordered/split to match (wq c-pieces, then
    xT seg-0 pieces, biases, wk, the rest of xT, wv)
  - logits computed TRANSPOSED, [k,q] per 128-k-block (stationary = kT
    block, moving = qT 256-column pair-chunk): exp(logitsT) is then
    directly the stationary operand of attn@v — no PE transposes at all
  - softmax row-sums via 1-row ones-matmuls sharing the attnT stationary
    (PE hwdecode makes the extra instructions ~free); normalization is
    folded into the LayerNorm epilogue analytically
  - attn@v accumulated over 16 k-blocks into one PSUM bank per 128-row
    q-chunk; exp on ACT; no max-subtraction (|logits| < ~2.5)
  - dummy PE matmuls during the initial DMA wait ramp the tensor engine
    to full clock before real work arrives
  - last pair runs its two q-chunks back-to-back (not interleaved) so the
    first chunk's epilogue+store overlaps the second chunk's matmuls, and
    the final store is column-quartered to pipeline DVE with DMA
"""

import sys

import numpy as np

_BASS_REPO = "/opt/trn_rl_repo"
if _BASS_REPO not in sys.path:
    sys.path.insert(0, _BASS_REPO)

import ml_dtypes  # noqa: E402

B, S, D = 8, 2048, 512
P = 128
NC_D = D // P  # 4 contraction chunks
SEG = 512
NSEG = S // SEG  # 4 free-dim segments
NBLK = S // P  # 16 k blocks
QP = 256  # q columns per produce (pair of 128-row chunks)
NPAIR = S // QP  # 8
EPS = 1e-5
BF = ml_dtypes.bfloat16
WARMUP_MM = 46  # dummy PE matmuls issued during the initial DMA wait

_cached_nc = {}
last_results = None  # BassKernelResults of the most recent run (for test.py)


def _build_nc(g1b0):
    import concourse.mybir as mybir
    from concourse import bacc
    from concourse.tile import TileContext

    BF16 = mybir.dt.bfloat16
    F32 = mybir.dt.float32
    Alu = mybir.AluOpType
    Act = mybir.ActivationFunctionType

    nc = bacc.Bacc("TRN2", target_bir_lowering=False, debug=False)

    xT_d = nc.declare_dram_parameter("xT", [D, S], BF16, isOutput=False)
    # m = Wq @ Wk^T / sqrt(D) folded on host: logits = x @ m @ x^T needs one
    # projection (u = x@m) instead of two (q and k). gk = Wk @ bq / sqrt(D)
    # carries the only softmax-relevant bias term (per-k, added pre-exp);
    # the q-bias term is constant per row and softmax-invariant, and the
    # epilogue's c1 form is invariant to the resulting rowsum rescale.
    m_d = nc.declare_dram_parameter("m", [D, D], BF16, isOutput=False)
    gk_d = nc.declare_dram_parameter("gk", [D], BF16, isOutput=False)
    wv_d = nc.declare_dram_parameter("wv", [D, D], BF16, isOutput=False)
    bv_d = nc.declare_dram_parameter("bv", [D], F32, isOutput=False)
    if not g1b0:
        gamma_d = nc.declare_dram_parameter("gamma", [D], F32, isOutput=False)
        beta_d = nc.declare_dram_parameter("beta", [D], F32, isOutput=False)
    out_d = nc.declare_dram_parameter("out", [S, D], F32, isOutput=True)

    import concourse.bass as bass

    def bcast(param_ap, parts=P):
        # [N] dram vector -> [parts, N] partition-broadcast AP
        return bass.AP(
            tensor=param_ap.tensor,
            offset=param_ap.offset,
            ap=[[0, parts]] + list(param_ap.ap),
        )

    with TileContext(nc) as tc:
        with (
            tc.tile_pool(name="pers", bufs=1) as pers,
            tc.tile_pool(name="attnp", bufs=2) as attnp,
            tc.tile_pool(name="work", bufs=4) as work,
            tc.tile_pool(name="small", bufs=6) as small,
            tc.tile_pool(name="psA", bufs=5, space="PSUM") as psA,
            tc.tile_pool(name="psO", bufs=2, space="PSUM") as psO,
            tc.tile_pool(name="psS", bufs=1, space="PSUM") as psS,
        ):
            # ---- persistent tiles ----
            w_sbs = {
                nm: pers.tile([P, NC_D, D], BF16, tag=nm, name=nm)
                for nm in ("m", "wv")
            }
            xT_sb = pers.tile([P, NC_D, S], BF16, tag="xT")

            # ---- input DMAs, ordered around the single HWDGE queue ----
            # Issue serializes at ~625ns/DMA and transfers serialize on the
            # DMA engines, so: few DMAs, ordered to match PE consumption.
            # M's first c-chunk, then all of xT seg 0 (one rearranged DMA),
            # then the rest of M, gk, remaining xT segs, wv.
            nc.sync.dma_start(out=w_sbs["m"][:, 0, :], in_=m_d.ap()[0:P, :])
            nc.sync.dma_start(
                out=xT_sb[:, :, 0:SEG],
                in_=xT_d.ap()[:, 0:SEG].rearrange("(c p) n -> p c n", p=P),
            )
            for c in range(1, NC_D):
                nc.sync.dma_start(
                    out=w_sbs["m"][:, c, :], in_=m_d.ap()[c * P : (c + 1) * P, :]
                )
            gk_sb = pers.tile([P, NC_D], BF16, tag="gk")
            nc.sync.dma_start(out=gk_sb, in_=gk_d.ap().rearrange("(c p) -> p c", p=P))
            for g in range(1, NSEG):
                nc.sync.dma_start(
                    out=xT_sb[:, :, g * SEG : (g + 1) * SEG],
                    in_=xT_d.ap()[:, g * SEG : (g + 1) * SEG].rearrange(
                        "(c p) n -> p c n", p=P
                    ),
                )
            nc.sync.dma_start(
                out=w_sbs["wv"], in_=wv_d.ap().rearrange("(c p) n -> p c n", p=P)
            )
            bv_bc = pers.tile([P, D], F32, tag="bv")
            nc.sync.dma_start(out=bv_bc, in_=bcast(bv_d.ap()))
            if not g1b0:
                gamma_bc = pers.tile([P, D], F32, tag="gamma")
                nc.sync.dma_start(out=gamma_bc, in_=bcast(gamma_d.ap()))
                beta_bc = pers.tile([P, D], F32, tag="beta")
                nc.sync.dma_start(out=beta_bc, in_=bcast(beta_d.ap()))

            # PE clock warmup: the tensor engine ramps to full speed only
            # after ~3us of continuous execution. Chew through dummy 128-row
            # matmuls on a zeroed tile while the first input DMAs land.
            # wz is memset on GPSIMD (idle, short preamble) so warmup can
            # start ~0.5us in instead of waiting out the DVE preamble.
            wz = pers.tile([P, P], BF16, tag="wz")
            nc.gpsimd.memset(wz, 0.0)
            eps_sb = pers.tile([P, 1], F32, tag="eps")
            nc.vector.memset(eps_sb, EPS)
            # module-init const, ready at t=0 with no engine dependency
            ones_sb = nc.const_aps.tensor(1.0, (P, 1), BF16)
            # dummy activation right at kernel start: pulls the one-time
            # 1.28us act-table load off the first exp eviction's critical
            # path — it runs concurrently with the input DMAs
            warm = pers.tile([P, 1], F32, tag="warm")
            nc.scalar.activation(out=warm, in_=eps_sb, func=Act.Exp)

            if WARMUP_MM:
                wps = psA.tile([P, SEG], F32, tag="mm", name="warmps")
                for _ in range(WARMUP_MM):
                    nc.tensor.matmul(wps[:, 0:P], wz, wz, start=True, stop=True)

            # ---- phase 1: u projection + gamma matvec, seg-outer ----
            # uT[d',s] (u = x@m): stationary = m chunk [d, d'-block],
            # moving = xT [d, s-seg]; accumulate over 4 d-chunks. seg-outer
            # so only xT's first 512 columns gate the start of compute.
            uT_sb = pers.tile([P, NC_D, S], BF16, tag="uT")
            for g in range(NSEG):
                pss = [
                    psA.tile([P, SEG], F32, tag="mm", name=f"pj{m}")
                    for m in range(NC_D)
                ]
                for c in range(NC_D):
                    for m in range(NC_D):
                        nc.tensor.matmul(
                            pss[m],
                            w_sbs["m"][:, c, m * P : (m + 1) * P],
                            xT_sb[:, c, g * SEG : (g + 1) * SEG],
                            start=(c == 0),
                            stop=(c == NC_D - 1),
                        )
                for m in range(NC_D):
                    # evict + cast to bf16; alternate ACT/DVE so two
                    # engines drain PSUM
                    if m % 2 == 0:
                        nc.scalar.activation(
                            out=uT_sb[:, m, g * SEG : (g + 1) * SEG],
                            in_=pss[m],
                            func=Act.Identity,
                        )
                    else:
                        nc.vector.tensor_copy(
                            out=uT_sb[:, m, g * SEG : (g + 1) * SEG],
                            in_=pss[m],
                        )
            # gamma[k] = x[k,:] @ gk: 1-row matmuls, ~free on the hwdecode
            # PE; added per-partition as the exp eviction's bias
            gam_sb = pers.tile([P, NBLK], F32, tag="gam")
            gps = psA.tile([P, SEG], F32, tag="mm", name="gps")
            for kb in range(NBLK):
                for c in range(NC_D):
                    nc.tensor.matmul(
                        gps[:, kb : kb + 1],
                        xT_sb[:, c, kb * P : (kb + 1) * P],
                        gk_sb[:, c : c + 1],
                        start=(c == 0),
                        stop=(c == NC_D - 1),
                    )
            nc.vector.tensor_copy(out=gam_sb, in_=gps[:, 0:NBLK])
            # v[s,d']: stationary = xT block [d, s-block], moving = Wv [d, d']
            v_sb = pers.tile([P, NBLK, D], BF16, tag="v")
            for j in range(NBLK):
                ps = psA.tile([P, SEG], F32, tag="mm", name="vps")
                for c in range(NC_D):
                    nc.tensor.matmul(
                        ps,
                        xT_sb[:, c, j * P : (j + 1) * P],
                        w_sbs["wv"][:, c, :],
                        start=(c == 0),
                        stop=(c == NC_D - 1),
                    )
                # evict + bias along free dim + cast
                nc.vector.tensor_add(v_sb[:, j, :], ps, bv_bc)

            # ---- phase 2: attention + layernorm, per 256-column q pair ----
            # Software-pipelined: produce pair p+1 (logitsT+exp) before
            # consuming pair p (attn@v + LN epilogue), so the PE never waits
            # on the ACT exp latency.
            def produce(p):
                # logitsT[k, q] per 128-k-block: stationary = xT block,
                # moving = uT pair-chunk. exp(logitsT) lands in attnT ready
                # to be the stationary operand of attn@v — no transposes.
                attnT = attnp.tile([P, NBLK, QP], BF16, tag="attnT")
                for kb in range(NBLK):
                    lg = psA.tile([P, SEG], F32, tag="mm", name=f"lg{kb % 5}")
                    for c in range(NC_D):
                        nc.tensor.matmul(
                            lg[:, 0:QP],
                            xT_sb[:, c, kb * P : (kb + 1) * P],
                            uT_sb[:, c, p * QP : (p + 1) * QP],
                            start=(c == 0),
                            stop=(c == NC_D - 1),
                        )
                    # no max subtraction (|logits| < ~2.5 for this problem);
                    # gamma carries the k-bias term (zero for zero bq)
                    nc.scalar.activation(
                        out=attnT[:, kb, :],
                        in_=lg[:, 0:QP],
                        func=Act.Exp,
                        bias=gam_sb[:, kb : kb + 1],
                        scale=1.0,
                    )
                return attnT

            # ---- epilogue, split in two stages ----
            # softmax normalization folded into LN:
            #   raw = attn_unnorm @ v; normalized x = raw / rowsum
            #   out = (raw - mean_raw) * c1 * gamma + beta, where
            #   c1 = (var_raw + eps*rowsum^2)^-0.5
            # (equals rstd(x)/rowsum analytically; eps*rowsum^2 keeps the
            # torch eps semantics). Stage A (DVE stats) is emitted with the
            # consume; stage B (ACT rsqrt via Exp(-0.5*Ln), final pass,
            # store) is deferred until after the NEXT produce so the ACT
            # FIFO never blocks that pair's exp evictions behind a
            # DVE-dependent Ln.
            def epi_a(p, j, out_ps, sums):
                sc = small.tile([P, 1], F32, tag="sc")
                nc.vector.tensor_copy(out=sc, in_=sums[:, j : j + 1])
                bst = small.tile([P, 6], F32, tag="bst")
                nc.vector.bn_stats(out=bst, in_=out_ps)
                mv = small.tile([P, 2], F32, tag="mv")
                nc.vector.bn_aggr(out=mv, in_=bst)
                t = small.tile([P, 1], F32, tag="t")
                nc.vector.tensor_scalar(
                    out=t,
                    in0=sc,
                    scalar1=sc,
                    scalar2=float(EPS),
                    op0=Alu.mult,
                    op1=Alu.mult,
                )
                return mv, t

            def epi_b(p, j, out_ps, mv, t, split, alt_queue=False):
                # rstd = (var + eps*s^2)^-0.5 as Exp(-0.5*Ln(.)) — the ACT
                # engine stays on the single ln+exp function table (a Sqrt
                # would force a 1.3us table reload twice per chunk)
                lnv = small.tile([P, 1], F32, tag="lnv")
                nc.scalar.activation(
                    out=lnv, in_=mv[:, 1:2], func=Act.Ln, bias=t, scale=1.0
                )
                c1 = small.tile([P, 1], F32, tag="c1")
                nc.scalar.activation(out=c1, in_=lnv, func=Act.Exp, scale=-0.5)

                row = (p * 2 + j) * P
                hw_ = D // split
                for h in range(split):
                    cols = slice(h * hw_, (h + 1) * hw_)
                    y = work.tile([P, hw_], F32, tag=f"y{h}", name=f"y{h}")
                    nc.vector.tensor_scalar(
                        out=y,
                        in0=out_ps[:, cols],
                        scalar1=mv[:, 0:1],
                        scalar2=c1,
                        op0=Alu.subtract,
                        op1=Alu.mult,
                    )
                    if not g1b0:
                        o1 = work.tile([P, hw_], F32, tag=f"o1{h}", name=f"o1{h}")
                        nc.vector.tensor_mul(o1, y, gamma_bc[:, cols])
                        y = work.tile([P, hw_], F32, tag=f"o{h}", name=f"o{h}")
                        nc.vector.tensor_add(y, o1, beta_bc[:, cols])
                    # alternate trigger queues on the tail so the final
                    # stores issue in parallel instead of serializing on SP
                    eng = nc.scalar if (alt_queue and h % 2 == 1) else nc.sync
                    eng.dma_start(out=out_d.ap()[row : row + P, cols], in_=y)

            # one persistent sums bank, column-region double-buffered by pair
            # parity so consecutive pairs' rowsum accumulations never share a
            # WAR dependency on the epilogue's read
            sums_all = psS.tile([P, 6], F32, tag="s")

            def consume_mm(p, attnT, outps, sums, j):
                for kb in range(NBLK):
                    st = attnT[:, kb, j * P : (j + 1) * P]
                    nc.tensor.matmul(
                        outps[j],
                        st,
                        v_sb[:, kb, :],
                        start=(kb == 0),
                        stop=(kb == NBLK - 1),
                    )
                    # 1-row matmul reusing the stationary: rowsum of the
                    # exact bf16 attn weights used above
                    nc.tensor.matmul(
                        sums[:, j : j + 1],
                        st,
                        ones_sb,
                        start=(kb == 0),
                        stop=(kb == NBLK - 1),
                    )

            def consume_a(p, attnT):
                outps = [
                    psO.tile([P, D], F32, tag="out", name=f"out{j}") for j in (0, 1)
                ]
                sums = sums_all[:, (p % 2) * 2 : (p % 2) * 2 + 2]
                for kb in range(NBLK):
                    for j in (0, 1):
                        st = attnT[:, kb, j * P : (j + 1) * P]
                        nc.tensor.matmul(
                            outps[j],
                            st,
                            v_sb[:, kb, :],
                            start=(kb == 0),
                            stop=(kb == NBLK - 1),
                        )
                        nc.tensor.matmul(
                            sums[:, j : j + 1],
                            st,
                            ones_sb,
                            start=(kb == 0),
                            stop=(kb == NBLK - 1),
                        )
                state = []
                for j in (0, 1):
                    mv, t = epi_a(p, j, outps[j], sums)
                    state.append((outps[j], mv, t))
                return state

            pend_attn = None  # produce(p) awaiting consume
            pend_epi = None  # (p, state) awaiting epi_b
            for p in range(NPAIR):
                produced = produce(p)
                if pend_epi is not None:
                    ep, st = pend_epi
                    for j in (0, 1):
                        epi_b(ep, j, st[j][0], st[j][1], st[j][2], split=1)
                if pend_attn is not None:
                    pend_epi = (p - 1, consume_a(p - 1, pend_attn))
                pend_attn = produced
            ep, st = pend_epi
            for j in (0, 1):
                epi_b(ep, j, st[j][0], st[j][1], st[j][2], split=1)

            # last pair: accumulate into now-idle psA banks (no WAR against
            # the previous pair's psO epilogue reads), run the two q-chunks
            # back-to-back so chunk j=1's full epilogue+store overlaps chunk
            # j=0's matmuls, and column-halve j=0's accumulation so its
            # bn_stats mostly overlaps the final matmuls
            pl = NPAIR - 1
            attnT = pend_attn
            sums = sums_all[:, (pl % 2) * 2 : (pl % 2) * 2 + 2]
            lout1 = psA.tile([P, D], F32, tag="mm", name="lout1")
            consume_mm(pl, attnT, {1: lout1}, sums, 1)
            mv, t = epi_a(pl, 1, lout1, sums)
            epi_b(pl, 1, lout1, mv, t, split=2)

            # j=0 accumulates its two column halves into SEPARATE tiles:
            # tile-level dependency tracking would otherwise see the h0
            # bn_stats (emitted between the halves so it overlaps the h1
            # matmuls) as conflicting with the h1 writes and stall the PE
            lsums = sums_all[:, 4:5]  # untouched column: no tracked deps
            # asymmetric halves: the small trailing piece minimizes the
            # post-PE bn_stats and the final store's transfer time
            HSPLIT = (slice(0, 384), slice(384, D))
            louts = [
                psA.tile([P, 384 if h == 0 else D - 384], F32, tag="mm", name=f"l0h{h}")
                for h in (0, 1)
            ]
            bst2 = small.tile([P, 12], F32, tag="bst2")
            for h in (0, 1):
                cols = HSPLIT[h]
                for kb in range(NBLK):
                    st = attnT[:, kb, 0:P]
                    nc.tensor.matmul(
                        louts[h],
                        st,
                        v_sb[:, kb, cols],
                        start=(kb == 0),
                        stop=(kb == NBLK - 1),
                    )
                    if h == 0:
                        nc.tensor.matmul(
                            lsums,
                            st,
                            ones_sb,
                            start=(kb == 0),
                            stop=(kb == NBLK - 1),
                        )
                if h == 0:
                    sc = small.tile([P, 1], F32, tag="sc")
                    nc.vector.tensor_copy(out=sc, in_=lsums)
                    t = small.tile([P, 1], F32, tag="t")
                    nc.vector.tensor_scalar(
                        out=t,
                        in0=sc,
                        scalar1=sc,
                        scalar2=float(EPS),
                        op0=Alu.mult,
                        op1=Alu.mult,
                    )
                nc.vector.bn_stats(out=bst2[:, h * 6 : (h + 1) * 6], in_=louts[h])
            mv = small.tile([P, 2], F32, tag="mv")
            nc.vector.bn_aggr(out=mv, in_=bst2)
            lnv = small.tile([P, 1], F32, tag="lnv")
            nc.scalar.activation(
                out=lnv, in_=mv[:, 1:2], func=Act.Ln, bias=t, scale=1.0
            )
            c1 = small.tile([P, 1], F32, tag="c1")
            nc.scalar.activation(out=c1, in_=lnv, func=Act.Exp, scale=-0.5)
            row = pl * 2 * P
            for h in (0, 1):
                cols = HSPLIT[h]
                hw_ = cols.stop - cols.start
                y = work.tile([P, hw_], F32, tag=f"y{h}", name=f"ly{h}")
                nc.vector.tensor_scalar(
                    out=y,
                    in0=louts[h],
                    scalar1=mv[:, 0:1],
                    scalar2=c1,
                    op0=Alu.subtract,
                    op1=Alu.mult,
                )
                if not g1b0:
                    o1 = work.tile([P, hw_], F32, tag=f"o1{h}", name=f"lo1{h}")
                    nc.vector.tensor_mul(o1, y, gamma_bc[:, cols])
                    y = work.tile([P, hw_], F32, tag=f"o{h}", name=f"lo{h}")
                    nc.vector.tensor_add(y, o1, beta_bc[:, cols])
                eng = nc.scalar if h % 2 == 1 else nc.sync
                eng.dma_start(out=out_d.ap()[row : row + P, cols], in_=y)

    # Force every ACT instruction onto the one table set that contains all
    # functions we use ({exp, ln, identity} ⊆ natural_log_exp_and_others).
    # The default chooser picks the FIRST set containing each function
    # (exp→set0, ln→set5), inserting a 1.28us table reload twice per
    # chunk. Entries must keep their positions (act_func_set_id is the
    # index), so unwanted sets are emptied rather than removed.
    import concourse.bacc as bacc_mod

    orig_get_tables = bacc_mod.get_activation_tables

    def pinned_tables(arch):
        out = {}
        for name, funcs in orig_get_tables(arch).items():
            out[name] = funcs if name == "natural_log_exp_and_others" else set()
        return out

    bacc_mod.get_activation_tables = pinned_tables
    try:
        nc.compile()
    finally:
        bacc_mod.get_activation_tables = orig_get_tables
    return nc


def _numpy_fallback(query, mask, Wq, bq, Wk, bk, Wv, bv, gamma, beta):
    q = query @ Wq + bq
    k = query @ Wk + bk
    v = query @ Wv + bv
    scale = 1.0 / np.sqrt(np.float32(q.shape[-1]))
    logits = np.einsum("bqd,bkd->bqk", q, k) * scale
    m = np.swapaxes(mask, 1, 2)
    logits = np.where(m, logits, np.float32(-1e9))
    logits = logits - logits.max(axis=2, keepdims=True)
    attn = np.exp(logits)
    attn = attn / attn.sum(axis=2, keepdims=True)
    out = np.einsum("bqk,bkd->bqd", attn, v)
    mu = out.mean(axis=-1, keepdims=True)
    var = out.var(axis=-1, keepdims=True)
    return (out - mu) / np.sqrt(var + 1e-5) * gamma + beta


def kernel(query, mask, Wq, bq, Wk, bk, Wv, bv, gamma, beta):
    global last_results
    from concourse.bass_utils import run_bass_kernel_spmd

    query = np.asarray(query, dtype=np.float32)
    mask = np.asarray(mask)
    Wq = np.asarray(Wq, dtype=np.float32)
    Wk = np.asarray(Wk, dtype=np.float32)
    Wv = np.asarray(Wv, dtype=np.float32)
    bq = np.asarray(bq, dtype=np.float32)
    bk = np.asarray(bk, dtype=np.float32)
    bv = np.asarray(bv, dtype=np.float32)
    gamma = np.asarray(gamma, dtype=np.float32)
    beta = np.asarray(beta, dtype=np.float32)

    if not mask.all():
        # General-mask path (never hit for this problem's all-ones mask).
        return _numpy_fallback(
            query, mask, Wq, bq, Wk, bk, Wv, bv, gamma, beta
        ).astype(np.float32)

    g1b0 = bool((gamma == 1.0).all() and (beta == 0.0).all())
    if g1b0 not in _cached_nc:
        _cached_nc[g1b0] = _build_nc(g1b0)
    nc = _cached_nc[g1b0]

    scale = 1.0 / np.sqrt(np.float64(D))
    # fold the two q/k projections into one: logits = x @ m @ x^T + gk-term
    m_b = ((Wq.astype(np.float64) @ Wk.astype(np.float64).T) * scale).astype(BF)
    gk_b = ((Wk.astype(np.float64) @ bq.astype(np.float64)) * scale).astype(BF)
    wv_b = Wv.astype(BF)

    in_maps = []
    for b in range(B):
        m = {
            "xT": np.ascontiguousarray(query[b].T).astype(BF),
            "m": m_b,
            "gk": gk_b,
            "wv": wv_b,
            "bv": bv,
        }
        if not g1b0:
            m["gamma"] = gamma
            m["beta"] = beta
        in_maps.append(m)

    res = run_bass_kernel_spmd(nc, in_maps, core_ids=list(range(B)))
    last_results = res
    out = np.stack([res.results[b]["out"] for b in range(B)], axis=0)
    return out.astype(np.float32)
